# revision 27
# baseline (speedup 1.0000x reference)
"""Trainium2 Bass kernel for the ContinuousRNN problem.

Reference (per batch row b):
    h_0 = 0                               # [N], N=100
    z_t = W_rec h_t + W_in u_t
    h_{t+1} = 0.85 h_t + 0.15 tanh(z_t) + NOISE_STD noise_t
    out_t = W_out h_{t+1}

z-space reformulation (state z_t, N rows):
    z_{t+1} = 0.85 z_t + Mz (th_t + v_t)
      Mz   = 0.15 [W_rec, W_in]                          (100x103)
      th_t = [tanh(z_t) ; 0]
      v_t  = [noise_t*NS/DT ; (u_{t+1}-0.85 u_t)/DT]
    bootstrap (h=0): z_0 = Mz [0 ; u_0/DT] = W_in u_0
    out_t = 0.85 out_{t-1} + 0.15 W_out (tanh(z_t) + noise_t*NS/DT)
    (the out IIR is a linear readout of the device-produced tanh
    stream; it runs on host, exactly mirroring the device recurrence)

Per-core per-step critical path is two hops:
    MM_t -> { ACT tanh (psum->sbuf fp16)  ||  DVE prep z' = 0.85 z + C } -> MM_{t+1}
where MM_{t+1} accumulates (start=False) onto the DVE-prepped psum slot.
That accumulation works because each z bank is primed once by a
start=True matmul (sets the psum has_written bits, which non-PE writes
do not clear).  C_t = Mz v_t is host-precomputed and streamed.  The PE
never reloads weights (redundant LDWEIGHTS are deduped by a post-tile
pass).  The tanh stream drains from SBUF by DMA.

The Tile framework tracks PSUM deps as a single linear chain per tile
(each accessor waits for the previous accessor), which would serialize
ACT and DVE (both read the same z slot).  A post-pass
(_parallelize_act_stt) removes those reader-after-reader edges so ACT
and DVE truly run in parallel.

Sharding: each core runs `groups` independent chains of 128 batch
columns; chains are (batch-block, time-shard) pairs.  With 8 cores and
n chains/core there are 2n time shards over 4 batch blocks.  The RNN
contracts (~0.983/step), so time shards s>0 warm up for L steps from
h=0; shard 0 "warms up" on zero-padded inputs (exactly h=0).  All cores
run the identical SPMD program; host slices each chain's valid range.
"""

import sys

for _p in ("/opt/trn_rl_repo",):
    if _p not in sys.path:
        sys.path.insert(0, _p)

import numpy as np

import concourse.bass as bass
import concourse.bacc as bacc
import concourse.mybir as mybir
from concourse import tile
from concourse.bass_utils import run_bass_kernel_spmd

F32 = mybir.dt.float32
F16 = mybir.dt.float16

N = 100
NB = 3
K = N + NB        # 103 (matmul contraction: tanh rows + zero-padded u rows)
B = 512
T = 2048
NCORES = 8
DT = np.float32(0.15)
NOISE_STD = np.float32(0.015)
DECAY = np.float32(0.85)

GW = 128                # batch columns per chain
L_WARM = 240            # warmup steps for time shards > 0

NQ = 8                  # z slots (2 parity tiles x 4 slots per chain)
NTH = 16                # th ring slots
DRAIN = 8               # drain period (iters)


def plan_for(groups):
    """Shard plan for `groups` chains/core: S time shards, per-shard
    (start, warm, r), uniform ITERS."""
    S = 2 * groups
    A = -(-(T + (S - 1) * L_WARM) // S)     # ceil
    rs = [A] + [A - L_WARM] * (S - 1)
    excess = sum(rs) - T
    rs[-1] -= excess
    assert rs[-1] > 0
    plan = []
    s = 0
    for j in range(S):
        plan.append((s, 0 if j == 0 else L_WARM, rs[j]))
        s += rs[j]
    assert s == T
    # ITERS = A + 2: one bootstrap iteration up front, plus one trailing
    # iteration so the last output's o-state gets tanh'd and drained
    # (the drained o at iteration k is the out IIR of iteration k-1).
    return plan, A + 2


def emit_scan(tc, nc, aps, *, iters, cols, groups, ch, filler=0, cut=1):
    """aps: m_mat [K,M] f16 (lhsT, M=K with the 3 readout columns),
    c_t [M, iters*cols] f16 (iteration-major, host-computed C = Mz v),
    o_out [NB, iters*cols] f16 (the tanh-encoded out IIR).

    The stationary's columns 100:103 compute the out IIR readout
    p = DT*W_out*th into psum rows 100:103 of the same z slot; the STT
    applies the same 0.85 decay (out IIR == z decay), and the per-step
    tanh ACT covers rows 0:103, so tanh(out_t) rides the tht ring for
    free (tanh is invertible; host applies atanh).  The stationary's
    u-rows are zeroed so the tanh(o) rhs rows contribute nothing.

    PSUM dependency tracking is per-tile, so z state is split into
    per-(group x parity) psum tiles.  C streams through SBUF (walrus
    rejects TensorScalarPtr with all-PSUM operands, and SBUF tiles get
    fine-grained dep tracking)."""
    gw = cols // groups
    mult = mybir.AluOpType.mult
    add = mybir.AluOpType.add
    tanh = mybir.ActivationFunctionType.Tanh
    hq = NQ // 2           # z slots per parity tile

    cpool = tc.alloc_tile_pool(name="const", bufs=1)
    vpool = tc.alloc_tile_pool(name="cstream", bufs=2)
    tpool = tc.alloc_tile_pool(name="th", bufs=1)
    ppool = tc.alloc_tile_pool(name="psum", bufs=1, space="PSUM")

    wb = cpool.tile([K, K], F16, name="wb")
    nc.sync.dma_start(wb[:, :], aps["m_mat"][:, :])

    zt = cpool.tile([K, 512], F16, name="zt")   # zero rhs for priming
    nc.vector.memset(zt[:, :], 0.0)

    # z state: per (group, parity) psum tiles, hq slots of [K, gw] each
    qts = [[ppool.tile([128, hq * gw], F32, name=f"qt{g}p{par}")
            for par in range(2)] for g in range(groups)]
    # scratch bank for PE-warming filler matmuls
    fts = ppool.tile([128, 512], F32, name="fts") if filler else None

    # th ring (fp16); rows 0:100 tanh(z), rows 100:103 tanh(out IIR)
    tht = tpool.tile([K, NTH * cols], F16, name="tht")
    nc.vector.memset(tht[96:K, :], 0.0)

    # prime z tiles: start=True matmuls set has_written over all z slots
    for g in range(groups):
        for par in range(2):
            w = hq * gw
            assert w <= 512
            nc.tensor.matmul(qts[g][par][0:K, 0:w], wb[:, :],
                             zt[:, 0:w], start=True, stop=True)

    prev_mm = [None] * groups
    # C staging, double buffered
    ctiles = {}

    def c_chunk(ci):
        if ci * ch >= iters:
            return None
        if ci not in ctiles:
            tl = vpool.tile([K, ch * cols], F16, tag="cs", name=f"cs{ci}")
            hi = min((ci + 1) * ch, iters)
            nc.sync.dma_start(tl[:, 0:(hi - ci * ch) * cols],
                              aps["c_t"][:, ci * ch * cols:hi * cols])
            ctiles[ci] = tl
        return ctiles[ci]

    c_chunk(0)

    def drain(k_lo, k_hi):
        """DMA the tanh(out) rows for iterations k_lo..k_hi (inclusive,
        contiguous in the ring) to DRAM."""
        c0 = (k_lo % NTH) * cols
        c1 = c0 + (k_hi - k_lo + 1) * cols
        nc.sync.dma_start(aps["o_out"][:, k_lo * cols:k_lo * cols + c1 - c0],
                          tht[N:K, c0:c1])

    for k in range(iters):
        ci = k // ch
        if k % ch == 0:
            c_chunk(ci + 1)
        cc = (k % ch) * cols
        ctile = ctiles[ci]

        qs = ((k // 2) % hq) * gw       # read slot col (parity k%2)
        qn = (((k + 1) // 2) % hq) * gw  # write slot col (parity (k+1)%2)
        tc0 = (k % NTH) * cols
        for g in range(groups):
            rd = qts[g][k % 2]
            wr = qts[g][(k + 1) % 2]
            # ACT: th = tanh(z | o) psum -> sbuf fp16
            act_inst = nc.scalar.activation(
                tht[0:K, tc0 + g * gw:tc0 + (g + 1) * gw],
                rd[0:K, qs:qs + gw], tanh)
            # DVE prep: z' = 0.85 z + C  (psum+sbuf -> psum, other parity)
            stt_inst = nc.vector.scalar_tensor_tensor(
                wr[0:K, qn:qn + gw], rd[0:K, qs:qs + gw],
                float(DECAY), ctile[0:K, cc + g * gw:cc + (g + 1) * gw],
                mult, add)
            # The tile framework keeps a single linear dependency chain
            # per PSUM tile, so the STT would wait for the same-step
            # tanh even though both only READ the z slot.  Cut that
            # reader-after-reader edge (inline: sem waits are assigned
            # from these edges at TileContext exit), inheriting the
            # ACT's own deps (the producing matmul).
            if cut == 1:
                # cut the edge, inherit the ACT's own deps
                ai, si = act_inst.ins, stt_inst.ins
                for tname, _info in list(si.sync_dependencies()):
                    if tname == ai.name:
                        si.remove_dependency(tname)
                        si.merge_dependencies_from(ai)
            elif cut == 2:
                # cut the edge, add a clean edge to the producing matmul
                ai, si = act_inst.ins, stt_inst.ins
                for tname, info in list(si.sync_dependencies()):
                    if tname == ai.name:
                        si.remove_dependency(tname)
                        if prev_mm[g] is not None:
                            si.add_dependency(prev_mm[g].ins.name, info)
            # chain MM accumulates onto the prepped slot
            prev_mm[g] = nc.tensor.matmul(
                wr[0:K, qn:qn + gw], wb[:, :],
                tht[0:K, tc0 + g * gw:tc0 + (g + 1) * gw],
                start=False, stop=True, skip_group_check=True)
        if filler:
            # keep the PE pipeline warm with a throwaway matmul
            nc.tensor.matmul(fts[0:K, 0:filler], wb[:, :], zt[:, 0:filler],
                             start=True, stop=True)

        if k % DRAIN == DRAIN - 1:
            drain(k - DRAIN + 1, k)
    # tail
    rem = iters % DRAIN
    if rem:
        drain(iters - rem, iters - 1)

    for p in (ppool, tpool, vpool, cpool):
        p.release()


def _dedup_ldweights(nc):
    """Remove legalizer-inserted LDWEIGHTS that reload an identical
    stationary; merge their deps into the following matmul."""
    removed = 0
    for f in nc.m.functions:
        for blk in f.blocks:
            insts = list(blk.instructions)
            last_key = None
            keep = []
            pending = []
            for inst in insts:
                nm = type(inst).__name__
                if nm == "InstLdweights":
                    key = (str(inst.ins[0]), str(inst.tile_position),
                           str(inst.perf_mode), bool(inst.is_transpose))
                    if key == last_key:
                        pending.append(inst)
                        removed += 1
                        continue
                    last_key = key
                    keep.append(inst)
                elif nm == "InstMatmult":
                    for ld in pending:
                        inst.merge_dependencies_from(ld)
                    pending = []
                    keep.append(inst)
                else:
                    keep.append(inst)
            assert not pending, "dangling removed LDWEIGHTS"
            if len(keep) != len(insts):
                blk.instructions = keep
    return removed


def _parallelize_act_stt(nc):
    """The Tile framework keeps a single linear dependency chain per
    PSUM tile, so the per-step DVE prep (STT) waits for the same-step
    tanh (ACT) even though both only READ the z slot.  Remove each
    STT -> ACT edge where the ACT's input AP equals the STT's in0 AP
    (reader-after-reader on the same slot), merging the ACT's own deps
    (the producing matmul) into the STT."""
    removed = 0
    for f in nc.m.functions:
        for blk in f.blocks:
            by_name = {}
            for inst in blk.instructions:
                by_name[inst.name] = inst
            for inst in blk.instructions:
                if type(inst).__name__ != "InstTensorScalarPtr":
                    continue
                src = str(inst.ins[0])
                for tname, _info in list(inst.sync_dependencies()):
                    dep = by_name.get(tname)
                    if dep is None or type(dep).__name__ != "InstActivation":
                        continue
                    if str(dep.ins[0]) != src:
                        continue
                    inst.remove_dependency(tname)
                    inst.merge_dependencies_from(dep)
                    removed += 1
    return removed


def build_nc(*, iters, cols, groups=3, ch=64, dedup=True,
             filler=0, cut=1, num_devices=NCORES):
    nc = bacc.Bacc("TRN2", target_bir_lowering=False, debug=False,
                   num_devices=num_devices)
    # Row stride must not be 8KB-aligned: DMA packets are spread over
    # engines by a source-address hash, and 2^k-aligned row strides make
    # every row hash to the SAME engine (observed: 103-row chunk loads
    # serialized onto one engine at ~31GB/s).  An odd element count
    # cycles row starts through all hash buckets.
    lp = iters * cols + 63
    aps = {
        "m_mat": nc.dram_tensor("m_mat", [K, K], F16,
                                kind="ExternalInput").ap(),
        "c_t": nc.dram_tensor("c_t", [K, lp], F16,
                              kind="ExternalInput").ap(),
        "o_out": nc.dram_tensor("o_out", [NB, lp], F16,
                                kind="ExternalOutput").ap(),
    }
    with tile.TileContext(nc) as tcx:
        emit_scan(tcx, nc, aps, iters=iters, cols=cols, groups=groups, ch=ch,
                  filler=filler, cut=cut)
    if dedup:
        _dedup_ldweights(nc)
        # with a single resident stationary, moving waits onto the one
        # surviving LDWEIGHTS would be wrong — keep waits on matmuls
        nc.move_matmul_waits_to_ldweights = lambda: None
    nc.compile()
    return nc


def make_m_mat(recurrent_weights, input_weights, output_weights):
    """Device stationary, lhsT [K, K]: columns 0:100 produce the z
    update from the tanh rows (u-rows zeroed: the tanh(o) rhs rows must
    contribute nothing — the u drive arrives via the host C stream);
    columns 100:103 produce the out-IIR increment DT*W_out*th."""
    m = np.zeros((K, K), np.float32)
    m[:N, :N] = DT * recurrent_weights.T
    m[:N, N:] = DT * output_weights.T
    return np.ascontiguousarray(m).astype(np.float16)


def make_mz_host(recurrent_weights, input_weights, output_weights):
    """Host-side C projection [K, K]: rows 0:100 = DT*[W_rec, W_in]
    (the u path lives here), rows 100:103 = DT*[W_out, 0] (the readout
    noise drive)."""
    mz = np.zeros((K, K), np.float32)
    mz[:N, :N] = DT * recurrent_weights
    mz[:N, N:] = DT * input_weights
    mz[N:, :N] = DT * output_weights
    return mz


def make_v(inputs, noise, *, s, warm, iters, cols):
    """v stream [K, iters, cols] f32 for one chain (time shard).

    inputs [cols, T, NB], noise [cols, T, N] (batch-block slices).
    Iteration k=0 is the bootstrap block [0 ; u_{s-warm}/DT]; iteration
    k>=1 covers global step g = s - warm + k - 1 (g<0 -> zeros)."""
    v = np.zeros((K, iters, cols), np.float32)
    g0 = s - warm
    if 0 <= g0 < T:
        v[N:, 0] = inputs[:, g0].T / DT
    for k in range(1, iters):
        g = g0 + k - 1
        if g < 0 or g >= T:
            continue
        v[:N, k] = noise[:, g].T * (NOISE_STD / DT)
        un = inputs[:, g + 1].T if g + 1 < T else 0.0
        v[N:, k] = (un - DECAY * inputs[:, g].T) / DT
    return v


def make_c(v, mz):
    """Host C = Mz_aug v, fp16, [K, iters*cols + 63] (padded rows to
    keep the DRAM row stride off the DMA engine-hash pathology)."""
    k_, it, cols = v.shape
    c = np.zeros((K, it * cols + 63), np.float16)
    c[:, :it * cols] = (mz @ v.reshape(K, it * cols)).astype(np.float16)
    return c


def make_in_maps(inputs, noise, recurrent_weights, input_weights,
                 output_weights, *, groups, iters, plan):
    """Per-core input maps.  Core i runs chains i*groups..(i+1)*groups-1;
    chain q = (bblock, shard) = divmod(q, 2*groups).  The c stream per
    core interleaves its chains' 128-col blocks within each iteration."""
    m = make_m_mat(recurrent_weights, input_weights, output_weights)
    mz = make_mz_host(recurrent_weights, input_weights, output_weights)
    in_maps = []
    for core in range(NCORES):
        vs = []
        for g in range(groups):
            q = core * groups + g
            bb, sh = divmod(q, 2 * groups)
            bsl = slice(bb * GW, (bb + 1) * GW)
            ui = np.ascontiguousarray(inputs[bsl]).astype(np.float32)
            nz = np.ascontiguousarray(noise[bsl]).astype(np.float32)
            s, warm, r = plan[sh]
            vs.append(make_v(ui, nz, s=s, warm=warm, iters=iters, cols=GW))
        v = np.concatenate([vv[:, :, None, :] for vv in vs], axis=2)
        v = v.reshape(K, iters, groups * GW)
        in_maps.append({"m_mat": m, "c_t": make_c(v, mz)})
    return in_maps


def gather_out(results, *, groups, iters, plan):
    """Decode the tanh-encoded device out IIR: out = atanh(o_out)."""
    out = np.empty((B, T, NB), np.float32)
    cols = groups * GW
    for core in range(NCORES):
        o = results[core]["o_out"][:, :iters * cols].astype(np.float32)
        o = np.arctanh(np.clip(o, -0.999999, 0.999999))
        o = o.reshape(NB, iters, groups, GW)
        for g in range(groups):
            q = core * groups + g
            bb, sh = divmod(q, 2 * groups)
            bsl = slice(bb * GW, (bb + 1) * GW)
            s, warm, r = plan[sh]
            # o[:, k] = out IIR of iteration k-1 = global step s-warm+k-2
            out[bsl, s:s + r] = o[:, warm + 2:warm + 2 + r, g].transpose(2, 1, 0)
    return out


_NC_CACHE = {}


def kernel(inputs, noise, recurrent_weights, input_weights, output_weights,
           **run_kwargs):
    cfg = dict(run_kwargs.pop("cfg", {"filler": 0}))
    groups = cfg.setdefault("groups", 3)
    plan, iters = plan_for(groups)
    cfg.setdefault("iters", iters)
    cfg.setdefault("cols", groups * GW)
    key = tuple(sorted(cfg.items()))
    if key not in _NC_CACHE:
        _NC_CACHE[key] = build_nc(**cfg)
    nc = _NC_CACHE[key]
    in_maps = make_in_maps(inputs, noise, recurrent_weights,
                           input_weights, output_weights, groups=groups,
                           iters=cfg["iters"], plan=plan)
    res = run_bass_kernel_spmd(nc, in_maps, core_ids=list(range(NCORES)),
                               **run_kwargs)
    out = gather_out(res.results, groups=groups,
                     iters=cfg["iters"], plan=plan)
    if run_kwargs.get("trace"):
        return out, res
    return out


# revision 28
# speedup vs baseline: 2.6877x; 2.6877x over previous
"""Trainium2 Bass kernel for the ContinuousRNN problem.

Reference (per batch row b):
    h_0 = 0                               # [N], N=100
    z_t = W_rec h_t + W_in u_t
    h_{t+1} = 0.85 h_t + 0.15 tanh(z_t) + NOISE_STD noise_t
    out_t = W_out h_{t+1}

z-space reformulation (state z_t, N rows):
    z_{t+1} = 0.85 z_t + Mz (th_t + v_t)
      Mz   = 0.15 [W_rec, W_in]                          (100x103)
      th_t = [tanh(z_t) ; 0]
      v_t  = [noise_t*NS/DT ; (u_{t+1}-0.85 u_t)/DT]
    bootstrap (h=0): z_0 = Mz [0 ; u_0/DT] = W_in u_0
    out_t = 0.85 out_{t-1} + 0.15 W_out (tanh(z_t) + noise_t*NS/DT)
    (the out IIR is a linear readout of the device-produced tanh
    stream; it runs on host, exactly mirroring the device recurrence)

Per-core per-step critical path is two hops:
    MM_t -> { ACT tanh (psum->sbuf fp16)  ||  DVE prep z' = 0.85 z + C } -> MM_{t+1}
where MM_{t+1} accumulates (start=False) onto the DVE-prepped psum slot.
That accumulation works because each z bank is primed once by a
start=True matmul (sets the psum has_written bits, which non-PE writes
do not clear).  C_t = Mz v_t is host-precomputed and streamed.  The PE
never reloads weights (redundant LDWEIGHTS are deduped by a post-tile
pass).  The tanh stream drains from SBUF by DMA.

The Tile framework tracks PSUM deps as a single linear chain per tile
(each accessor waits for the previous accessor), which would serialize
ACT and DVE (both read the same z slot).  A post-pass
(_parallelize_act_stt) removes those reader-after-reader edges so ACT
and DVE truly run in parallel.

Sharding: each core runs `groups` independent chains of 128 batch
columns; chains are (batch-block, time-shard) pairs.  With 8 cores and
n chains/core there are 2n time shards over 4 batch blocks.  The RNN
contracts (~0.983/step), so time shards s>0 warm up for L steps from
h=0; shard 0 "warms up" on zero-padded inputs (exactly h=0).  All cores
run the identical SPMD program; host slices each chain's valid range.
"""

import sys

for _p in ("/opt/trn_rl_repo",):
    if _p not in sys.path:
        sys.path.insert(0, _p)

import numpy as np

import concourse.bass as bass
import concourse.bacc as bacc
import concourse.mybir as mybir
from concourse import tile
from concourse.bass_utils import run_bass_kernel_spmd

F32 = mybir.dt.float32
F16 = mybir.dt.float16

N = 100
NB = 3
K = N + NB        # 103 (matmul contraction: tanh rows + zero-padded u rows)
B = 512
T = 2048
NCORES = 8
DT = np.float32(0.15)
NOISE_STD = np.float32(0.015)
DECAY = np.float32(0.85)

GW = 128                # batch columns per chain
L_WARM = 240            # warmup steps for time shards > 0

NQ = 8                  # z slots (2 parity tiles x 4 slots per chain)
NTH = 16                # th ring slots
DRAIN = 8               # drain period (iters)


def plan_for(groups):
    """Shard plan for `groups` chains/core: S time shards, per-shard
    (start, warm, r), uniform ITERS."""
    S = 2 * groups
    A = -(-(T + (S - 1) * L_WARM) // S)     # ceil
    rs = [A] + [A - L_WARM] * (S - 1)
    excess = sum(rs) - T
    rs[-1] -= excess
    assert rs[-1] > 0
    plan = []
    s = 0
    for j in range(S):
        plan.append((s, 0 if j == 0 else L_WARM, rs[j]))
        s += rs[j]
    assert s == T
    # ITERS = A + 2: one bootstrap iteration up front, plus one trailing
    # iteration so the last output's o-state gets tanh'd and drained
    # (the drained o at iteration k is the out IIR of iteration k-1).
    return plan, A + 2


def emit_scan(tc, nc, aps, *, iters, cols, groups, ch, filler=0, cut=1):
    """aps: m_mat [K,M] f16 (lhsT, M=K with the 3 readout columns),
    c_t [M, iters*cols] f16 (iteration-major, host-computed C = Mz v),
    o_out [NB, iters*cols] f16 (the tanh-encoded out IIR).

    The stationary's columns 100:103 compute the out IIR readout
    p = DT*W_out*th into psum rows 100:103 of the same z slot; the STT
    applies the same 0.85 decay (out IIR == z decay), and the per-step
    tanh ACT covers rows 0:103, so tanh(out_t) rides the tht ring for
    free (tanh is invertible; host applies atanh).  The stationary's
    u-rows are zeroed so the tanh(o) rhs rows contribute nothing.

    PSUM dependency tracking is per-tile, so z state is split into
    per-(group x parity) psum tiles.  C streams through SBUF (walrus
    rejects TensorScalarPtr with all-PSUM operands, and SBUF tiles get
    fine-grained dep tracking)."""
    gw = cols // groups
    mult = mybir.AluOpType.mult
    add = mybir.AluOpType.add
    tanh = mybir.ActivationFunctionType.Tanh
    hq = NQ // 2           # z slots per parity tile

    cpool = tc.alloc_tile_pool(name="const", bufs=1)
    vpool = tc.alloc_tile_pool(name="cstream", bufs=2)
    tpool = tc.alloc_tile_pool(name="th", bufs=1)
    ppool = tc.alloc_tile_pool(name="psum", bufs=1, space="PSUM")

    wb = cpool.tile([K, K], F16, name="wb")
    nc.sync.dma_start(wb[:, :], aps["m_mat"][:, :])

    zt = cpool.tile([K, 512], F16, name="zt")   # zero rhs for priming
    nc.vector.memset(zt[:, :], 0.0)

    # z state: per (group, parity) psum tiles, hq slots of [K, gw] each
    qts = [[ppool.tile([128, hq * gw], F32, name=f"qt{g}p{par}")
            for par in range(2)] for g in range(groups)]
    # scratch bank for PE-warming filler matmuls
    fts = ppool.tile([128, 512], F32, name="fts") if filler else None

    # th ring (fp16); rows 0:100 tanh(z), rows 100:103 tanh(out IIR)
    tht = tpool.tile([K, NTH * cols], F16, name="tht")
    nc.vector.memset(tht[96:K, :], 0.0)

    # prime z tiles: start=True matmuls set has_written over all z slots
    for g in range(groups):
        for par in range(2):
            w = hq * gw
            assert w <= 512
            nc.tensor.matmul(qts[g][par][0:K, 0:w], wb[:, :],
                             zt[:, 0:w], start=True, stop=True)

    prev_mm = [None] * groups
    # C staging, double buffered
    ctiles = {}

    def c_chunk(ci):
        if ci * ch >= iters:
            return None
        if ci not in ctiles:
            tl = vpool.tile([K, ch * cols], F16, tag="cs", name=f"cs{ci}")
            hi = min((ci + 1) * ch, iters)
            w = (hi - ci * ch) * cols
            # split rows 0:100 / 100:103: the HWDGE spreads a DMA's
            # rows across engines only for round row counts (100 -> 10
            # engines, 3 -> 3); a 103-row DMA lands on ONE engine and
            # serializes the whole chunk load at ~31GB/s.
            nc.sync.dma_start(tl[0:N, 0:w],
                              aps["c_t"][0:N, ci * ch * cols:hi * cols])
            nc.sync.dma_start(tl[N:K, 0:w],
                              aps["c_t"][N:K, ci * ch * cols:hi * cols])
            ctiles[ci] = tl
        return ctiles[ci]

    c_chunk(0)

    def drain(k_lo, k_hi):
        """DMA the tanh(out) rows for iterations k_lo..k_hi (inclusive,
        contiguous in the ring) to DRAM."""
        c0 = (k_lo % NTH) * cols
        c1 = c0 + (k_hi - k_lo + 1) * cols
        nc.sync.dma_start(aps["o_out"][:, k_lo * cols:k_lo * cols + c1 - c0],
                          tht[N:K, c0:c1])

    for k in range(iters):
        ci = k // ch
        if k % ch == 0:
            c_chunk(ci + 1)
        cc = (k % ch) * cols
        ctile = ctiles[ci]

        qs = ((k // 2) % hq) * gw       # read slot col (parity k%2)
        qn = (((k + 1) // 2) % hq) * gw  # write slot col (parity (k+1)%2)
        tc0 = (k % NTH) * cols
        for g in range(groups):
            rd = qts[g][k % 2]
            wr = qts[g][(k + 1) % 2]
            # ACT: th = tanh(z | o) psum -> sbuf fp16
            act_inst = nc.scalar.activation(
                tht[0:K, tc0 + g * gw:tc0 + (g + 1) * gw],
                rd[0:K, qs:qs + gw], tanh)
            # DVE prep: z' = 0.85 z + C  (psum+sbuf -> psum, other parity)
            stt_inst = nc.vector.scalar_tensor_tensor(
                wr[0:K, qn:qn + gw], rd[0:K, qs:qs + gw],
                float(DECAY), ctile[0:K, cc + g * gw:cc + (g + 1) * gw],
                mult, add)
            # The tile framework keeps a single linear dependency chain
            # per PSUM tile, so the STT would wait for the same-step
            # tanh even though both only READ the z slot.  Cut that
            # reader-after-reader edge (inline: sem waits are assigned
            # from these edges at TileContext exit), inheriting the
            # ACT's own deps (the producing matmul).
            if cut == 1:
                # cut the edge, inherit the ACT's own deps
                ai, si = act_inst.ins, stt_inst.ins
                for tname, _info in list(si.sync_dependencies()):
                    if tname == ai.name:
                        si.remove_dependency(tname)
                        si.merge_dependencies_from(ai)
            elif cut == 2:
                # cut the edge, add a clean edge to the producing matmul
                ai, si = act_inst.ins, stt_inst.ins
                for tname, info in list(si.sync_dependencies()):
                    if tname == ai.name:
                        si.remove_dependency(tname)
                        if prev_mm[g] is not None:
                            si.add_dependency(prev_mm[g].ins.name, info)
            # chain MM accumulates onto the prepped slot
            prev_mm[g] = nc.tensor.matmul(
                wr[0:K, qn:qn + gw], wb[:, :],
                tht[0:K, tc0 + g * gw:tc0 + (g + 1) * gw],
                start=False, stop=True, skip_group_check=True)
        if filler:
            # keep the PE pipeline warm with a throwaway matmul
            nc.tensor.matmul(fts[0:K, 0:filler], wb[:, :], zt[:, 0:filler],
                             start=True, stop=True)

        if k % DRAIN == DRAIN - 1:
            drain(k - DRAIN + 1, k)
    # tail
    rem = iters % DRAIN
    if rem:
        drain(iters - rem, iters - 1)

    for p in (ppool, tpool, vpool, cpool):
        p.release()


def _dedup_ldweights(nc):
    """Remove legalizer-inserted LDWEIGHTS that reload an identical
    stationary; merge their deps into the following matmul."""
    removed = 0
    for f in nc.m.functions:
        for blk in f.blocks:
            insts = list(blk.instructions)
            last_key = None
            keep = []
            pending = []
            for inst in insts:
                nm = type(inst).__name__
                if nm == "InstLdweights":
                    key = (str(inst.ins[0]), str(inst.tile_position),
                           str(inst.perf_mode), bool(inst.is_transpose))
                    if key == last_key:
                        pending.append(inst)
                        removed += 1
                        continue
                    last_key = key
                    keep.append(inst)
                elif nm == "InstMatmult":
                    for ld in pending:
                        inst.merge_dependencies_from(ld)
                    pending = []
                    keep.append(inst)
                else:
                    keep.append(inst)
            assert not pending, "dangling removed LDWEIGHTS"
            if len(keep) != len(insts):
                blk.instructions = keep
    return removed


def _parallelize_act_stt(nc):
    """The Tile framework keeps a single linear dependency chain per
    PSUM tile, so the per-step DVE prep (STT) waits for the same-step
    tanh (ACT) even though both only READ the z slot.  Remove each
    STT -> ACT edge where the ACT's input AP equals the STT's in0 AP
    (reader-after-reader on the same slot), merging the ACT's own deps
    (the producing matmul) into the STT."""
    removed = 0
    for f in nc.m.functions:
        for blk in f.blocks:
            by_name = {}
            for inst in blk.instructions:
                by_name[inst.name] = inst
            for inst in blk.instructions:
                if type(inst).__name__ != "InstTensorScalarPtr":
                    continue
                src = str(inst.ins[0])
                for tname, _info in list(inst.sync_dependencies()):
                    dep = by_name.get(tname)
                    if dep is None or type(dep).__name__ != "InstActivation":
                        continue
                    if str(dep.ins[0]) != src:
                        continue
                    inst.remove_dependency(tname)
                    inst.merge_dependencies_from(dep)
                    removed += 1
    return removed


def build_nc(*, iters, cols, groups=3, ch=64, dedup=True,
             filler=0, cut=1, num_devices=NCORES):
    nc = bacc.Bacc("TRN2", target_bir_lowering=False, debug=False,
                   num_devices=num_devices)
    # Row stride must not be 8KB-aligned: DMA packets are spread over
    # engines by a source-address hash, and 2^k-aligned row strides make
    # every row hash to the SAME engine (observed: 103-row chunk loads
    # serialized onto one engine at ~31GB/s).  An odd element count
    # cycles row starts through all hash buckets.
    lp = iters * cols + 63
    aps = {
        "m_mat": nc.dram_tensor("m_mat", [K, K], F16,
                                kind="ExternalInput").ap(),
        "c_t": nc.dram_tensor("c_t", [K, lp], F16,
                              kind="ExternalInput").ap(),
        "o_out": nc.dram_tensor("o_out", [NB, lp], F16,
                                kind="ExternalOutput").ap(),
    }
    with tile.TileContext(nc) as tcx:
        emit_scan(tcx, nc, aps, iters=iters, cols=cols, groups=groups, ch=ch,
                  filler=filler, cut=cut)
    if dedup:
        _dedup_ldweights(nc)
        # with a single resident stationary, moving waits onto the one
        # surviving LDWEIGHTS would be wrong — keep waits on matmuls
        nc.move_matmul_waits_to_ldweights = lambda: None
    nc.compile()
    return nc


def make_m_mat(recurrent_weights, input_weights, output_weights):
    """Device stationary, lhsT [K, K]: columns 0:100 produce the z
    update from the tanh rows (u-rows zeroed: the tanh(o) rhs rows must
    contribute nothing — the u drive arrives via the host C stream);
    columns 100:103 produce the out-IIR increment DT*W_out*th."""
    m = np.zeros((K, K), np.float32)
    m[:N, :N] = DT * recurrent_weights.T
    m[:N, N:] = DT * output_weights.T
    return np.ascontiguousarray(m).astype(np.float16)


def make_mz_host(recurrent_weights, input_weights, output_weights):
    """Host-side C projection [K, K]: rows 0:100 = DT*[W_rec, W_in]
    (the u path lives here), rows 100:103 = DT*[W_out, 0] (the readout
    noise drive)."""
    mz = np.zeros((K, K), np.float32)
    mz[:N, :N] = DT * recurrent_weights
    mz[:N, N:] = DT * input_weights
    mz[N:, :N] = DT * output_weights
    return mz


def make_v(inputs, noise, *, s, warm, iters, cols):
    """v stream [K, iters, cols] f32 for one chain (time shard).

    inputs [cols, T, NB], noise [cols, T, N] (batch-block slices).
    Iteration k=0 is the bootstrap block [0 ; u_{s-warm}/DT]; iteration
    k>=1 covers global step g = s - warm + k - 1 (g<0 -> zeros)."""
    v = np.zeros((K, iters, cols), np.float32)
    g0 = s - warm
    if 0 <= g0 < T:
        v[N:, 0] = inputs[:, g0].T / DT
    for k in range(1, iters):
        g = g0 + k - 1
        if g < 0 or g >= T:
            continue
        v[:N, k] = noise[:, g].T * (NOISE_STD / DT)
        un = inputs[:, g + 1].T if g + 1 < T else 0.0
        v[N:, k] = (un - DECAY * inputs[:, g].T) / DT
    return v


def make_c(v, mz):
    """Host C = Mz_aug v, fp16, [K, iters*cols + 63] (padded rows to
    keep the DRAM row stride off the DMA engine-hash pathology)."""
    k_, it, cols = v.shape
    c = np.zeros((K, it * cols + 63), np.float16)
    c[:, :it * cols] = (mz @ v.reshape(K, it * cols)).astype(np.float16)
    return c


def make_in_maps(inputs, noise, recurrent_weights, input_weights,
                 output_weights, *, groups, iters, plan):
    """Per-core input maps.  Core i runs chains i*groups..(i+1)*groups-1;
    chain q = (bblock, shard) = divmod(q, 2*groups).  The c stream per
    core interleaves its chains' 128-col blocks within each iteration."""
    m = make_m_mat(recurrent_weights, input_weights, output_weights)
    mz = make_mz_host(recurrent_weights, input_weights, output_weights)
    in_maps = []
    for core in range(NCORES):
        vs = []
        for g in range(groups):
            q = core * groups + g
            bb, sh = divmod(q, 2 * groups)
            bsl = slice(bb * GW, (bb + 1) * GW)
            ui = np.ascontiguousarray(inputs[bsl]).astype(np.float32)
            nz = np.ascontiguousarray(noise[bsl]).astype(np.float32)
            s, warm, r = plan[sh]
            vs.append(make_v(ui, nz, s=s, warm=warm, iters=iters, cols=GW))
        v = np.concatenate([vv[:, :, None, :] for vv in vs], axis=2)
        v = v.reshape(K, iters, groups * GW)
        in_maps.append({"m_mat": m, "c_t": make_c(v, mz)})
    return in_maps


def gather_out(results, *, groups, iters, plan):
    """Decode the tanh-encoded device out IIR: out = atanh(o_out)."""
    out = np.empty((B, T, NB), np.float32)
    cols = groups * GW
    for core in range(NCORES):
        o = results[core]["o_out"][:, :iters * cols].astype(np.float32)
        o = np.arctanh(np.clip(o, -0.999999, 0.999999))
        o = o.reshape(NB, iters, groups, GW)
        for g in range(groups):
            q = core * groups + g
            bb, sh = divmod(q, 2 * groups)
            bsl = slice(bb * GW, (bb + 1) * GW)
            s, warm, r = plan[sh]
            # o[:, k] = out IIR of iteration k-1 = global step s-warm+k-2
            out[bsl, s:s + r] = o[:, warm + 2:warm + 2 + r, g].transpose(2, 1, 0)
    return out


_NC_CACHE = {}


def kernel(inputs, noise, recurrent_weights, input_weights, output_weights,
           **run_kwargs):
    cfg = dict(run_kwargs.pop("cfg", {"filler": 0}))
    groups = cfg.setdefault("groups", 3)
    plan, iters = plan_for(groups)
    cfg.setdefault("iters", iters)
    cfg.setdefault("cols", groups * GW)
    key = tuple(sorted(cfg.items()))
    if key not in _NC_CACHE:
        _NC_CACHE[key] = build_nc(**cfg)
    nc = _NC_CACHE[key]
    in_maps = make_in_maps(inputs, noise, recurrent_weights,
                           input_weights, output_weights, groups=groups,
                           iters=cfg["iters"], plan=plan)
    res = run_bass_kernel_spmd(nc, in_maps, core_ids=list(range(NCORES)),
                               **run_kwargs)
    out = gather_out(res.results, groups=groups,
                     iters=cfg["iters"], plan=plan)
    if run_kwargs.get("trace"):
        return out, res
    return out


# revision 38
# speedup vs baseline: 2.7891x; 1.0377x over previous
"""Trainium2 Bass kernel for the ContinuousRNN problem.

Reference (per batch row b):
    h_0 = 0                               # [N], N=100
    z_t = W_rec h_t + W_in u_t
    h_{t+1} = 0.85 h_t + 0.15 tanh(z_t) + NOISE_STD noise_t
    out_t = W_out h_{t+1}

z-space reformulation (state z_t, N rows):
    z_{t+1} = 0.85 z_t + Mz (th_t + v_t)
      Mz   = 0.15 [W_rec, W_in]                          (100x103)
      th_t = [tanh(z_t) ; 0]
      v_t  = [noise_t*NS/DT ; (u_{t+1}-0.85 u_t)/DT]
    bootstrap (h=0): z_0 = Mz [0 ; u_0/DT] = W_in u_0
    out_t = 0.85 out_{t-1} + 0.15 W_out (tanh(z_t) + noise_t*NS/DT)
    (the out IIR is a linear readout of the device-produced tanh
    stream; it runs on host, exactly mirroring the device recurrence)

Per-core per-step critical path is two hops:
    MM_t -> { ACT tanh (psum->sbuf fp16)  ||  DVE prep z' = 0.85 z + C } -> MM_{t+1}
where MM_{t+1} accumulates (start=False) onto the DVE-prepped psum slot.
That accumulation works because each z bank is primed once by a
start=True matmul (sets the psum has_written bits, which non-PE writes
do not clear).  C_t = Mz v_t is host-precomputed and streamed.  The PE
never reloads weights (redundant LDWEIGHTS are deduped by a post-tile
pass).  The tanh stream drains from SBUF by DMA.

The Tile framework tracks PSUM deps as a single linear chain per tile
(each accessor waits for the previous accessor), which would serialize
ACT and DVE (both read the same z slot).  A post-pass
(_parallelize_act_stt) removes those reader-after-reader edges so ACT
and DVE truly run in parallel.

Sharding: each core runs `groups` independent chains of 128 batch
columns; chains are (batch-block, time-shard) pairs.  With 8 cores and
n chains/core there are 2n time shards over 4 batch blocks.  The RNN
contracts (~0.983/step), so time shards s>0 warm up for L steps from
h=0; shard 0 "warms up" on zero-padded inputs (exactly h=0).  All cores
run the identical SPMD program; host slices each chain's valid range.
"""

import sys

for _p in ("/opt/trn_rl_repo",):
    if _p not in sys.path:
        sys.path.insert(0, _p)

import numpy as np

import concourse.bass as bass
import concourse.bacc as bacc
import concourse.mybir as mybir
from concourse import tile
from concourse.bass_utils import run_bass_kernel_spmd

F32 = mybir.dt.float32
F16 = mybir.dt.float16

N = 100
NB = 3
K = N + NB        # 103 (matmul contraction: tanh rows + zero-padded u rows)
B = 512
T = 2048
NCORES = 8
DT = np.float32(0.15)
NOISE_STD = np.float32(0.015)
DECAY = np.float32(0.85)

GW = 128                # batch columns per chain
L_WARM = 240            # warmup steps for time shards > 0

NQ = 8                  # z slots (2 parity tiles x 4 slots per chain)
NTH = 16                # th ring slots
DRAIN = 8               # drain period (iters)


def plan_for(groups):
    """Shard plan for `groups` chains/core: S time shards, per-shard
    (start, warm, r), uniform ITERS."""
    S = 2 * groups
    A = -(-(T + (S - 1) * L_WARM) // S)     # ceil
    rs = [A] + [A - L_WARM] * (S - 1)
    excess = sum(rs) - T
    rs[-1] -= excess
    assert rs[-1] > 0
    plan = []
    s = 0
    for j in range(S):
        plan.append((s, 0 if j == 0 else L_WARM, rs[j]))
        s += rs[j]
    assert s == T
    # ITERS = A + 2: one bootstrap iteration up front, plus one trailing
    # iteration so the last output's o-state gets tanh'd and drained
    # (the drained o at iteration k is the out IIR of iteration k-1).
    return plan, A + 2


def emit_scan(tc, nc, aps, *, iters, cols, groups, ch, filler=0, cut=0):
    """aps: m_mat [K,M] f16 (lhsT, M=K with the 3 readout columns),
    c_t [M, iters*cols] f16 (iteration-major, host-computed C = Mz v),
    o_out [NB, iters*cols] f16 (the tanh-encoded out IIR).

    The stationary's columns 100:103 compute the out IIR readout
    p = DT*W_out*th into psum rows 100:103 of the same z slot; the STT
    applies the same 0.85 decay (out IIR == z decay), and the per-step
    tanh ACT covers rows 0:103, so tanh(out_t) rides the tht ring for
    free (tanh is invertible; host applies atanh).  The stationary's
    u-rows are zeroed so the tanh(o) rhs rows contribute nothing.

    PSUM dependency tracking is per-tile, so z state is split into
    per-(group x parity) psum tiles.  C streams through SBUF (walrus
    rejects TensorScalarPtr with all-PSUM operands, and SBUF tiles get
    fine-grained dep tracking)."""
    gw = cols // groups
    mult = mybir.AluOpType.mult
    add = mybir.AluOpType.add
    tanh = mybir.ActivationFunctionType.Tanh
    hq = NQ // 2           # z slots per parity tile

    cpool = tc.alloc_tile_pool(name="const", bufs=1)
    vpool = tc.alloc_tile_pool(name="cstream", bufs=3)
    tpool = tc.alloc_tile_pool(name="th", bufs=1)
    ppool = tc.alloc_tile_pool(name="psum", bufs=1, space="PSUM")

    wb = cpool.tile([K, K], F16, name="wb")
    nc.sync.dma_start(wb[:, :], aps["m_mat"][:, :])

    zt = cpool.tile([K, 512], F16, name="zt")   # zero rhs for priming
    nc.vector.memset(zt[:, :], 0.0)

    # z state: per (group, parity) psum tiles, hq slots of [K, gw] each
    qts = [[ppool.tile([128, hq * gw], F32, name=f"qt{g}p{par}")
            for par in range(2)] for g in range(groups)]
    # scratch bank for PE-warming filler matmuls
    fts = ppool.tile([128, 512], F32, name="fts") if filler else None

    # th ring (fp16); rows 0:100 tanh(z), rows 100:103 tanh(out IIR)
    tht = tpool.tile([K, NTH * cols], F16, name="tht")
    nc.vector.memset(tht[96:K, :], 0.0)

    # prime z tiles: start=True matmuls set has_written over all z slots
    for g in range(groups):
        for par in range(2):
            w = hq * gw
            assert w <= 512
            nc.tensor.matmul(qts[g][par][0:K, 0:w], wb[:, :],
                             zt[:, 0:w], start=True, stop=True)

    prev_mm = [None] * groups
    # C staging, double buffered
    ctiles = {}

    def c_chunk(ci):
        if ci * ch >= iters:
            return None
        if ci not in ctiles:
            tl = vpool.tile([K, ch * cols], F16, tag="cs", name=f"cs{ci}")
            hi = min((ci + 1) * ch, iters)
            w = (hi - ci * ch) * cols
            # split rows 0:100 / 100:103: the HWDGE spreads a DMA's
            # rows across engines only for round row counts (100 -> 10
            # engines, 3 -> 3); a 103-row DMA lands on ONE engine and
            # serializes the whole chunk load at ~31GB/s.
            nc.sync.dma_start(tl[0:N, 0:w],
                              aps["c_t"][0:N, ci * ch * cols:hi * cols])
            nc.sync.dma_start(tl[N:K, 0:w],
                              aps["c_t"][N:K, ci * ch * cols:hi * cols])
            ctiles[ci] = tl
        return ctiles[ci]

    c_chunk(0)
    c_chunk(1)

    def drain(k_lo, k_hi):
        """DMA the tanh(out) rows for iterations k_lo..k_hi (inclusive,
        contiguous in the ring) to DRAM."""
        c0 = (k_lo % NTH) * cols
        c1 = c0 + (k_hi - k_lo + 1) * cols
        nc.sync.dma_start(aps["o_out"][:, k_lo * cols:k_lo * cols + c1 - c0],
                          tht[N:K, c0:c1])

    for k in range(iters):
        ci = k // ch
        if k % ch == 0:
            c_chunk(ci + 2)   # two chunks ahead (triple buffered)
        cc = (k % ch) * cols
        ctile = ctiles[ci]

        qs = ((k // 2) % hq) * gw       # read slot col (parity k%2)
        qn = (((k + 1) // 2) % hq) * gw  # write slot col (parity (k+1)%2)
        tc0 = (k % NTH) * cols
        for g in range(groups):
            rd = qts[g][k % 2]
            wr = qts[g][(k + 1) % 2]
            # ACT: th = tanh(z | o) psum -> sbuf fp16
            act_inst = nc.scalar.activation(
                tht[0:K, tc0 + g * gw:tc0 + (g + 1) * gw],
                rd[0:K, qs:qs + gw], tanh)
            # DVE prep: z' = 0.85 z + C  (psum+sbuf -> psum, other parity)
            stt_inst = nc.vector.scalar_tensor_tensor(
                wr[0:K, qn:qn + gw], rd[0:K, qs:qs + gw],
                float(DECAY), ctile[0:K, cc + g * gw:cc + (g + 1) * gw],
                mult, add)
            # The tile framework keeps a single linear dependency chain
            # per PSUM tile, so the STT would wait for the same-step
            # tanh even though both only READ the z slot.  Cut that
            # reader-after-reader edge (inline: sem waits are assigned
            # from these edges at TileContext exit), inheriting the
            # ACT's own deps (the producing matmul).
            if cut == 1:
                # cut the edge, inherit the ACT's own deps
                ai, si = act_inst.ins, stt_inst.ins
                for tname, _info in list(si.sync_dependencies()):
                    if tname == ai.name:
                        si.remove_dependency(tname)
                        si.merge_dependencies_from(ai)
            elif cut == 2:
                # cut the edge, add a clean edge to the producing matmul
                ai, si = act_inst.ins, stt_inst.ins
                for tname, info in list(si.sync_dependencies()):
                    if tname == ai.name:
                        si.remove_dependency(tname)
                        if prev_mm[g] is not None:
                            si.add_dependency(prev_mm[g].ins.name, info)
            # chain MM accumulates onto the prepped slot
            prev_mm[g] = nc.tensor.matmul(
                wr[0:K, qn:qn + gw], wb[:, :],
                tht[0:K, tc0 + g * gw:tc0 + (g + 1) * gw],
                start=False, stop=True, skip_group_check=True)
        if filler:
            # keep the PE pipeline warm with a throwaway matmul
            nc.tensor.matmul(fts[0:K, 0:filler], wb[:, :], zt[:, 0:filler],
                             start=True, stop=True)

        if k % DRAIN == DRAIN - 1:
            drain(k - DRAIN + 1, k)
    # tail
    rem = iters % DRAIN
    if rem:
        drain(iters - rem, iters - 1)

    for p in (ppool, tpool, vpool, cpool):
        p.release()


def _dedup_ldweights(nc):
    """Remove legalizer-inserted LDWEIGHTS that reload an identical
    stationary; merge their deps into the following matmul."""
    removed = 0
    for f in nc.m.functions:
        for blk in f.blocks:
            insts = list(blk.instructions)
            last_key = None
            keep = []
            pending = []
            for inst in insts:
                nm = type(inst).__name__
                if nm == "InstLdweights":
                    key = (str(inst.ins[0]), str(inst.tile_position),
                           str(inst.perf_mode), bool(inst.is_transpose))
                    if key == last_key:
                        pending.append(inst)
                        removed += 1
                        continue
                    last_key = key
                    keep.append(inst)
                elif nm == "InstMatmult":
                    for ld in pending:
                        inst.merge_dependencies_from(ld)
                    pending = []
                    keep.append(inst)
                else:
                    keep.append(inst)
            assert not pending, "dangling removed LDWEIGHTS"
            if len(keep) != len(insts):
                blk.instructions = keep
    return removed


def _swap_stt_waits(nc):
    """Post-compile variant of the ACT/STT parallelization: the
    scheduler serialized STT(k) behind the same-step tanh ACT(k) (both
    only READ the z slot) and, by wait-dominance elision, materialized
    ONLY an Activation-sem wait on the STT.  Replace that wait with a
    copy of the paired ACT's own PE wait (the producing matmul), which
    is the STT's true dependency.  This edits only the final SyncInfo
    the hardware executes; scheduling/ticks/queues are untouched."""
    swapped = 0
    for f in nc.m.functions:
        for blk in f.blocks:
            pe_tick = 0
            pe_tmpl = None      # a SyncWait on the PE sem, as template
            mm_tick = {}        # dst slot ap -> PE tick of last writing MM
            for inst in blk.instructions:
                nm = type(inst).__name__
                si = inst.sync_info
                if si is None:
                    continue
                if pe_tmpl is None:
                    for w in si.on_wait:
                        if (w.ant_name or "").startswith("PE_"):
                            pe_tmpl = w
                            break
                for u in si.on_update:
                    if (u.ant_name or "").startswith("PE_"):
                        pe_tick += u.update_value or 1
                if nm == "InstMatmult":
                    mm_tick[str(inst.outs[0])] = pe_tick
                    continue
                if nm != "InstTensorScalarPtr" or pe_tmpl is None:
                    continue
                need = mm_tick.get(str(inst.ins[0]))
                if need is None:
                    continue        # priming phase: keep serialized wait
                new_wait = []
                hit = False
                for w in si.on_wait:
                    if (w.ant_name or "").startswith("Activation"):
                        hit = True
                        new_wait.append(mybir.SyncWait(
                            sync_type=pe_tmpl.sync_type, id=pe_tmpl.id,
                            wait_mode=pe_tmpl.wait_mode,
                            ant_name=pe_tmpl.ant_name, wait_value=need))
                    else:
                        new_wait.append(w)
                if hit:
                    inst.sync_info = mybir.SyncInfo(
                        on_wait=new_wait, on_update=list(si.on_update))
                    swapped += 1
    return swapped


def _parallelize_act_stt(nc):
    """The Tile framework keeps a single linear dependency chain per
    PSUM tile, so the per-step DVE prep (STT) waits for the same-step
    tanh (ACT) even though both only READ the z slot.  Remove each
    STT -> ACT edge where the ACT's input AP equals the STT's in0 AP
    (reader-after-reader on the same slot), merging the ACT's own deps
    (the producing matmul) into the STT."""
    removed = 0
    for f in nc.m.functions:
        for blk in f.blocks:
            by_name = {}
            for inst in blk.instructions:
                by_name[inst.name] = inst
            for inst in blk.instructions:
                if type(inst).__name__ != "InstTensorScalarPtr":
                    continue
                src = str(inst.ins[0])
                for tname, _info in list(inst.sync_dependencies()):
                    dep = by_name.get(tname)
                    if dep is None or type(dep).__name__ != "InstActivation":
                        continue
                    if str(dep.ins[0]) != src:
                        continue
                    inst.remove_dependency(tname)
                    inst.merge_dependencies_from(dep)
                    removed += 1
    return removed


def build_nc(*, iters, cols, groups=3, ch=48, dedup=True,
             filler=0, cut=0, num_devices=NCORES):
    nc = bacc.Bacc("TRN2", target_bir_lowering=False, debug=False,
                   num_devices=num_devices)
    # Row stride must not be 8KB-aligned: DMA packets are spread over
    # engines by a source-address hash, and 2^k-aligned row strides make
    # every row hash to the SAME engine (observed: 103-row chunk loads
    # serialized onto one engine at ~31GB/s).  An odd element count
    # cycles row starts through all hash buckets.
    lp = iters * cols + 63
    aps = {
        "m_mat": nc.dram_tensor("m_mat", [K, K], F16,
                                kind="ExternalInput").ap(),
        "c_t": nc.dram_tensor("c_t", [K, lp], F16,
                              kind="ExternalInput").ap(),
        "o_out": nc.dram_tensor("o_out", [NB, lp], F16,
                                kind="ExternalOutput").ap(),
    }
    with tile.TileContext(nc) as tcx:
        emit_scan(tcx, nc, aps, iters=iters, cols=cols, groups=groups, ch=ch,
                  filler=filler, cut=cut)
    if dedup:
        _dedup_ldweights(nc)
        # with a single resident stationary, moving waits onto the one
        # surviving LDWEIGHTS would be wrong — keep waits on matmuls
        nc.move_matmul_waits_to_ldweights = lambda: None
    if cut == 3:
        # run the wait swap inside compile, right before the ISA
        # encodings are generated (post-compile edits to sync_info
        # don't reach the serialized program)
        orig_codegen = nc.codegen_inst_isa_subclasses

        def _patched_codegen():
            _swap_stt_waits(nc)
            orig_codegen()

        nc.codegen_inst_isa_subclasses = _patched_codegen
    nc.compile()
    return nc


def make_m_mat(recurrent_weights, input_weights, output_weights):
    """Device stationary, lhsT [K, K]: columns 0:100 produce the z
    update from the tanh rows (u-rows zeroed: the tanh(o) rhs rows must
    contribute nothing — the u drive arrives via the host C stream);
    columns 100:103 produce the out-IIR increment DT*W_out*th."""
    m = np.zeros((K, K), np.float32)
    m[:N, :N] = DT * recurrent_weights.T
    m[:N, N:] = DT * output_weights.T
    return np.ascontiguousarray(m).astype(np.float16)


def make_mz_host(recurrent_weights, input_weights, output_weights):
    """Host-side C projection [K, K]: rows 0:100 = DT*[W_rec, W_in]
    (the u path lives here), rows 100:103 = DT*[W_out, 0] (the readout
    noise drive)."""
    mz = np.zeros((K, K), np.float32)
    mz[:N, :N] = DT * recurrent_weights
    mz[:N, N:] = DT * input_weights
    mz[N:, :N] = DT * output_weights
    return mz


def make_v(inputs, noise, *, s, warm, iters, cols):
    """v stream [K, iters, cols] f32 for one chain (time shard).

    inputs [cols, T, NB], noise [cols, T, N] (batch-block slices).
    Iteration k=0 is the bootstrap block [0 ; u_{s-warm}/DT]; iteration
    k>=1 covers global step g = s - warm + k - 1 (g<0 -> zeros)."""
    v = np.zeros((K, iters, cols), np.float32)
    g0 = s - warm
    if 0 <= g0 < T:
        v[N:, 0] = inputs[:, g0].T / DT
    for k in range(1, iters):
        g = g0 + k - 1
        if g < 0 or g >= T:
            continue
        v[:N, k] = noise[:, g].T * (NOISE_STD / DT)
        un = inputs[:, g + 1].T if g + 1 < T else 0.0
        v[N:, k] = (un - DECAY * inputs[:, g].T) / DT
    return v


def make_c(v, mz):
    """Host C = Mz_aug v, fp16, [K, iters*cols + 63] (padded rows to
    keep the DRAM row stride off the DMA engine-hash pathology)."""
    k_, it, cols = v.shape
    c = np.zeros((K, it * cols + 63), np.float16)
    c[:, :it * cols] = (mz @ v.reshape(K, it * cols)).astype(np.float16)
    return c


def make_in_maps(inputs, noise, recurrent_weights, input_weights,
                 output_weights, *, groups, iters, plan):
    """Per-core input maps.  Core i runs chains i*groups..(i+1)*groups-1;
    chain q = (bblock, shard) = divmod(q, 2*groups).  The c stream per
    core interleaves its chains' 128-col blocks within each iteration."""
    m = make_m_mat(recurrent_weights, input_weights, output_weights)
    mz = make_mz_host(recurrent_weights, input_weights, output_weights)
    in_maps = []
    for core in range(NCORES):
        vs = []
        for g in range(groups):
            q = core * groups + g
            bb, sh = divmod(q, 2 * groups)
            bsl = slice(bb * GW, (bb + 1) * GW)
            ui = np.ascontiguousarray(inputs[bsl]).astype(np.float32)
            nz = np.ascontiguousarray(noise[bsl]).astype(np.float32)
            s, warm, r = plan[sh]
            vs.append(make_v(ui, nz, s=s, warm=warm, iters=iters, cols=GW))
        v = np.concatenate([vv[:, :, None, :] for vv in vs], axis=2)
        v = v.reshape(K, iters, groups * GW)
        in_maps.append({"m_mat": m, "c_t": make_c(v, mz)})
    return in_maps


def gather_out(results, *, groups, iters, plan):
    """Decode the tanh-encoded device out IIR: out = atanh(o_out)."""
    out = np.empty((B, T, NB), np.float32)
    cols = groups * GW
    for core in range(NCORES):
        o = results[core]["o_out"][:, :iters * cols].astype(np.float32)
        o = np.arctanh(np.clip(o, -0.999999, 0.999999))
        o = o.reshape(NB, iters, groups, GW)
        for g in range(groups):
            q = core * groups + g
            bb, sh = divmod(q, 2 * groups)
            bsl = slice(bb * GW, (bb + 1) * GW)
            s, warm, r = plan[sh]
            # o[:, k] = out IIR of iteration k-1 = global step s-warm+k-2
            out[bsl, s:s + r] = o[:, warm + 2:warm + 2 + r, g].transpose(2, 1, 0)
    return out


_NC_CACHE = {}


def kernel(inputs, noise, recurrent_weights, input_weights, output_weights,
           **run_kwargs):
    cfg = dict(run_kwargs.pop("cfg", {"filler": 0}))
    groups = cfg.setdefault("groups", 3)
    plan, iters = plan_for(groups)
    cfg.setdefault("iters", iters)
    cfg.setdefault("cols", groups * GW)
    key = tuple(sorted(cfg.items()))
    if key not in _NC_CACHE:
        _NC_CACHE[key] = build_nc(**cfg)
    nc = _NC_CACHE[key]
    in_maps = make_in_maps(inputs, noise, recurrent_weights,
                           input_weights, output_weights, groups=groups,
                           iters=cfg["iters"], plan=plan)
    res = run_bass_kernel_spmd(nc, in_maps, core_ids=list(range(NCORES)),
                               **run_kwargs)
    out = gather_out(res.results, groups=groups,
                     iters=cfg["iters"], plan=plan)
    if run_kwargs.get("trace"):
        return out, res
    return out


# revision 39
# speedup vs baseline: 2.8232x; 1.0122x over previous
"""Trainium2 Bass kernel for the ContinuousRNN problem.

Reference (per batch row b):
    h_0 = 0                               # [N], N=100
    z_t = W_rec h_t + W_in u_t
    h_{t+1} = 0.85 h_t + 0.15 tanh(z_t) + NOISE_STD noise_t
    out_t = W_out h_{t+1}

z-space reformulation (state z_t, N rows):
    z_{t+1} = 0.85 z_t + Mz (th_t + v_t)
      Mz   = 0.15 [W_rec, W_in]                          (100x103)
      th_t = [tanh(z_t) ; 0]
      v_t  = [noise_t*NS/DT ; (u_{t+1}-0.85 u_t)/DT]
    bootstrap (h=0): z_0 = Mz [0 ; u_0/DT] = W_in u_0
    out_t = 0.85 out_{t-1} + 0.15 W_out (tanh(z_t) + noise_t*NS/DT)
    (the out IIR is a linear readout of the device-produced tanh
    stream; it runs on host, exactly mirroring the device recurrence)

Per-core per-step critical path is two hops:
    MM_t -> { ACT tanh (psum->sbuf fp16)  ||  DVE prep z' = 0.85 z + C } -> MM_{t+1}
where MM_{t+1} accumulates (start=False) onto the DVE-prepped psum slot.
That accumulation works because each z bank is primed once by a
start=True matmul (sets the psum has_written bits, which non-PE writes
do not clear).  C_t = Mz v_t is host-precomputed and streamed.  The PE
never reloads weights (redundant LDWEIGHTS are deduped by a post-tile
pass).  The tanh stream drains from SBUF by DMA.

The Tile framework tracks PSUM deps as a single linear chain per tile
(each accessor waits for the previous accessor), which would serialize
ACT and DVE (both read the same z slot).  A post-pass
(_parallelize_act_stt) removes those reader-after-reader edges so ACT
and DVE truly run in parallel.

Sharding: each core runs `groups` independent chains of 128 batch
columns; chains are (batch-block, time-shard) pairs.  With 8 cores and
n chains/core there are 2n time shards over 4 batch blocks.  The RNN
contracts (~0.983/step), so time shards s>0 warm up for L steps from
h=0; shard 0 "warms up" on zero-padded inputs (exactly h=0).  All cores
run the identical SPMD program; host slices each chain's valid range.
"""

import sys

for _p in ("/opt/trn_rl_repo",):
    if _p not in sys.path:
        sys.path.insert(0, _p)

import numpy as np

import concourse.bass as bass
import concourse.bacc as bacc
import concourse.mybir as mybir
from concourse import tile
from concourse.bass_utils import run_bass_kernel_spmd

F32 = mybir.dt.float32
F16 = mybir.dt.float16

N = 100
NB = 3
K = N + NB        # 103 (matmul contraction: tanh rows + zero-padded u rows)
B = 512
T = 2048
NCORES = 8
DT = np.float32(0.15)
NOISE_STD = np.float32(0.015)
DECAY = np.float32(0.85)

GW = 128                # batch columns per chain
L_WARM = 224            # warmup steps for time shards > 0

NQ = 8                  # z slots (2 parity tiles x 4 slots per chain)
NTH = 16                # th ring slots
DRAIN = 8               # drain period (iters)


def plan_for(groups):
    """Shard plan for `groups` chains/core: S time shards, per-shard
    (start, warm, r), uniform ITERS."""
    S = 2 * groups
    A = -(-(T + (S - 1) * L_WARM) // S)     # ceil
    rs = [A] + [A - L_WARM] * (S - 1)
    excess = sum(rs) - T
    rs[-1] -= excess
    assert rs[-1] > 0
    plan = []
    s = 0
    for j in range(S):
        plan.append((s, 0 if j == 0 else L_WARM, rs[j]))
        s += rs[j]
    assert s == T
    # ITERS = A + 2: one bootstrap iteration up front, plus one trailing
    # iteration so the last output's o-state gets tanh'd and drained
    # (the drained o at iteration k is the out IIR of iteration k-1).
    return plan, A + 2


def emit_scan(tc, nc, aps, *, iters, cols, groups, ch, filler=0, cut=0):
    """aps: m_mat [K,M] f16 (lhsT, M=K with the 3 readout columns),
    c_t [M, iters*cols] f16 (iteration-major, host-computed C = Mz v),
    o_out [NB, iters*cols] f16 (the tanh-encoded out IIR).

    The stationary's columns 100:103 compute the out IIR readout
    p = DT*W_out*th into psum rows 100:103 of the same z slot; the STT
    applies the same 0.85 decay (out IIR == z decay), and the per-step
    tanh ACT covers rows 0:103, so tanh(out_t) rides the tht ring for
    free (tanh is invertible; host applies atanh).  The stationary's
    u-rows are zeroed so the tanh(o) rhs rows contribute nothing.

    PSUM dependency tracking is per-tile, so z state is split into
    per-(group x parity) psum tiles.  C streams through SBUF (walrus
    rejects TensorScalarPtr with all-PSUM operands, and SBUF tiles get
    fine-grained dep tracking)."""
    gw = cols // groups
    mult = mybir.AluOpType.mult
    add = mybir.AluOpType.add
    tanh = mybir.ActivationFunctionType.Tanh
    hq = NQ // 2           # z slots per parity tile

    cpool = tc.alloc_tile_pool(name="const", bufs=1)
    vpool = tc.alloc_tile_pool(name="cstream", bufs=3)
    tpool = tc.alloc_tile_pool(name="th", bufs=1)
    ppool = tc.alloc_tile_pool(name="psum", bufs=1, space="PSUM")

    wb = cpool.tile([K, K], F16, name="wb")
    nc.sync.dma_start(wb[:, :], aps["m_mat"][:, :])

    zt = cpool.tile([K, 512], F16, name="zt")   # zero rhs for priming
    nc.vector.memset(zt[:, :], 0.0)

    # z state: per (group, parity) psum tiles, hq slots of [K, gw] each
    qts = [[ppool.tile([128, hq * gw], F32, name=f"qt{g}p{par}")
            for par in range(2)] for g in range(groups)]
    # scratch bank for PE-warming filler matmuls
    fts = ppool.tile([128, 512], F32, name="fts") if filler else None

    # th ring (fp16); rows 0:100 tanh(z), rows 100:103 tanh(out IIR)
    tht = tpool.tile([K, NTH * cols], F16, name="tht")
    nc.vector.memset(tht[96:K, :], 0.0)

    # prime z tiles: start=True matmuls set has_written over all z slots
    for g in range(groups):
        for par in range(2):
            w = hq * gw
            assert w <= 512
            nc.tensor.matmul(qts[g][par][0:K, 0:w], wb[:, :],
                             zt[:, 0:w], start=True, stop=True)

    prev_mm = [None] * groups
    # C staging, double buffered
    ctiles = {}

    def c_chunk(ci):
        if ci * ch >= iters:
            return None
        if ci not in ctiles:
            tl = vpool.tile([K, ch * cols], F16, tag="cs", name=f"cs{ci}")
            hi = min((ci + 1) * ch, iters)
            w = (hi - ci * ch) * cols
            # split rows 0:100 / 100:103: the HWDGE spreads a DMA's
            # rows across engines only for round row counts (100 -> 10
            # engines, 3 -> 3); a 103-row DMA lands on ONE engine and
            # serializes the whole chunk load at ~31GB/s.
            nc.sync.dma_start(tl[0:N, 0:w],
                              aps["c_t"][0:N, ci * ch * cols:hi * cols])
            nc.sync.dma_start(tl[N:K, 0:w],
                              aps["c_t"][N:K, ci * ch * cols:hi * cols])
            ctiles[ci] = tl
        return ctiles[ci]

    c_chunk(0)
    c_chunk(1)

    def drain(k_lo, k_hi):
        """DMA the tanh(out) rows for iterations k_lo..k_hi (inclusive,
        contiguous in the ring) to DRAM."""
        c0 = (k_lo % NTH) * cols
        c1 = c0 + (k_hi - k_lo + 1) * cols
        nc.sync.dma_start(aps["o_out"][:, k_lo * cols:k_lo * cols + c1 - c0],
                          tht[N:K, c0:c1])

    for k in range(iters):
        ci = k // ch
        if k % ch == 0:
            c_chunk(ci + 2)   # two chunks ahead (triple buffered)
        cc = (k % ch) * cols
        ctile = ctiles[ci]

        qs = ((k // 2) % hq) * gw       # read slot col (parity k%2)
        qn = (((k + 1) // 2) % hq) * gw  # write slot col (parity (k+1)%2)
        tc0 = (k % NTH) * cols
        for g in range(groups):
            rd = qts[g][k % 2]
            wr = qts[g][(k + 1) % 2]
            # ACT: th = tanh(z | o) psum -> sbuf fp16
            act_inst = nc.scalar.activation(
                tht[0:K, tc0 + g * gw:tc0 + (g + 1) * gw],
                rd[0:K, qs:qs + gw], tanh)
            # DVE prep: z' = 0.85 z + C  (psum+sbuf -> psum, other parity)
            stt_inst = nc.vector.scalar_tensor_tensor(
                wr[0:K, qn:qn + gw], rd[0:K, qs:qs + gw],
                float(DECAY), ctile[0:K, cc + g * gw:cc + (g + 1) * gw],
                mult, add)
            # The tile framework keeps a single linear dependency chain
            # per PSUM tile, so the STT would wait for the same-step
            # tanh even though both only READ the z slot.  Cut that
            # reader-after-reader edge (inline: sem waits are assigned
            # from these edges at TileContext exit), inheriting the
            # ACT's own deps (the producing matmul).
            if cut == 1:
                # cut the edge, inherit the ACT's own deps
                ai, si = act_inst.ins, stt_inst.ins
                for tname, _info in list(si.sync_dependencies()):
                    if tname == ai.name:
                        si.remove_dependency(tname)
                        si.merge_dependencies_from(ai)
            elif cut == 2:
                # cut the edge, add a clean edge to the producing matmul
                ai, si = act_inst.ins, stt_inst.ins
                for tname, info in list(si.sync_dependencies()):
                    if tname == ai.name:
                        si.remove_dependency(tname)
                        if prev_mm[g] is not None:
                            si.add_dependency(prev_mm[g].ins.name, info)
            # chain MM accumulates onto the prepped slot
            prev_mm[g] = nc.tensor.matmul(
                wr[0:K, qn:qn + gw], wb[:, :],
                tht[0:K, tc0 + g * gw:tc0 + (g + 1) * gw],
                start=False, stop=True, skip_group_check=True)
        if filler:
            # keep the PE pipeline warm with a throwaway matmul
            nc.tensor.matmul(fts[0:K, 0:filler], wb[:, :], zt[:, 0:filler],
                             start=True, stop=True)

        if k % DRAIN == DRAIN - 1:
            drain(k - DRAIN + 1, k)
    # tail
    rem = iters % DRAIN
    if rem:
        drain(iters - rem, iters - 1)

    for p in (ppool, tpool, vpool, cpool):
        p.release()


def _dedup_ldweights(nc):
    """Remove legalizer-inserted LDWEIGHTS that reload an identical
    stationary; merge their deps into the following matmul."""
    removed = 0
    for f in nc.m.functions:
        for blk in f.blocks:
            insts = list(blk.instructions)
            last_key = None
            keep = []
            pending = []
            for inst in insts:
                nm = type(inst).__name__
                if nm == "InstLdweights":
                    key = (str(inst.ins[0]), str(inst.tile_position),
                           str(inst.perf_mode), bool(inst.is_transpose))
                    if key == last_key:
                        pending.append(inst)
                        removed += 1
                        continue
                    last_key = key
                    keep.append(inst)
                elif nm == "InstMatmult":
                    for ld in pending:
                        inst.merge_dependencies_from(ld)
                    pending = []
                    keep.append(inst)
                else:
                    keep.append(inst)
            assert not pending, "dangling removed LDWEIGHTS"
            if len(keep) != len(insts):
                blk.instructions = keep
    return removed


def _swap_stt_waits(nc):
    """Post-compile variant of the ACT/STT parallelization: the
    scheduler serialized STT(k) behind the same-step tanh ACT(k) (both
    only READ the z slot) and, by wait-dominance elision, materialized
    ONLY an Activation-sem wait on the STT.  Replace that wait with a
    copy of the paired ACT's own PE wait (the producing matmul), which
    is the STT's true dependency.  This edits only the final SyncInfo
    the hardware executes; scheduling/ticks/queues are untouched."""
    swapped = 0
    for f in nc.m.functions:
        for blk in f.blocks:
            pe_tick = 0
            pe_tmpl = None      # a SyncWait on the PE sem, as template
            mm_tick = {}        # dst slot ap -> PE tick of last writing MM
            for inst in blk.instructions:
                nm = type(inst).__name__
                si = inst.sync_info
                if si is None:
                    continue
                if pe_tmpl is None:
                    for w in si.on_wait:
                        if (w.ant_name or "").startswith("PE_"):
                            pe_tmpl = w
                            break
                for u in si.on_update:
                    if (u.ant_name or "").startswith("PE_"):
                        pe_tick += u.update_value or 1
                if nm == "InstMatmult":
                    mm_tick[str(inst.outs[0])] = pe_tick
                    continue
                if nm != "InstTensorScalarPtr" or pe_tmpl is None:
                    continue
                need = mm_tick.get(str(inst.ins[0]))
                if need is None:
                    continue        # priming phase: keep serialized wait
                new_wait = []
                hit = False
                for w in si.on_wait:
                    if (w.ant_name or "").startswith("Activation"):
                        hit = True
                        new_wait.append(mybir.SyncWait(
                            sync_type=pe_tmpl.sync_type, id=pe_tmpl.id,
                            wait_mode=pe_tmpl.wait_mode,
                            ant_name=pe_tmpl.ant_name, wait_value=need))
                    else:
                        new_wait.append(w)
                if hit:
                    inst.sync_info = mybir.SyncInfo(
                        on_wait=new_wait, on_update=list(si.on_update))
                    swapped += 1
    return swapped


def _parallelize_act_stt(nc):
    """The Tile framework keeps a single linear dependency chain per
    PSUM tile, so the per-step DVE prep (STT) waits for the same-step
    tanh (ACT) even though both only READ the z slot.  Remove each
    STT -> ACT edge where the ACT's input AP equals the STT's in0 AP
    (reader-after-reader on the same slot), merging the ACT's own deps
    (the producing matmul) into the STT."""
    removed = 0
    for f in nc.m.functions:
        for blk in f.blocks:
            by_name = {}
            for inst in blk.instructions:
                by_name[inst.name] = inst
            for inst in blk.instructions:
                if type(inst).__name__ != "InstTensorScalarPtr":
                    continue
                src = str(inst.ins[0])
                for tname, _info in list(inst.sync_dependencies()):
                    dep = by_name.get(tname)
                    if dep is None or type(dep).__name__ != "InstActivation":
                        continue
                    if str(dep.ins[0]) != src:
                        continue
                    inst.remove_dependency(tname)
                    inst.merge_dependencies_from(dep)
                    removed += 1
    return removed


def build_nc(*, iters, cols, groups=3, ch=48, dedup=True,
             filler=0, cut=0, num_devices=NCORES):
    nc = bacc.Bacc("TRN2", target_bir_lowering=False, debug=False,
                   num_devices=num_devices)
    # Row stride must not be 8KB-aligned: DMA packets are spread over
    # engines by a source-address hash, and 2^k-aligned row strides make
    # every row hash to the SAME engine (observed: 103-row chunk loads
    # serialized onto one engine at ~31GB/s).  An odd element count
    # cycles row starts through all hash buckets.
    lp = iters * cols + 63
    aps = {
        "m_mat": nc.dram_tensor("m_mat", [K, K], F16,
                                kind="ExternalInput").ap(),
        "c_t": nc.dram_tensor("c_t", [K, lp], F16,
                              kind="ExternalInput").ap(),
        "o_out": nc.dram_tensor("o_out", [NB, lp], F16,
                                kind="ExternalOutput").ap(),
    }
    with tile.TileContext(nc) as tcx:
        emit_scan(tcx, nc, aps, iters=iters, cols=cols, groups=groups, ch=ch,
                  filler=filler, cut=cut)
    if dedup:
        _dedup_ldweights(nc)
        # with a single resident stationary, moving waits onto the one
        # surviving LDWEIGHTS would be wrong — keep waits on matmuls
        nc.move_matmul_waits_to_ldweights = lambda: None
    if cut == 3:
        # run the wait swap inside compile, right before the ISA
        # encodings are generated (post-compile edits to sync_info
        # don't reach the serialized program)
        orig_codegen = nc.codegen_inst_isa_subclasses

        def _patched_codegen():
            _swap_stt_waits(nc)
            orig_codegen()

        nc.codegen_inst_isa_subclasses = _patched_codegen
    nc.compile()
    return nc


def make_m_mat(recurrent_weights, input_weights, output_weights):
    """Device stationary, lhsT [K, K]: columns 0:100 produce the z
    update from the tanh rows (u-rows zeroed: the tanh(o) rhs rows must
    contribute nothing — the u drive arrives via the host C stream);
    columns 100:103 produce the out-IIR increment DT*W_out*th."""
    m = np.zeros((K, K), np.float32)
    m[:N, :N] = DT * recurrent_weights.T
    m[:N, N:] = DT * output_weights.T
    return np.ascontiguousarray(m).astype(np.float16)


def make_mz_host(recurrent_weights, input_weights, output_weights):
    """Host-side C projection [K, K]: rows 0:100 = DT*[W_rec, W_in]
    (the u path lives here), rows 100:103 = DT*[W_out, 0] (the readout
    noise drive)."""
    mz = np.zeros((K, K), np.float32)
    mz[:N, :N] = DT * recurrent_weights
    mz[:N, N:] = DT * input_weights
    mz[N:, :N] = DT * output_weights
    return mz


def make_v(inputs, noise, *, s, warm, iters, cols):
    """v stream [K, iters, cols] f32 for one chain (time shard).

    inputs [cols, T, NB], noise [cols, T, N] (batch-block slices).
    Iteration k=0 is the bootstrap block [0 ; u_{s-warm}/DT]; iteration
    k>=1 covers global step g = s - warm + k - 1 (g<0 -> zeros)."""
    v = np.zeros((K, iters, cols), np.float32)
    g0 = s - warm
    if 0 <= g0 < T:
        v[N:, 0] = inputs[:, g0].T / DT
    for k in range(1, iters):
        g = g0 + k - 1
        if g < 0 or g >= T:
            continue
        v[:N, k] = noise[:, g].T * (NOISE_STD / DT)
        un = inputs[:, g + 1].T if g + 1 < T else 0.0
        v[N:, k] = (un - DECAY * inputs[:, g].T) / DT
    return v


def make_c(v, mz):
    """Host C = Mz_aug v, fp16, [K, iters*cols + 63] (padded rows to
    keep the DRAM row stride off the DMA engine-hash pathology)."""
    k_, it, cols = v.shape
    c = np.zeros((K, it * cols + 63), np.float16)
    c[:, :it * cols] = (mz @ v.reshape(K, it * cols)).astype(np.float16)
    return c


def make_in_maps(inputs, noise, recurrent_weights, input_weights,
                 output_weights, *, groups, iters, plan):
    """Per-core input maps.  Core i runs chains i*groups..(i+1)*groups-1;
    chain q = (bblock, shard) = divmod(q, 2*groups).  The c stream per
    core interleaves its chains' 128-col blocks within each iteration."""
    m = make_m_mat(recurrent_weights, input_weights, output_weights)
    mz = make_mz_host(recurrent_weights, input_weights, output_weights)
    in_maps = []
    for core in range(NCORES):
        vs = []
        for g in range(groups):
            q = core * groups + g
            bb, sh = divmod(q, 2 * groups)
            bsl = slice(bb * GW, (bb + 1) * GW)
            ui = np.ascontiguousarray(inputs[bsl]).astype(np.float32)
            nz = np.ascontiguousarray(noise[bsl]).astype(np.float32)
            s, warm, r = plan[sh]
            vs.append(make_v(ui, nz, s=s, warm=warm, iters=iters, cols=GW))
        v = np.concatenate([vv[:, :, None, :] for vv in vs], axis=2)
        v = v.reshape(K, iters, groups * GW)
        in_maps.append({"m_mat": m, "c_t": make_c(v, mz)})
    return in_maps


def gather_out(results, *, groups, iters, plan):
    """Decode the tanh-encoded device out IIR: out = atanh(o_out)."""
    out = np.empty((B, T, NB), np.float32)
    cols = groups * GW
    for core in range(NCORES):
        o = results[core]["o_out"][:, :iters * cols].astype(np.float32)
        o = np.arctanh(np.clip(o, -0.999999, 0.999999))
        o = o.reshape(NB, iters, groups, GW)
        for g in range(groups):
            q = core * groups + g
            bb, sh = divmod(q, 2 * groups)
            bsl = slice(bb * GW, (bb + 1) * GW)
            s, warm, r = plan[sh]
            # o[:, k] = out IIR of iteration k-1 = global step s-warm+k-2
            out[bsl, s:s + r] = o[:, warm + 2:warm + 2 + r, g].transpose(2, 1, 0)
    return out


_NC_CACHE = {}


def kernel(inputs, noise, recurrent_weights, input_weights, output_weights,
           **run_kwargs):
    cfg = dict(run_kwargs.pop("cfg", {"filler": 0}))
    groups = cfg.setdefault("groups", 3)
    plan, iters = plan_for(groups)
    cfg.setdefault("iters", iters)
    cfg.setdefault("cols", groups * GW)
    key = tuple(sorted(cfg.items()))
    if key not in _NC_CACHE:
        _NC_CACHE[key] = build_nc(**cfg)
    nc = _NC_CACHE[key]
    in_maps = make_in_maps(inputs, noise, recurrent_weights,
                           input_weights, output_weights, groups=groups,
                           iters=cfg["iters"], plan=plan)
    res = run_bass_kernel_spmd(nc, in_maps, core_ids=list(range(NCORES)),
                               **run_kwargs)
    out = gather_out(res.results, groups=groups,
                     iters=cfg["iters"], plan=plan)
    if run_kwargs.get("trace"):
        return out, res
    return out


# revision 40
# speedup vs baseline: 3.7421x; 1.3255x over previous
"""Trainium2 Bass kernel for the ContinuousRNN problem.

Reference (per batch row b):
    h_0 = 0                               # [N], N=100
    z_t = W_rec h_t + W_in u_t
    h_{t+1} = 0.85 h_t + 0.15 tanh(z_t) + NOISE_STD noise_t
    out_t = W_out h_{t+1}

z-space reformulation (state z_t, N rows):
    z_{t+1} = 0.85 z_t + Mz (th_t + v_t)
      Mz   = 0.15 [W_rec, W_in]                          (100x103)
      th_t = [tanh(z_t) ; 0]
      v_t  = [noise_t*NS/DT ; (u_{t+1}-0.85 u_t)/DT]
    bootstrap (h=0): z_0 = Mz [0 ; u_0/DT] = W_in u_0
    out_t = 0.85 out_{t-1} + 0.15 W_out (tanh(z_t) + noise_t*NS/DT)
    (the out IIR is a linear readout of the device-produced tanh
    stream; it runs on host, exactly mirroring the device recurrence)

Per-core per-step critical path is two hops:
    MM_t -> { ACT tanh (psum->sbuf fp16)  ||  DVE prep z' = 0.85 z + C } -> MM_{t+1}
where MM_{t+1} accumulates (start=False) onto the DVE-prepped psum slot.
That accumulation works because each z bank is primed once by a
start=True matmul (sets the psum has_written bits, which non-PE writes
do not clear).  C_t = Mz v_t is host-precomputed and streamed.  The PE
never reloads weights (redundant LDWEIGHTS are deduped by a post-tile
pass).  The tanh stream drains from SBUF by DMA.

The Tile framework tracks PSUM deps as a single linear chain per tile
(each accessor waits for the previous accessor), which would serialize
ACT and DVE (both read the same z slot).  A post-pass
(_parallelize_act_stt) removes those reader-after-reader edges so ACT
and DVE truly run in parallel.

Sharding: each core runs `groups` independent chains of 128 batch
columns; chains are (batch-block, time-shard) pairs.  With 8 cores and
n chains/core there are 2n time shards over 4 batch blocks.  The RNN
contracts (~0.983/step), so time shards s>0 warm up for L steps from
h=0; shard 0 "warms up" on zero-padded inputs (exactly h=0).  All cores
run the identical SPMD program; host slices each chain's valid range.
"""

import sys

for _p in ("/opt/trn_rl_repo",):
    if _p not in sys.path:
        sys.path.insert(0, _p)

import numpy as np

import concourse.bass as bass
import concourse.bacc as bacc
import concourse.mybir as mybir
from concourse import tile
from concourse.bass_utils import run_bass_kernel_spmd

F32 = mybir.dt.float32
F16 = mybir.dt.float16

N = 100
NB = 3
K = N + NB        # 103 (matmul contraction: tanh rows + zero-padded u rows)
B = 512
T = 2048
NCORES = 8
DT = np.float32(0.15)
NOISE_STD = np.float32(0.015)
DECAY = np.float32(0.85)

GW = 128                # batch columns per chain
L_WARM = 224            # warmup steps for time shards > 0

NQ = 8                  # z slots (2 parity tiles x 4 slots per chain)
NTH = 32                # th ring slots (slack must exceed the o-drain
                        # DMA delay behind in-flight chunk transfers)
DRAIN = 8               # drain period (iters)


def plan_for(groups):
    """Shard plan for `groups` chains/core: S time shards, per-shard
    (start, warm, r), uniform ITERS."""
    S = 2 * groups
    A = -(-(T + (S - 1) * L_WARM) // S)     # ceil
    rs = [A] + [A - L_WARM] * (S - 1)
    excess = sum(rs) - T
    rs[-1] -= excess
    assert rs[-1] > 0
    plan = []
    s = 0
    for j in range(S):
        plan.append((s, 0 if j == 0 else L_WARM, rs[j]))
        s += rs[j]
    assert s == T
    # ITERS = A + 2: one bootstrap iteration up front, plus one trailing
    # iteration so the last output's o-state gets tanh'd and drained
    # (the drained o at iteration k is the out IIR of iteration k-1).
    return plan, A + 2


def emit_scan(tc, nc, aps, *, iters, cols, groups, ch, filler=0, cut=0):
    """aps: m_mat [K,M] f16 (lhsT, M=K with the 3 readout columns),
    c_t [M, iters*cols] f16 (iteration-major, host-computed C = Mz v),
    o_out [NB, iters*cols] f16 (the tanh-encoded out IIR).

    The stationary's columns 100:103 compute the out IIR readout
    p = DT*W_out*th into psum rows 100:103 of the same z slot; the STT
    applies the same 0.85 decay (out IIR == z decay), and the per-step
    tanh ACT covers rows 0:103, so tanh(out_t) rides the tht ring for
    free (tanh is invertible; host applies atanh).  The stationary's
    u-rows are zeroed so the tanh(o) rhs rows contribute nothing.

    PSUM dependency tracking is per-tile, so z state is split into
    per-(group x parity) psum tiles.  C streams through SBUF (walrus
    rejects TensorScalarPtr with all-PSUM operands, and SBUF tiles get
    fine-grained dep tracking)."""
    gw = cols // groups
    mult = mybir.AluOpType.mult
    add = mybir.AluOpType.add
    tanh = mybir.ActivationFunctionType.Tanh
    hq = NQ // 2           # z slots per parity tile

    cpool = tc.alloc_tile_pool(name="const", bufs=1)
    vpool = tc.alloc_tile_pool(name="cstream", bufs=3)
    tpool = tc.alloc_tile_pool(name="th", bufs=1)
    ppool = tc.alloc_tile_pool(name="psum", bufs=1, space="PSUM")

    wb = cpool.tile([K, K], F16, name="wb")
    nc.sync.dma_start(wb[:, :], aps["m_mat"][:, :])

    zt = cpool.tile([K, 512], F16, name="zt")   # zero rhs for priming
    nc.vector.memset(zt[:, :], 0.0)

    # z state: per (group, parity) psum tiles, hq slots of [K, gw] each
    qts = [[ppool.tile([128, hq * gw], F32, name=f"qt{g}p{par}")
            for par in range(2)] for g in range(groups)]
    # scratch bank for PE-warming filler matmuls
    fts = ppool.tile([128, 512], F32, name="fts") if filler else None

    # th ring (fp16); rows 0:100 tanh(z), rows 100:103 tanh(out IIR)
    tht = tpool.tile([K, NTH * cols], F16, name="tht")
    nc.vector.memset(tht[96:K, :], 0.0)

    # prime z tiles: start=True matmuls set has_written over all z slots
    for g in range(groups):
        for par in range(2):
            w = hq * gw
            assert w <= 512
            nc.tensor.matmul(qts[g][par][0:K, 0:w], wb[:, :],
                             zt[:, 0:w], start=True, stop=True)

    prev_mm = [None] * groups
    # C staging, double buffered
    ctiles = {}

    def c_chunk(ci):
        if ci * ch >= iters:
            return None
        if ci not in ctiles:
            tl = vpool.tile([K, ch * cols], F16, tag="cs", name=f"cs{ci}")
            hi = min((ci + 1) * ch, iters)
            w = (hi - ci * ch) * cols
            # split rows 0:100 / 100:103: the HWDGE spreads a DMA's
            # rows across engines only for round row counts (100 -> 10
            # engines, 3 -> 3); a 103-row DMA lands on ONE engine and
            # serializes the whole chunk load at ~31GB/s.
            nc.sync.dma_start(tl[0:N, 0:w],
                              aps["c_t"][0:N, ci * ch * cols:hi * cols])
            nc.sync.dma_start(tl[N:K, 0:w],
                              aps["c_t"][N:K, ci * ch * cols:hi * cols])
            ctiles[ci] = tl
        return ctiles[ci]

    c_chunk(0)
    c_chunk(1)

    def drain(k_lo, k_hi):
        """DMA the tanh(out) rows for iterations k_lo..k_hi (inclusive,
        contiguous in the ring) to DRAM."""
        c0 = (k_lo % NTH) * cols
        c1 = c0 + (k_hi - k_lo + 1) * cols
        nc.sync.dma_start(aps["o_out"][:, k_lo * cols:k_lo * cols + c1 - c0],
                          tht[N:K, c0:c1])

    for k in range(iters):
        ci = k // ch
        if k % ch == 0:
            c_chunk(ci + 2)   # two chunks ahead (triple buffered)
        cc = (k % ch) * cols
        ctile = ctiles[ci]

        qs = ((k // 2) % hq) * gw       # read slot col (parity k%2)
        qn = (((k + 1) // 2) % hq) * gw  # write slot col (parity (k+1)%2)
        tc0 = (k % NTH) * cols
        for g in range(groups):
            rd = qts[g][k % 2]
            wr = qts[g][(k + 1) % 2]
            # ACT: th = tanh(z | o) psum -> sbuf fp16
            act_inst = nc.scalar.activation(
                tht[0:K, tc0 + g * gw:tc0 + (g + 1) * gw],
                rd[0:K, qs:qs + gw], tanh)
            # DVE prep: z' = 0.85 z + C  (psum+sbuf -> psum, other parity)
            stt_inst = nc.vector.scalar_tensor_tensor(
                wr[0:K, qn:qn + gw], rd[0:K, qs:qs + gw],
                float(DECAY), ctile[0:K, cc + g * gw:cc + (g + 1) * gw],
                mult, add)
            # The tile framework keeps a single linear dependency chain
            # per PSUM tile, so the STT would wait for the same-step
            # tanh even though both only READ the z slot.  Cut that
            # reader-after-reader edge (inline: sem waits are assigned
            # from these edges at TileContext exit), inheriting the
            # ACT's own deps (the producing matmul).
            if cut == 1:
                # cut the edge, inherit the ACT's own deps
                ai, si = act_inst.ins, stt_inst.ins
                for tname, _info in list(si.sync_dependencies()):
                    if tname == ai.name:
                        si.remove_dependency(tname)
                        si.merge_dependencies_from(ai)
            elif cut == 2:
                # cut the edge, add a clean edge to the producing matmul
                ai, si = act_inst.ins, stt_inst.ins
                for tname, info in list(si.sync_dependencies()):
                    if tname == ai.name:
                        si.remove_dependency(tname)
                        if prev_mm[g] is not None:
                            si.add_dependency(prev_mm[g].ins.name, info)
            # chain MM accumulates onto the prepped slot
            prev_mm[g] = nc.tensor.matmul(
                wr[0:K, qn:qn + gw], wb[:, :],
                tht[0:K, tc0 + g * gw:tc0 + (g + 1) * gw],
                start=False, stop=True, skip_group_check=True)
        if filler:
            # keep the PE pipeline warm with a throwaway matmul
            nc.tensor.matmul(fts[0:K, 0:filler], wb[:, :], zt[:, 0:filler],
                             start=True, stop=True)

        if k % DRAIN == DRAIN - 1:
            drain(k - DRAIN + 1, k)
    # tail
    rem = iters % DRAIN
    if rem:
        drain(iters - rem, iters - 1)

    for p in (ppool, tpool, vpool, cpool):
        p.release()


def _dedup_ldweights(nc):
    """Remove legalizer-inserted LDWEIGHTS that reload an identical
    stationary; merge their deps into the following matmul."""
    removed = 0
    for f in nc.m.functions:
        for blk in f.blocks:
            insts = list(blk.instructions)
            last_key = None
            keep = []
            pending = []
            for inst in insts:
                nm = type(inst).__name__
                if nm == "InstLdweights":
                    key = (str(inst.ins[0]), str(inst.tile_position),
                           str(inst.perf_mode), bool(inst.is_transpose))
                    if key == last_key:
                        pending.append(inst)
                        removed += 1
                        continue
                    last_key = key
                    keep.append(inst)
                elif nm == "InstMatmult":
                    for ld in pending:
                        inst.merge_dependencies_from(ld)
                    pending = []
                    keep.append(inst)
                else:
                    keep.append(inst)
            assert not pending, "dangling removed LDWEIGHTS"
            if len(keep) != len(insts):
                blk.instructions = keep
    return removed


def _swap_stt_waits(nc):
    """Post-compile variant of the ACT/STT parallelization: the
    scheduler serialized STT(k) behind the same-step tanh ACT(k) (both
    only READ the z slot) and, by wait-dominance elision, materialized
    ONLY an Activation-sem wait on the STT.  Replace that wait with a
    copy of the paired ACT's own PE wait (the producing matmul), which
    is the STT's true dependency.  This edits only the final SyncInfo
    the hardware executes; scheduling/ticks/queues are untouched."""
    swapped = 0
    for f in nc.m.functions:
        for blk in f.blocks:
            pe_tick = 0
            pe_tmpl = None      # a SyncWait on the PE sem, as template
            mm_tick = {}        # dst slot ap -> PE tick of last writing MM
            for inst in blk.instructions:
                nm = type(inst).__name__
                si = inst.sync_info
                if si is None:
                    continue
                if pe_tmpl is None:
                    for w in si.on_wait:
                        if (w.ant_name or "").startswith("PE_"):
                            pe_tmpl = w
                            break
                for u in si.on_update:
                    if (u.ant_name or "").startswith("PE_"):
                        pe_tick += u.update_value or 1
                if nm == "InstMatmult":
                    mm_tick[str(inst.outs[0])] = pe_tick
                    continue
                if nm != "InstTensorScalarPtr" or pe_tmpl is None:
                    continue
                need = mm_tick.get(str(inst.ins[0]))
                if need is None:
                    continue        # priming phase: keep serialized wait
                new_wait = []
                hit = False
                for w in si.on_wait:
                    if (w.ant_name or "").startswith("Activation"):
                        hit = True
                        new_wait.append(mybir.SyncWait(
                            sync_type=pe_tmpl.sync_type, id=pe_tmpl.id,
                            wait_mode=pe_tmpl.wait_mode,
                            ant_name=pe_tmpl.ant_name, wait_value=need))
                    else:
                        new_wait.append(w)
                if hit:
                    inst.sync_info = mybir.SyncInfo(
                        on_wait=new_wait, on_update=list(si.on_update))
                    swapped += 1
    return swapped


def _parallelize_act_stt(nc):
    """The Tile framework keeps a single linear dependency chain per
    PSUM tile, so the per-step DVE prep (STT) waits for the same-step
    tanh (ACT) even though both only READ the z slot.  Remove each
    STT -> ACT edge where the ACT's input AP equals the STT's in0 AP
    (reader-after-reader on the same slot), merging the ACT's own deps
    (the producing matmul) into the STT."""
    removed = 0
    for f in nc.m.functions:
        for blk in f.blocks:
            by_name = {}
            for inst in blk.instructions:
                by_name[inst.name] = inst
            for inst in blk.instructions:
                if type(inst).__name__ != "InstTensorScalarPtr":
                    continue
                src = str(inst.ins[0])
                for tname, _info in list(inst.sync_dependencies()):
                    dep = by_name.get(tname)
                    if dep is None or type(dep).__name__ != "InstActivation":
                        continue
                    if str(dep.ins[0]) != src:
                        continue
                    inst.remove_dependency(tname)
                    inst.merge_dependencies_from(dep)
                    removed += 1
    return removed


def build_nc(*, iters, cols, groups=3, ch=48, dedup=True,
             filler=0, cut=0, num_devices=NCORES):
    nc = bacc.Bacc("TRN2", target_bir_lowering=False, debug=False,
                   num_devices=num_devices)
    # Row stride must not be 8KB-aligned: DMA packets are spread over
    # engines by a source-address hash, and 2^k-aligned row strides make
    # every row hash to the SAME engine (observed: 103-row chunk loads
    # serialized onto one engine at ~31GB/s).  An odd element count
    # cycles row starts through all hash buckets.
    lp = iters * cols + 63
    aps = {
        "m_mat": nc.dram_tensor("m_mat", [K, K], F16,
                                kind="ExternalInput").ap(),
        "c_t": nc.dram_tensor("c_t", [K, lp], F16,
                              kind="ExternalInput").ap(),
        "o_out": nc.dram_tensor("o_out", [NB, lp], F16,
                                kind="ExternalOutput").ap(),
    }
    with tile.TileContext(nc) as tcx:
        emit_scan(tcx, nc, aps, iters=iters, cols=cols, groups=groups, ch=ch,
                  filler=filler, cut=cut)
    if dedup:
        _dedup_ldweights(nc)
        # with a single resident stationary, moving waits onto the one
        # surviving LDWEIGHTS would be wrong — keep waits on matmuls
        nc.move_matmul_waits_to_ldweights = lambda: None
    if cut == 3:
        # run the wait swap inside compile, right before the ISA
        # encodings are generated (post-compile edits to sync_info
        # don't reach the serialized program)
        orig_codegen = nc.codegen_inst_isa_subclasses

        def _patched_codegen():
            _swap_stt_waits(nc)
            orig_codegen()

        nc.codegen_inst_isa_subclasses = _patched_codegen
    nc.compile()
    return nc


def make_m_mat(recurrent_weights, input_weights, output_weights):
    """Device stationary, lhsT [K, K]: columns 0:100 produce the z
    update from the tanh rows (u-rows zeroed: the tanh(o) rhs rows must
    contribute nothing — the u drive arrives via the host C stream);
    columns 100:103 produce the out-IIR increment DT*W_out*th."""
    m = np.zeros((K, K), np.float32)
    m[:N, :N] = DT * recurrent_weights.T
    m[:N, N:] = DT * output_weights.T
    return np.ascontiguousarray(m).astype(np.float16)


def make_mz_host(recurrent_weights, input_weights, output_weights):
    """Host-side C projection [K, K]: rows 0:100 = DT*[W_rec, W_in]
    (the u path lives here), rows 100:103 = DT*[W_out, 0] (the readout
    noise drive)."""
    mz = np.zeros((K, K), np.float32)
    mz[:N, :N] = DT * recurrent_weights
    mz[:N, N:] = DT * input_weights
    mz[N:, :N] = DT * output_weights
    return mz


def make_v(inputs, noise, *, s, warm, iters, cols):
    """v stream [K, iters, cols] f32 for one chain (time shard).

    inputs [cols, T, NB], noise [cols, T, N] (batch-block slices).
    Iteration k=0 is the bootstrap block [0 ; u_{s-warm}/DT]; iteration
    k>=1 covers global step g = s - warm + k - 1 (g<0 -> zeros)."""
    v = np.zeros((K, iters, cols), np.float32)
    g0 = s - warm
    if 0 <= g0 < T:
        v[N:, 0] = inputs[:, g0].T / DT
    for k in range(1, iters):
        g = g0 + k - 1
        if g < 0 or g >= T:
            continue
        v[:N, k] = noise[:, g].T * (NOISE_STD / DT)
        un = inputs[:, g + 1].T if g + 1 < T else 0.0
        v[N:, k] = (un - DECAY * inputs[:, g].T) / DT
    return v


def make_c(v, mz):
    """Host C = Mz_aug v, fp16, [K, iters*cols + 63] (padded rows to
    keep the DRAM row stride off the DMA engine-hash pathology)."""
    k_, it, cols = v.shape
    c = np.zeros((K, it * cols + 63), np.float16)
    c[:, :it * cols] = (mz @ v.reshape(K, it * cols)).astype(np.float16)
    return c


def make_in_maps(inputs, noise, recurrent_weights, input_weights,
                 output_weights, *, groups, iters, plan):
    """Per-core input maps.  Core i runs chains i*groups..(i+1)*groups-1;
    chain q = (bblock, shard) = divmod(q, 2*groups).  The c stream per
    core interleaves its chains' 128-col blocks within each iteration."""
    m = make_m_mat(recurrent_weights, input_weights, output_weights)
    mz = make_mz_host(recurrent_weights, input_weights, output_weights)
    in_maps = []
    for core in range(NCORES):
        vs = []
        for g in range(groups):
            q = core * groups + g
            bb, sh = divmod(q, 2 * groups)
            bsl = slice(bb * GW, (bb + 1) * GW)
            ui = np.ascontiguousarray(inputs[bsl]).astype(np.float32)
            nz = np.ascontiguousarray(noise[bsl]).astype(np.float32)
            s, warm, r = plan[sh]
            vs.append(make_v(ui, nz, s=s, warm=warm, iters=iters, cols=GW))
        v = np.concatenate([vv[:, :, None, :] for vv in vs], axis=2)
        v = v.reshape(K, iters, groups * GW)
        in_maps.append({"m_mat": m, "c_t": make_c(v, mz)})
    return in_maps


def gather_out(results, *, groups, iters, plan):
    """Decode the tanh-encoded device out IIR: out = atanh(o_out)."""
    out = np.empty((B, T, NB), np.float32)
    cols = groups * GW
    for core in range(NCORES):
        o = results[core]["o_out"][:, :iters * cols].astype(np.float32)
        o = np.arctanh(np.clip(o, -0.999999, 0.999999))
        o = o.reshape(NB, iters, groups, GW)
        for g in range(groups):
            q = core * groups + g
            bb, sh = divmod(q, 2 * groups)
            bsl = slice(bb * GW, (bb + 1) * GW)
            s, warm, r = plan[sh]
            # o[:, k] = out IIR of iteration k-1 = global step s-warm+k-2
            out[bsl, s:s + r] = o[:, warm + 2:warm + 2 + r, g].transpose(2, 1, 0)
    return out


_NC_CACHE = {}


def kernel(inputs, noise, recurrent_weights, input_weights, output_weights,
           **run_kwargs):
    cfg = dict(run_kwargs.pop("cfg", {"filler": 0}))
    groups = cfg.setdefault("groups", 3)
    plan, iters = plan_for(groups)
    cfg.setdefault("iters", iters)
    cfg.setdefault("cols", groups * GW)
    key = tuple(sorted(cfg.items()))
    if key not in _NC_CACHE:
        _NC_CACHE[key] = build_nc(**cfg)
    nc = _NC_CACHE[key]
    in_maps = make_in_maps(inputs, noise, recurrent_weights,
                           input_weights, output_weights, groups=groups,
                           iters=cfg["iters"], plan=plan)
    res = run_bass_kernel_spmd(nc, in_maps, core_ids=list(range(NCORES)),
                               **run_kwargs)
    out = gather_out(res.results, groups=groups,
                     iters=cfg["iters"], plan=plan)
    if run_kwargs.get("trace"):
        return out, res
    return out


# revision 42
# speedup vs baseline: 3.7551x; 1.0035x over previous
"""Trainium2 Bass kernel for the ContinuousRNN problem.

Reference (per batch row b):
    h_0 = 0                               # [N], N=100
    z_t = W_rec h_t + W_in u_t
    h_{t+1} = 0.85 h_t + 0.15 tanh(z_t) + NOISE_STD noise_t
    out_t = W_out h_{t+1}

z-space reformulation (state z_t, N rows):
    z_{t+1} = 0.85 z_t + Mz (th_t + v_t)
      Mz   = 0.15 [W_rec, W_in]                          (100x103)
      th_t = [tanh(z_t) ; 0]
      v_t  = [noise_t*NS/DT ; (u_{t+1}-0.85 u_t)/DT]
    bootstrap (h=0): z_0 = Mz [0 ; u_0/DT] = W_in u_0
    out_t = 0.85 out_{t-1} + 0.15 W_out (tanh(z_t) + noise_t*NS/DT)
    (the out IIR is a linear readout of the device-produced tanh
    stream; it runs on host, exactly mirroring the device recurrence)

Per-core per-step critical path is two hops:
    MM_t -> { ACT tanh (psum->sbuf fp16)  ||  DVE prep z' = 0.85 z + C } -> MM_{t+1}
where MM_{t+1} accumulates (start=False) onto the DVE-prepped psum slot.
That accumulation works because each z bank is primed once by a
start=True matmul (sets the psum has_written bits, which non-PE writes
do not clear).  C_t = Mz v_t is host-precomputed and streamed.  The PE
never reloads weights (redundant LDWEIGHTS are deduped by a post-tile
pass).  The tanh stream drains from SBUF by DMA.

The Tile framework tracks PSUM deps as a single linear chain per tile
(each accessor waits for the previous accessor), which would serialize
ACT and DVE (both read the same z slot).  A post-pass
(_parallelize_act_stt) removes those reader-after-reader edges so ACT
and DVE truly run in parallel.

Sharding: each core runs `groups` independent chains of 128 batch
columns; chains are (batch-block, time-shard) pairs.  With 8 cores and
n chains/core there are 2n time shards over 4 batch blocks.  The RNN
contracts (~0.983/step), so time shards s>0 warm up for L steps from
h=0; shard 0 "warms up" on zero-padded inputs (exactly h=0).  All cores
run the identical SPMD program; host slices each chain's valid range.
"""

import sys

for _p in ("/opt/trn_rl_repo",):
    if _p not in sys.path:
        sys.path.insert(0, _p)

import numpy as np

import concourse.bass as bass
import concourse.bacc as bacc
import concourse.mybir as mybir
from concourse import tile
from concourse.bass_utils import run_bass_kernel_spmd

F32 = mybir.dt.float32
F16 = mybir.dt.float16

N = 100
NB = 3
K = N + NB        # 103 (matmul contraction: tanh rows + zero-padded u rows)
B = 512
T = 2048
NCORES = 8
DT = np.float32(0.15)
NOISE_STD = np.float32(0.015)
DECAY = np.float32(0.85)

GW = 128                # batch columns per chain
L_WARM = 224            # warmup steps for time shards > 0

NQ = 8                  # z slots (2 parity tiles x 4 slots per chain)
NTH = 32                # th ring slots (slack must exceed the o-drain
                        # DMA delay behind in-flight chunk transfers)
DRAIN = 8               # drain period (iters)


def plan_for(groups):
    """Shard plan for `groups` chains/core: S time shards, per-shard
    (start, warm, r), uniform ITERS."""
    S = 2 * groups
    A = -(-(T + (S - 1) * L_WARM) // S)     # ceil
    rs = [A] + [A - L_WARM] * (S - 1)
    excess = sum(rs) - T
    rs[-1] -= excess
    assert rs[-1] > 0
    plan = []
    s = 0
    for j in range(S):
        plan.append((s, 0 if j == 0 else L_WARM, rs[j]))
        s += rs[j]
    assert s == T
    # ITERS = A + 2: one bootstrap iteration up front, plus one trailing
    # iteration so the last output's o-state gets tanh'd and drained
    # (the drained o at iteration k is the out IIR of iteration k-1).
    return plan, A + 2


def emit_scan(tc, nc, aps, *, iters, cols, groups, ch, filler=0, cut=0):
    """aps: m_mat [K,M] f16 (lhsT, M=K with the 3 readout columns),
    c_t [M, iters*cols] f16 (iteration-major, host-computed C = Mz v),
    o_out [NB, iters*cols] f16 (the tanh-encoded out IIR).

    The stationary's columns 100:103 compute the out IIR readout
    p = DT*W_out*th into psum rows 100:103 of the same z slot; the STT
    applies the same 0.85 decay (out IIR == z decay), and the per-step
    tanh ACT covers rows 0:103, so tanh(out_t) rides the tht ring for
    free (tanh is invertible; host applies atanh).  The stationary's
    u-rows are zeroed so the tanh(o) rhs rows contribute nothing.

    PSUM dependency tracking is per-tile, so z state is split into
    per-(group x parity) psum tiles.  C streams through SBUF (walrus
    rejects TensorScalarPtr with all-PSUM operands, and SBUF tiles get
    fine-grained dep tracking)."""
    gw = cols // groups
    mult = mybir.AluOpType.mult
    add = mybir.AluOpType.add
    tanh = mybir.ActivationFunctionType.Tanh
    hq = NQ // 2           # z slots per parity tile

    cpool = tc.alloc_tile_pool(name="const", bufs=1)
    vpool = tc.alloc_tile_pool(name="cstream", bufs=3)
    tpool = tc.alloc_tile_pool(name="th", bufs=1)
    ppool = tc.alloc_tile_pool(name="psum", bufs=1, space="PSUM")

    wb = cpool.tile([K, K], F16, name="wb")
    nc.sync.dma_start(wb[:, :], aps["m_mat"][:, :])

    zt = cpool.tile([K, 512], F16, name="zt")   # zero rhs for priming
    nc.vector.memset(zt[:, :], 0.0)

    # z state: per (group, parity) psum tiles, hq slots of [K, gw] each
    qts = [[ppool.tile([128, hq * gw], F32, name=f"qt{g}p{par}")
            for par in range(2)] for g in range(groups)]
    # scratch bank for PE-warming filler matmuls
    fts = ppool.tile([128, 512], F32, name="fts") if filler else None

    # th ring (fp16); rows 0:100 tanh(z), rows 100:103 tanh(out IIR)
    tht = tpool.tile([K, NTH * cols], F16, name="tht")
    nc.vector.memset(tht[96:K, :], 0.0)

    # prime z tiles: start=True matmuls set has_written over all z slots
    for g in range(groups):
        for par in range(2):
            w = hq * gw
            assert w <= 512
            nc.tensor.matmul(qts[g][par][0:K, 0:w], wb[:, :],
                             zt[:, 0:w], start=True, stop=True)

    prev_mm = [None] * groups
    # C staging, double buffered
    ctiles = {}

    def c_chunk(ci):
        if ci * ch >= iters:
            return None
        if ci not in ctiles:
            tl = vpool.tile([K, ch * cols], F16, tag="cs", name=f"cs{ci}")
            hi = min((ci + 1) * ch, iters)
            w = (hi - ci * ch) * cols
            # split rows 0:100 / 100:103: the HWDGE spreads a DMA's
            # rows across engines only for round row counts (100 -> 10
            # engines, 3 -> 3); a 103-row DMA lands on ONE engine and
            # serializes the whole chunk load at ~31GB/s.
            nc.sync.dma_start(tl[0:N, 0:w],
                              aps["c_t"][0:N, ci * ch * cols:hi * cols])
            nc.sync.dma_start(tl[N:K, 0:w],
                              aps["c_t"][N:K, ci * ch * cols:hi * cols])
            ctiles[ci] = tl
        return ctiles[ci]

    c_chunk(0)
    c_chunk(1)

    def drain(k_lo, k_hi):
        """DMA the tanh(out) rows for iterations k_lo..k_hi (inclusive,
        contiguous in the ring) to DRAM."""
        c0 = (k_lo % NTH) * cols
        c1 = c0 + (k_hi - k_lo + 1) * cols
        nc.sync.dma_start(aps["o_out"][:, k_lo * cols:k_lo * cols + c1 - c0],
                          tht[N:K, c0:c1])

    for k in range(iters):
        ci = k // ch
        if k % ch == 0:
            c_chunk(ci + 2)   # two chunks ahead (triple buffered)
        cc = (k % ch) * cols
        ctile = ctiles[ci]

        qs = ((k // 2) % hq) * gw       # read slot col (parity k%2)
        qn = (((k + 1) // 2) % hq) * gw  # write slot col (parity (k+1)%2)
        tc0 = (k % NTH) * cols
        for g in range(groups):
            rd = qts[g][k % 2]
            wr = qts[g][(k + 1) % 2]
            # ACT: th = tanh(z | o) psum -> sbuf fp16
            act_inst = nc.scalar.activation(
                tht[0:K, tc0 + g * gw:tc0 + (g + 1) * gw],
                rd[0:K, qs:qs + gw], tanh)
            # DVE prep: z' = 0.85 z + C  (psum+sbuf -> psum, other parity)
            stt_inst = nc.vector.scalar_tensor_tensor(
                wr[0:K, qn:qn + gw], rd[0:K, qs:qs + gw],
                float(DECAY), ctile[0:K, cc + g * gw:cc + (g + 1) * gw],
                mult, add)
            # The tile framework keeps a single linear dependency chain
            # per PSUM tile, so the STT would wait for the same-step
            # tanh even though both only READ the z slot.  Cut that
            # reader-after-reader edge (inline: sem waits are assigned
            # from these edges at TileContext exit), inheriting the
            # ACT's own deps (the producing matmul).
            if cut == 1:
                # cut the edge, inherit the ACT's own deps
                ai, si = act_inst.ins, stt_inst.ins
                for tname, _info in list(si.sync_dependencies()):
                    if tname == ai.name:
                        si.remove_dependency(tname)
                        si.merge_dependencies_from(ai)
            elif cut == 2:
                # cut the edge, add a clean edge to the producing matmul
                ai, si = act_inst.ins, stt_inst.ins
                for tname, info in list(si.sync_dependencies()):
                    if tname == ai.name:
                        si.remove_dependency(tname)
                        if prev_mm[g] is not None:
                            si.add_dependency(prev_mm[g].ins.name, info)
            # chain MM accumulates onto the prepped slot
            prev_mm[g] = nc.tensor.matmul(
                wr[0:K, qn:qn + gw], wb[:, :],
                tht[0:K, tc0 + g * gw:tc0 + (g + 1) * gw],
                start=False, stop=True, skip_group_check=True)
        if filler:
            # keep the PE pipeline warm with a throwaway matmul
            nc.tensor.matmul(fts[0:K, 0:filler], wb[:, :], zt[:, 0:filler],
                             start=True, stop=True)

        if k % DRAIN == DRAIN - 1:
            drain(k - DRAIN + 1, k)
    # tail
    rem = iters % DRAIN
    if rem:
        drain(iters - rem, iters - 1)

    for p in (ppool, tpool, vpool, cpool):
        p.release()


def _dedup_ldweights(nc):
    """Remove legalizer-inserted LDWEIGHTS that reload an identical
    stationary; merge their deps into the following matmul."""
    removed = 0
    for f in nc.m.functions:
        for blk in f.blocks:
            insts = list(blk.instructions)
            last_key = None
            keep = []
            pending = []
            for inst in insts:
                nm = type(inst).__name__
                if nm == "InstLdweights":
                    key = (str(inst.ins[0]), str(inst.tile_position),
                           str(inst.perf_mode), bool(inst.is_transpose))
                    if key == last_key:
                        pending.append(inst)
                        removed += 1
                        continue
                    last_key = key
                    keep.append(inst)
                elif nm == "InstMatmult":
                    for ld in pending:
                        inst.merge_dependencies_from(ld)
                    pending = []
                    keep.append(inst)
                else:
                    keep.append(inst)
            assert not pending, "dangling removed LDWEIGHTS"
            if len(keep) != len(insts):
                blk.instructions = keep
    return removed


def _swap_stt_waits(nc):
    """Post-compile variant of the ACT/STT parallelization: the
    scheduler serialized STT(k) behind the same-step tanh ACT(k) (both
    only READ the z slot) and, by wait-dominance elision, materialized
    ONLY an Activation-sem wait on the STT.  Replace that wait with a
    copy of the paired ACT's own PE wait (the producing matmul), which
    is the STT's true dependency.  This edits only the final SyncInfo
    the hardware executes; scheduling/ticks/queues are untouched."""
    swapped = 0
    for f in nc.m.functions:
        for blk in f.blocks:
            pe_tick = 0
            pe_tmpl = None      # a SyncWait on the PE sem, as template
            mm_tick = {}        # dst slot ap -> PE tick of last writing MM
            for inst in blk.instructions:
                nm = type(inst).__name__
                si = inst.sync_info
                if si is None:
                    continue
                if pe_tmpl is None:
                    for w in si.on_wait:
                        if (w.ant_name or "").startswith("PE_"):
                            pe_tmpl = w
                            break
                for u in si.on_update:
                    if (u.ant_name or "").startswith("PE_"):
                        pe_tick += u.update_value or 1
                if nm == "InstMatmult":
                    mm_tick[str(inst.outs[0])] = pe_tick
                    continue
                if nm != "InstTensorScalarPtr" or pe_tmpl is None:
                    continue
                need = mm_tick.get(str(inst.ins[0]))
                if need is None:
                    continue        # priming phase: keep serialized wait
                new_wait = []
                hit = False
                for w in si.on_wait:
                    if (w.ant_name or "").startswith("Activation"):
                        hit = True
                        new_wait.append(mybir.SyncWait(
                            sync_type=pe_tmpl.sync_type, id=pe_tmpl.id,
                            wait_mode=pe_tmpl.wait_mode,
                            ant_name=pe_tmpl.ant_name, wait_value=need))
                    else:
                        new_wait.append(w)
                if hit:
                    inst.sync_info = mybir.SyncInfo(
                        on_wait=new_wait, on_update=list(si.on_update))
                    swapped += 1
    return swapped


def _parallelize_act_stt(nc):
    """The Tile framework keeps a single linear dependency chain per
    PSUM tile, so the per-step DVE prep (STT) waits for the same-step
    tanh (ACT) even though both only READ the z slot.  Remove each
    STT -> ACT edge where the ACT's input AP equals the STT's in0 AP
    (reader-after-reader on the same slot), merging the ACT's own deps
    (the producing matmul) into the STT."""
    removed = 0
    for f in nc.m.functions:
        for blk in f.blocks:
            by_name = {}
            for inst in blk.instructions:
                by_name[inst.name] = inst
            for inst in blk.instructions:
                if type(inst).__name__ != "InstTensorScalarPtr":
                    continue
                src = str(inst.ins[0])
                for tname, _info in list(inst.sync_dependencies()):
                    dep = by_name.get(tname)
                    if dep is None or type(dep).__name__ != "InstActivation":
                        continue
                    if str(dep.ins[0]) != src:
                        continue
                    inst.remove_dependency(tname)
                    inst.merge_dependencies_from(dep)
                    removed += 1
    return removed


def build_nc(*, iters, cols, groups=4, ch=40, dedup=True,
             filler=0, cut=0, num_devices=NCORES):
    nc = bacc.Bacc("TRN2", target_bir_lowering=False, debug=False,
                   num_devices=num_devices)
    # Row stride must not be 8KB-aligned: DMA packets are spread over
    # engines by a source-address hash, and 2^k-aligned row strides make
    # every row hash to the SAME engine (observed: 103-row chunk loads
    # serialized onto one engine at ~31GB/s).  An odd element count
    # cycles row starts through all hash buckets.
    lp = iters * cols + 63
    aps = {
        "m_mat": nc.dram_tensor("m_mat", [K, K], F16,
                                kind="ExternalInput").ap(),
        "c_t": nc.dram_tensor("c_t", [K, lp], F16,
                              kind="ExternalInput").ap(),
        "o_out": nc.dram_tensor("o_out", [NB, lp], F16,
                                kind="ExternalOutput").ap(),
    }
    with tile.TileContext(nc) as tcx:
        emit_scan(tcx, nc, aps, iters=iters, cols=cols, groups=groups, ch=ch,
                  filler=filler, cut=cut)
    if dedup:
        _dedup_ldweights(nc)
        # with a single resident stationary, moving waits onto the one
        # surviving LDWEIGHTS would be wrong — keep waits on matmuls
        nc.move_matmul_waits_to_ldweights = lambda: None
    if cut == 3:
        # run the wait swap inside compile, right before the ISA
        # encodings are generated (post-compile edits to sync_info
        # don't reach the serialized program)
        orig_codegen = nc.codegen_inst_isa_subclasses

        def _patched_codegen():
            _swap_stt_waits(nc)
            orig_codegen()

        nc.codegen_inst_isa_subclasses = _patched_codegen
    nc.compile()
    return nc


def make_m_mat(recurrent_weights, input_weights, output_weights):
    """Device stationary, lhsT [K, K]: columns 0:100 produce the z
    update from the tanh rows (u-rows zeroed: the tanh(o) rhs rows must
    contribute nothing — the u drive arrives via the host C stream);
    columns 100:103 produce the out-IIR increment DT*W_out*th."""
    m = np.zeros((K, K), np.float32)
    m[:N, :N] = DT * recurrent_weights.T
    m[:N, N:] = DT * output_weights.T
    return np.ascontiguousarray(m).astype(np.float16)


def make_mz_host(recurrent_weights, input_weights, output_weights):
    """Host-side C projection [K, K]: rows 0:100 = DT*[W_rec, W_in]
    (the u path lives here), rows 100:103 = DT*[W_out, 0] (the readout
    noise drive)."""
    mz = np.zeros((K, K), np.float32)
    mz[:N, :N] = DT * recurrent_weights
    mz[:N, N:] = DT * input_weights
    mz[N:, :N] = DT * output_weights
    return mz


def make_v(inputs, noise, *, s, warm, iters, cols):
    """v stream [K, iters, cols] f32 for one chain (time shard).

    inputs [cols, T, NB], noise [cols, T, N] (batch-block slices).
    Iteration k=0 is the bootstrap block [0 ; u_{s-warm}/DT]; iteration
    k>=1 covers global step g = s - warm + k - 1 (g<0 -> zeros)."""
    v = np.zeros((K, iters, cols), np.float32)
    g0 = s - warm
    if 0 <= g0 < T:
        v[N:, 0] = inputs[:, g0].T / DT
    for k in range(1, iters):
        g = g0 + k - 1
        if g < 0 or g >= T:
            continue
        v[:N, k] = noise[:, g].T * (NOISE_STD / DT)
        un = inputs[:, g + 1].T if g + 1 < T else 0.0
        v[N:, k] = (un - DECAY * inputs[:, g].T) / DT
    return v


def make_c(v, mz):
    """Host C = Mz_aug v, fp16, [K, iters*cols + 63] (padded rows to
    keep the DRAM row stride off the DMA engine-hash pathology)."""
    k_, it, cols = v.shape
    c = np.zeros((K, it * cols + 63), np.float16)
    c[:, :it * cols] = (mz @ v.reshape(K, it * cols)).astype(np.float16)
    return c


def make_in_maps(inputs, noise, recurrent_weights, input_weights,
                 output_weights, *, groups, iters, plan):
    """Per-core input maps.  Core i runs chains i*groups..(i+1)*groups-1;
    chain q = (bblock, shard) = divmod(q, 2*groups).  The c stream per
    core interleaves its chains' 128-col blocks within each iteration."""
    m = make_m_mat(recurrent_weights, input_weights, output_weights)
    mz = make_mz_host(recurrent_weights, input_weights, output_weights)
    in_maps = []
    for core in range(NCORES):
        vs = []
        for g in range(groups):
            q = core * groups + g
            bb, sh = divmod(q, 2 * groups)
            bsl = slice(bb * GW, (bb + 1) * GW)
            ui = np.ascontiguousarray(inputs[bsl]).astype(np.float32)
            nz = np.ascontiguousarray(noise[bsl]).astype(np.float32)
            s, warm, r = plan[sh]
            vs.append(make_v(ui, nz, s=s, warm=warm, iters=iters, cols=GW))
        v = np.concatenate([vv[:, :, None, :] for vv in vs], axis=2)
        v = v.reshape(K, iters, groups * GW)
        in_maps.append({"m_mat": m, "c_t": make_c(v, mz)})
    return in_maps


def gather_out(results, *, groups, iters, plan):
    """Decode the tanh-encoded device out IIR: out = atanh(o_out)."""
    out = np.empty((B, T, NB), np.float32)
    cols = groups * GW
    for core in range(NCORES):
        o = results[core]["o_out"][:, :iters * cols].astype(np.float32)
        o = np.arctanh(np.clip(o, -0.999999, 0.999999))
        o = o.reshape(NB, iters, groups, GW)
        for g in range(groups):
            q = core * groups + g
            bb, sh = divmod(q, 2 * groups)
            bsl = slice(bb * GW, (bb + 1) * GW)
            s, warm, r = plan[sh]
            # o[:, k] = out IIR of iteration k-1 = global step s-warm+k-2
            out[bsl, s:s + r] = o[:, warm + 2:warm + 2 + r, g].transpose(2, 1, 0)
    return out


_NC_CACHE = {}


def kernel(inputs, noise, recurrent_weights, input_weights, output_weights,
           **run_kwargs):
    cfg = dict(run_kwargs.pop("cfg", {"filler": 0}))
    groups = cfg.setdefault("groups", 4)
    plan, iters = plan_for(groups)
    cfg.setdefault("iters", iters)
    cfg.setdefault("cols", groups * GW)
    key = tuple(sorted(cfg.items()))
    if key not in _NC_CACHE:
        _NC_CACHE[key] = build_nc(**cfg)
    nc = _NC_CACHE[key]
    in_maps = make_in_maps(inputs, noise, recurrent_weights,
                           input_weights, output_weights, groups=groups,
                           iters=cfg["iters"], plan=plan)
    res = run_bass_kernel_spmd(nc, in_maps, core_ids=list(range(NCORES)),
                               **run_kwargs)
    out = gather_out(res.results, groups=groups,
                     iters=cfg["iters"], plan=plan)
    if run_kwargs.get("trace"):
        return out, res
    return out


# revision 49
# speedup vs baseline: 3.8046x; 1.0132x over previous
"""Trainium2 Bass kernel for the ContinuousRNN problem.

Reference (per batch row b):
    h_0 = 0                               # [N], N=100
    z_t = W_rec h_t + W_in u_t
    h_{t+1} = 0.85 h_t + 0.15 tanh(z_t) + NOISE_STD noise_t
    out_t = W_out h_{t+1}

z-space reformulation (state z_t, N rows):
    z_{t+1} = 0.85 z_t + Mz (th_t + v_t)
      Mz   = 0.15 [W_rec, W_in]                          (100x103)
      th_t = [tanh(z_t) ; 0]
      v_t  = [noise_t*NS/DT ; (u_{t+1}-0.85 u_t)/DT]
    bootstrap (h=0): z_0 = Mz [0 ; u_0/DT] = W_in u_0
    out_t = 0.85 out_{t-1} + 0.15 W_out (tanh(z_t) + noise_t*NS/DT)
    (the out IIR is a linear readout of the device-produced tanh
    stream; it runs on host, exactly mirroring the device recurrence)

Per-core per-step critical path is two hops:
    MM_t -> { ACT tanh (psum->sbuf fp16)  ||  DVE prep z' = 0.85 z + C } -> MM_{t+1}
where MM_{t+1} accumulates (start=False) onto the DVE-prepped psum slot.
That accumulation works because each z bank is primed once by a
start=True matmul (sets the psum has_written bits, which non-PE writes
do not clear).  C_t = Mz v_t is host-precomputed and streamed.  The PE
never reloads weights (redundant LDWEIGHTS are deduped by a post-tile
pass).  The tanh stream drains from SBUF by DMA.

The Tile framework tracks PSUM deps as a single linear chain per tile
(each accessor waits for the previous accessor), which would serialize
ACT and DVE (both read the same z slot).  A post-pass
(_parallelize_act_stt) removes those reader-after-reader edges so ACT
and DVE truly run in parallel.

Sharding: each core runs `groups` independent chains of 128 batch
columns; chains are (batch-block, time-shard) pairs.  With 8 cores and
n chains/core there are 2n time shards over 4 batch blocks.  The RNN
contracts (~0.983/step), so time shards s>0 warm up for L steps from
h=0; shard 0 "warms up" on zero-padded inputs (exactly h=0).  All cores
run the identical SPMD program; host slices each chain's valid range.
"""

import sys

for _p in ("/opt/trn_rl_repo",):
    if _p not in sys.path:
        sys.path.insert(0, _p)

import numpy as np

import concourse.bass as bass
import concourse.bacc as bacc
import concourse.mybir as mybir
from concourse import tile
from concourse.bass_utils import run_bass_kernel_spmd

F32 = mybir.dt.float32
F16 = mybir.dt.float16

N = 100
NB = 3
K = N + NB        # 103 (matmul contraction: tanh rows + zero-padded u rows)
B = 512
T = 2048
NCORES = 8
DT = np.float32(0.15)
NOISE_STD = np.float32(0.015)
DECAY = np.float32(0.85)

GW = 128                # batch columns per chain
L_WARM = 216            # warmup steps for time shards > 0

NQ = 8                  # z slots (2 parity tiles x 4 slots per chain)
NTH = 32                # th ring slots (slack must exceed the o-drain
                        # DMA delay behind in-flight chunk transfers)
DRAIN = 8               # drain period (iters)


def plan_for(groups):
    """Shard plan for `groups` chains/core: S time shards, per-shard
    (start, warm, r), uniform ITERS."""
    S = 2 * groups
    A = -(-(T + (S - 1) * L_WARM) // S)     # ceil
    rs = [A] + [A - L_WARM] * (S - 1)
    excess = sum(rs) - T
    rs[-1] -= excess
    assert rs[-1] > 0
    plan = []
    s = 0
    for j in range(S):
        plan.append((s, 0 if j == 0 else L_WARM, rs[j]))
        s += rs[j]
    assert s == T
    # ITERS = A + 2: one bootstrap iteration up front, plus one trailing
    # iteration so the last output's o-state gets tanh'd and drained
    # (the drained o at iteration k is the out IIR of iteration k-1).
    return plan, A + 2


def emit_scan(tc, nc, aps, *, iters, cols, groups, ch, filler=0, cut=0,
              stt_pool=0):
    """aps: m_mat [K,M] f16 (lhsT, M=K with the 3 readout columns),
    c_t [M, iters*cols] f16 (iteration-major, host-computed C = Mz v),
    o_out [NB, iters*cols] f16 (the tanh-encoded out IIR).

    The stationary's columns 100:103 compute the out IIR readout
    p = DT*W_out*th into psum rows 100:103 of the same z slot; the STT
    applies the same 0.85 decay (out IIR == z decay), and the per-step
    tanh ACT covers rows 0:103, so tanh(out_t) rides the tht ring for
    free (tanh is invertible; host applies atanh).  The stationary's
    u-rows are zeroed so the tanh(o) rhs rows contribute nothing.

    PSUM dependency tracking is per-tile, so z state is split into
    per-(group x parity) psum tiles.  C streams through SBUF (walrus
    rejects TensorScalarPtr with all-PSUM operands, and SBUF tiles get
    fine-grained dep tracking)."""
    gw = cols // groups
    mult = mybir.AluOpType.mult
    add = mybir.AluOpType.add
    tanh = mybir.ActivationFunctionType.Tanh
    hq = NQ // 2           # z slots per parity tile

    cpool = tc.alloc_tile_pool(name="const", bufs=1)
    vpool = tc.alloc_tile_pool(name="cstream", bufs=3)
    tpool = tc.alloc_tile_pool(name="th", bufs=1)
    ppool = tc.alloc_tile_pool(name="psum", bufs=1, space="PSUM")

    wb = cpool.tile([K, K], F16, name="wb")
    nc.sync.dma_start(wb[:, :], aps["m_mat"][:, :])

    zt = cpool.tile([K, 512], F16, name="zt")   # zero rhs for priming
    nc.vector.memset(zt[:, :], 0.0)

    # z state: per (group, parity) psum tiles, hq slots of [K, gw] each
    qts = [[ppool.tile([128, hq * gw], F32, name=f"qt{g}p{par}")
            for par in range(2)] for g in range(groups)]
    # scratch bank for PE-warming filler matmuls
    fts = ppool.tile([128, 512], F32, name="fts") if filler else None

    # th ring (fp16); rows 0:100 tanh(z), rows 100:103 tanh(out IIR)
    tht = tpool.tile([K, NTH * cols], F16, name="tht")
    nc.vector.memset(tht[96:K, :], 0.0)

    # prime z tiles: start=True matmuls set has_written over all z slots
    for g in range(groups):
        for par in range(2):
            w = hq * gw
            assert w <= 512
            nc.tensor.matmul(qts[g][par][0:K, 0:w], wb[:, :],
                             zt[:, 0:w], start=True, stop=True)

    prev_mm = [None] * groups
    # C staging, double buffered
    ctiles = {}

    def c_chunk(ci):
        if ci * ch >= iters:
            return None
        if ci not in ctiles:
            tl = vpool.tile([K, ch * cols], F16, tag="cs", name=f"cs{ci}")
            hi = min((ci + 1) * ch, iters)
            w = (hi - ci * ch) * cols
            # split rows 0:100 / 100:103: the HWDGE spreads a DMA's
            # rows across engines only for round row counts (100 -> 10
            # engines, 3 -> 3); a 103-row DMA lands on ONE engine and
            # serializes the whole chunk load at ~31GB/s.
            nc.sync.dma_start(tl[0:N, 0:w],
                              aps["c_t"][0:N, ci * ch * cols:hi * cols])
            nc.sync.dma_start(tl[N:K, 0:w],
                              aps["c_t"][N:K, ci * ch * cols:hi * cols])
            ctiles[ci] = tl
        return ctiles[ci]

    c_chunk(0)
    c_chunk(1)

    def drain(k_lo, k_hi):
        """DMA the tanh(out) rows for iterations k_lo..k_hi (inclusive,
        contiguous in the ring) to DRAM."""
        c0 = (k_lo % NTH) * cols
        c1 = c0 + (k_hi - k_lo + 1) * cols
        nc.sync.dma_start(aps["o_out"][:, k_lo * cols:k_lo * cols + c1 - c0],
                          tht[N:K, c0:c1])

    for k in range(iters):
        ci = k // ch
        if k % ch == 0:
            c_chunk(ci + 2)   # two chunks ahead (triple buffered)
        cc = (k % ch) * cols
        ctile = ctiles[ci]

        qs = ((k // 2) % hq) * gw       # read slot col (parity k%2)
        qn = (((k + 1) // 2) % hq) * gw  # write slot col (parity (k+1)%2)
        tc0 = (k % NTH) * cols
        for g in range(groups):
            rd = qts[g][k % 2]
            wr = qts[g][(k + 1) % 2]
            # ACT: th = tanh(z | o) psum -> sbuf fp16
            act_inst = nc.scalar.activation(
                tht[0:K, tc0 + g * gw:tc0 + (g + 1) * gw],
                rd[0:K, qs:qs + gw], tanh)
            # prep: z' = 0.85 z + C  (psum+sbuf -> psum, other parity);
            # on DVE by default, optionally on the idle Pool engine
            # (1.2GHz vs DVE 0.96GHz) to shorten the serial chain
            eng = nc.gpsimd if stt_pool else nc.vector
            stt_inst = eng.scalar_tensor_tensor(
                wr[0:K, qn:qn + gw], rd[0:K, qs:qs + gw],
                float(DECAY), ctile[0:K, cc + g * gw:cc + (g + 1) * gw],
                mult, add)
            # The tile framework keeps a single linear dependency chain
            # per PSUM tile, so the STT would wait for the same-step
            # tanh even though both only READ the z slot.  Cut that
            # reader-after-reader edge (inline: sem waits are assigned
            # from these edges at TileContext exit), inheriting the
            # ACT's own deps (the producing matmul).
            if cut == 1:
                # cut the edge, inherit the ACT's own deps
                ai, si = act_inst.ins, stt_inst.ins
                for tname, _info in list(si.sync_dependencies()):
                    if tname == ai.name:
                        si.remove_dependency(tname)
                        si.merge_dependencies_from(ai)
            elif cut == 2:
                # cut the edge, add a clean edge to the producing matmul
                ai, si = act_inst.ins, stt_inst.ins
                for tname, info in list(si.sync_dependencies()):
                    if tname == ai.name:
                        si.remove_dependency(tname)
                        if prev_mm[g] is not None:
                            si.add_dependency(prev_mm[g].ins.name, info)
            # chain MM accumulates onto the prepped slot
            prev_mm[g] = nc.tensor.matmul(
                wr[0:K, qn:qn + gw], wb[:, :],
                tht[0:K, tc0 + g * gw:tc0 + (g + 1) * gw],
                start=False, stop=True, skip_group_check=True)
        if filler:
            # keep the PE pipeline warm with a throwaway matmul
            nc.tensor.matmul(fts[0:K, 0:filler], wb[:, :], zt[:, 0:filler],
                             start=True, stop=True)

        if k % DRAIN == DRAIN - 1:
            drain(k - DRAIN + 1, k)
    # tail
    rem = iters % DRAIN
    if rem:
        drain(iters - rem, iters - 1)

    for p in (ppool, tpool, vpool, cpool):
        p.release()


def _dedup_ldweights(nc):
    """Remove legalizer-inserted LDWEIGHTS that reload an identical
    stationary; merge their deps into the following matmul."""
    removed = 0
    for f in nc.m.functions:
        for blk in f.blocks:
            insts = list(blk.instructions)
            last_key = None
            keep = []
            pending = []
            for inst in insts:
                nm = type(inst).__name__
                if nm == "InstLdweights":
                    key = (str(inst.ins[0]), str(inst.tile_position),
                           str(inst.perf_mode), bool(inst.is_transpose))
                    if key == last_key:
                        pending.append(inst)
                        removed += 1
                        continue
                    last_key = key
                    keep.append(inst)
                elif nm == "InstMatmult":
                    for ld in pending:
                        inst.merge_dependencies_from(ld)
                    pending = []
                    keep.append(inst)
                else:
                    keep.append(inst)
            assert not pending, "dangling removed LDWEIGHTS"
            if len(keep) != len(insts):
                blk.instructions = keep
    return removed


def _swap_stt_waits(nc):
    """Post-compile variant of the ACT/STT parallelization: the
    scheduler serialized STT(k) behind the same-step tanh ACT(k) (both
    only READ the z slot) and, by wait-dominance elision, materialized
    ONLY an Activation-sem wait on the STT.  Replace that wait with a
    copy of the paired ACT's own PE wait (the producing matmul), which
    is the STT's true dependency.  This edits only the final SyncInfo
    the hardware executes; scheduling/ticks/queues are untouched."""
    swapped = 0
    for f in nc.m.functions:
        for blk in f.blocks:
            pe_tick = 0
            pe_tmpl = None      # a SyncWait on the PE sem, as template
            mm_tick = {}        # dst slot ap -> PE tick of last writing MM
            for inst in blk.instructions:
                nm = type(inst).__name__
                si = inst.sync_info
                if si is None:
                    continue
                if pe_tmpl is None:
                    for w in si.on_wait:
                        if (w.ant_name or "").startswith("PE_"):
                            pe_tmpl = w
                            break
                for u in si.on_update:
                    if (u.ant_name or "").startswith("PE_"):
                        pe_tick += u.update_value or 1
                if nm == "InstMatmult":
                    mm_tick[str(inst.outs[0])] = pe_tick
                    continue
                if nm != "InstTensorScalarPtr" or pe_tmpl is None:
                    continue
                need = mm_tick.get(str(inst.ins[0]))
                if need is None:
                    continue        # priming phase: keep serialized wait
                new_wait = []
                hit = False
                for w in si.on_wait:
                    if (w.ant_name or "").startswith("Activation"):
                        hit = True
                        new_wait.append(mybir.SyncWait(
                            sync_type=pe_tmpl.sync_type, id=pe_tmpl.id,
                            wait_mode=pe_tmpl.wait_mode,
                            ant_name=pe_tmpl.ant_name, wait_value=need))
                    else:
                        new_wait.append(w)
                if hit:
                    inst.sync_info = mybir.SyncInfo(
                        on_wait=new_wait, on_update=list(si.on_update))
                    swapped += 1
    return swapped


def _parallelize_act_stt(nc):
    """The Tile framework keeps a single linear dependency chain per
    PSUM tile, so the per-step DVE prep (STT) waits for the same-step
    tanh (ACT) even though both only READ the z slot.  Remove each
    STT -> ACT edge where the ACT's input AP equals the STT's in0 AP
    (reader-after-reader on the same slot), merging the ACT's own deps
    (the producing matmul) into the STT."""
    removed = 0
    for f in nc.m.functions:
        for blk in f.blocks:
            by_name = {}
            for inst in blk.instructions:
                by_name[inst.name] = inst
            for inst in blk.instructions:
                if type(inst).__name__ != "InstTensorScalarPtr":
                    continue
                src = str(inst.ins[0])
                for tname, _info in list(inst.sync_dependencies()):
                    dep = by_name.get(tname)
                    if dep is None or type(dep).__name__ != "InstActivation":
                        continue
                    if str(dep.ins[0]) != src:
                        continue
                    inst.remove_dependency(tname)
                    inst.merge_dependencies_from(dep)
                    removed += 1
    return removed


def build_nc(*, iters, cols, groups=4, ch=40, dedup=True,
             filler=0, cut=0, stt_pool=0, num_devices=NCORES):
    nc = bacc.Bacc("TRN2", target_bir_lowering=False, debug=False,
                   num_devices=num_devices)
    # Row stride must not be 8KB-aligned: DMA packets are spread over
    # engines by a source-address hash, and 2^k-aligned row strides make
    # every row hash to the SAME engine (observed: 103-row chunk loads
    # serialized onto one engine at ~31GB/s).  An odd element count
    # cycles row starts through all hash buckets.
    lp = iters * cols + 63
    aps = {
        "m_mat": nc.dram_tensor("m_mat", [K, K], F16,
                                kind="ExternalInput").ap(),
        "c_t": nc.dram_tensor("c_t", [K, lp], F16,
                              kind="ExternalInput").ap(),
        "o_out": nc.dram_tensor("o_out", [NB, lp], F16,
                                kind="ExternalOutput").ap(),
    }
    with tile.TileContext(nc) as tcx:
        emit_scan(tcx, nc, aps, iters=iters, cols=cols, groups=groups, ch=ch,
                  filler=filler, cut=cut, stt_pool=stt_pool)
    if dedup:
        _dedup_ldweights(nc)
        # with a single resident stationary, moving waits onto the one
        # surviving LDWEIGHTS would be wrong — keep waits on matmuls
        nc.move_matmul_waits_to_ldweights = lambda: None
    if cut == 3:
        # run the wait swap inside compile, right before the ISA
        # encodings are generated (post-compile edits to sync_info
        # don't reach the serialized program)
        orig_codegen = nc.codegen_inst_isa_subclasses

        def _patched_codegen():
            _swap_stt_waits(nc)
            orig_codegen()

        nc.codegen_inst_isa_subclasses = _patched_codegen
    nc.compile()
    return nc


def make_m_mat(recurrent_weights, input_weights, output_weights):
    """Device stationary, lhsT [K, K]: columns 0:100 produce the z
    update from the tanh rows (u-rows zeroed: the tanh(o) rhs rows must
    contribute nothing — the u drive arrives via the host C stream);
    columns 100:103 produce the out-IIR increment DT*W_out*th."""
    m = np.zeros((K, K), np.float32)
    m[:N, :N] = DT * recurrent_weights.T
    m[:N, N:] = DT * output_weights.T
    return np.ascontiguousarray(m).astype(np.float16)


def make_mz_host(recurrent_weights, input_weights, output_weights):
    """Host-side C projection [K, K]: rows 0:100 = DT*[W_rec, W_in]
    (the u path lives here), rows 100:103 = DT*[W_out, 0] (the readout
    noise drive)."""
    mz = np.zeros((K, K), np.float32)
    mz[:N, :N] = DT * recurrent_weights
    mz[:N, N:] = DT * input_weights
    mz[N:, :N] = DT * output_weights
    return mz


def make_v(inputs, noise, *, s, warm, iters, cols):
    """v stream [K, iters, cols] f32 for one chain (time shard).

    inputs [cols, T, NB], noise [cols, T, N] (batch-block slices).
    Iteration k=0 is the bootstrap block [0 ; u_{s-warm}/DT]; iteration
    k>=1 covers global step g = s - warm + k - 1 (g<0 -> zeros)."""
    v = np.zeros((K, iters, cols), np.float32)
    g0 = s - warm
    if 0 <= g0 < T:
        v[N:, 0] = inputs[:, g0].T / DT
    for k in range(1, iters):
        g = g0 + k - 1
        if g < 0 or g >= T:
            continue
        v[:N, k] = noise[:, g].T * (NOISE_STD / DT)
        un = inputs[:, g + 1].T if g + 1 < T else 0.0
        v[N:, k] = (un - DECAY * inputs[:, g].T) / DT
    return v


def make_c(v, mz):
    """Host C = Mz_aug v, fp16, [K, iters*cols + 63] (padded rows to
    keep the DRAM row stride off the DMA engine-hash pathology)."""
    k_, it, cols = v.shape
    c = np.zeros((K, it * cols + 63), np.float16)
    c[:, :it * cols] = (mz @ v.reshape(K, it * cols)).astype(np.float16)
    return c


def make_in_maps(inputs, noise, recurrent_weights, input_weights,
                 output_weights, *, groups, iters, plan):
    """Per-core input maps.  Core i runs chains i*groups..(i+1)*groups-1;
    chain q = (bblock, shard) = divmod(q, 2*groups).  The c stream per
    core interleaves its chains' 128-col blocks within each iteration."""
    m = make_m_mat(recurrent_weights, input_weights, output_weights)
    mz = make_mz_host(recurrent_weights, input_weights, output_weights)
    in_maps = []
    for core in range(NCORES):
        vs = []
        for g in range(groups):
            q = core * groups + g
            bb, sh = divmod(q, 2 * groups)
            bsl = slice(bb * GW, (bb + 1) * GW)
            ui = np.ascontiguousarray(inputs[bsl]).astype(np.float32)
            nz = np.ascontiguousarray(noise[bsl]).astype(np.float32)
            s, warm, r = plan[sh]
            vs.append(make_v(ui, nz, s=s, warm=warm, iters=iters, cols=GW))
        v = np.concatenate([vv[:, :, None, :] for vv in vs], axis=2)
        v = v.reshape(K, iters, groups * GW)
        in_maps.append({"m_mat": m, "c_t": make_c(v, mz)})
    return in_maps


def gather_out(results, *, groups, iters, plan):
    """Decode the tanh-encoded device out IIR: out = atanh(o_out)."""
    out = np.empty((B, T, NB), np.float32)
    cols = groups * GW
    for core in range(NCORES):
        o = results[core]["o_out"][:, :iters * cols].astype(np.float32)
        o = np.arctanh(np.clip(o, -0.999999, 0.999999))
        o = o.reshape(NB, iters, groups, GW)
        for g in range(groups):
            q = core * groups + g
            bb, sh = divmod(q, 2 * groups)
            bsl = slice(bb * GW, (bb + 1) * GW)
            s, warm, r = plan[sh]
            # o[:, k] = out IIR of iteration k-1 = global step s-warm+k-2
            out[bsl, s:s + r] = o[:, warm + 2:warm + 2 + r, g].transpose(2, 1, 0)
    return out


_NC_CACHE = {}


def kernel(inputs, noise, recurrent_weights, input_weights, output_weights,
           **run_kwargs):
    cfg = dict(run_kwargs.pop("cfg", {"filler": 0}))
    groups = cfg.setdefault("groups", 4)
    plan, iters = plan_for(groups)
    cfg.setdefault("iters", iters)
    cfg.setdefault("cols", groups * GW)
    key = tuple(sorted(cfg.items()))
    if key not in _NC_CACHE:
        _NC_CACHE[key] = build_nc(**cfg)
    nc = _NC_CACHE[key]
    in_maps = make_in_maps(inputs, noise, recurrent_weights,
                           input_weights, output_weights, groups=groups,
                           iters=cfg["iters"], plan=plan)
    res = run_bass_kernel_spmd(nc, in_maps, core_ids=list(range(NCORES)),
                               **run_kwargs)
    out = gather_out(res.results, groups=groups,
                     iters=cfg["iters"], plan=plan)
    if run_kwargs.get("trace"):
        return out, res
    return out


# revision 52
# speedup vs baseline: 3.8336x; 1.0076x over previous
"""Trainium2 Bass kernel for the ContinuousRNN problem.

Reference (per batch row b):
    h_0 = 0                               # [N], N=100
    z_t = W_rec h_t + W_in u_t
    h_{t+1} = 0.85 h_t + 0.15 tanh(z_t) + NOISE_STD noise_t
    out_t = W_out h_{t+1}

z-space reformulation (state z_t, N rows):
    z_{t+1} = 0.85 z_t + Mz (th_t + v_t)
      Mz   = 0.15 [W_rec, W_in]                          (100x103)
      th_t = [tanh(z_t) ; 0]
      v_t  = [noise_t*NS/DT ; (u_{t+1}-0.85 u_t)/DT]
    bootstrap (h=0): z_0 = Mz [0 ; u_0/DT] = W_in u_0
    out_t = 0.85 out_{t-1} + 0.15 W_out (tanh(z_t) + noise_t*NS/DT)
    (the out IIR is a linear readout of the device-produced tanh
    stream; it runs on host, exactly mirroring the device recurrence)

Per-core per-step critical path is two hops:
    MM_t -> { ACT tanh (psum->sbuf fp16)  ||  DVE prep z' = 0.85 z + C } -> MM_{t+1}
where MM_{t+1} accumulates (start=False) onto the DVE-prepped psum slot.
That accumulation works because each z bank is primed once by a
start=True matmul (sets the psum has_written bits, which non-PE writes
do not clear).  C_t = Mz v_t is host-precomputed and streamed.  The PE
never reloads weights (redundant LDWEIGHTS are deduped by a post-tile
pass).  The tanh stream drains from SBUF by DMA.

The Tile framework tracks PSUM deps as a single linear chain per tile
(each accessor waits for the previous accessor), which would serialize
ACT and DVE (both read the same z slot).  A post-pass
(_parallelize_act_stt) removes those reader-after-reader edges so ACT
and DVE truly run in parallel.

Sharding: each core runs `groups` independent chains of 128 batch
columns; chains are (batch-block, time-shard) pairs.  With 8 cores and
n chains/core there are 2n time shards over 4 batch blocks.  The RNN
contracts (~0.983/step), so time shards s>0 warm up for L steps from
h=0; shard 0 "warms up" on zero-padded inputs (exactly h=0).  All cores
run the identical SPMD program; host slices each chain's valid range.
"""

import sys

for _p in ("/opt/trn_rl_repo",):
    if _p not in sys.path:
        sys.path.insert(0, _p)

import numpy as np

import concourse.bass as bass
import concourse.bacc as bacc
import concourse.mybir as mybir
from concourse import tile
from concourse.bass_utils import run_bass_kernel_spmd

F32 = mybir.dt.float32
F16 = mybir.dt.float16

N = 100
NB = 3
K = N + NB        # 103 (matmul contraction: tanh rows + zero-padded u rows)
B = 512
T = 2048
NCORES = 8
DT = np.float32(0.15)
NOISE_STD = np.float32(0.015)
DECAY = np.float32(0.85)

GW = 128                # batch columns per chain
L_WARM = 216            # warmup steps for time shards > 0

NQ = 8                  # z slots (2 parity tiles x 4 slots per chain)
NTH = 32                # th ring slots (slack must exceed the o-drain
                        # DMA delay behind in-flight chunk transfers)
DRAIN = 8               # drain period (iters)


def plan_for(groups):
    """Shard plan for `groups` chains/core: S time shards, per-shard
    (start, warm, r), uniform ITERS."""
    S = 2 * groups
    A = -(-(T + (S - 1) * L_WARM) // S)     # ceil
    rs = [A] + [A - L_WARM] * (S - 1)
    excess = sum(rs) - T
    rs[-1] -= excess
    assert rs[-1] > 0
    plan = []
    s = 0
    for j in range(S):
        plan.append((s, 0 if j == 0 else L_WARM, rs[j]))
        s += rs[j]
    assert s == T
    # ITERS = A + 2: one bootstrap iteration up front, plus one trailing
    # iteration so the last output's o-state gets tanh'd and drained
    # (the drained o at iteration k is the out IIR of iteration k-1).
    return plan, A + 2


def emit_scan(tc, nc, aps, *, iters, cols, groups, ch, filler=0, cut=0,
              stt_pool=0):
    """aps: m_mat [K,M] f16 (lhsT, M=K with the 3 readout columns),
    c_t [M, iters*cols] f16 (iteration-major, host-computed C = Mz v),
    o_out [NB, iters*cols] f16 (the tanh-encoded out IIR).

    The stationary's columns 100:103 compute the out IIR readout
    p = DT*W_out*th into psum rows 100:103 of the same z slot; the STT
    applies the same 0.85 decay (out IIR == z decay), and the per-step
    tanh ACT covers rows 0:103, so tanh(out_t) rides the tht ring for
    free (tanh is invertible; host applies atanh).  The stationary's
    u-rows are zeroed so the tanh(o) rhs rows contribute nothing.

    PSUM dependency tracking is per-tile, so z state is split into
    per-(group x parity) psum tiles.  C streams through SBUF (walrus
    rejects TensorScalarPtr with all-PSUM operands, and SBUF tiles get
    fine-grained dep tracking)."""
    gw = cols // groups
    mult = mybir.AluOpType.mult
    add = mybir.AluOpType.add
    tanh = mybir.ActivationFunctionType.Tanh
    hq = NQ // 2           # z slots per parity tile

    cpool = tc.alloc_tile_pool(name="const", bufs=1)
    vpool = tc.alloc_tile_pool(name="cstream", bufs=3)
    tpool = tc.alloc_tile_pool(name="th", bufs=1)
    ppool = tc.alloc_tile_pool(name="psum", bufs=1, space="PSUM")

    wb = cpool.tile([K, K], F16, name="wb")
    nc.sync.dma_start(wb[:, :], aps["m_mat"][:, :])

    zt = cpool.tile([K, 512], F16, name="zt")   # zero rhs for priming
    nc.vector.memset(zt[:, :], 0.0)

    # z state: per (group, parity) psum tiles, hq slots of [K, gw] each
    qts = [[ppool.tile([128, hq * gw], F32, name=f"qt{g}p{par}")
            for par in range(2)] for g in range(groups)]
    # scratch bank for PE-warming filler matmuls
    fts = ppool.tile([128, 512], F32, name="fts") if filler else None

    # th ring (fp16); rows 0:100 tanh(z), rows 100:103 tanh(out IIR)
    tht = tpool.tile([K, NTH * cols], F16, name="tht")
    nc.vector.memset(tht[96:K, :], 0.0)

    # prime z tiles: start=True matmuls set has_written over all z slots
    for g in range(groups):
        for par in range(2):
            w = hq * gw
            assert w <= 512
            nc.tensor.matmul(qts[g][par][0:K, 0:w], wb[:, :],
                             zt[:, 0:w], start=True, stop=True)

    prev_mm = [None] * groups
    # C staging, triple buffered.  The first chunks are SMALL so the
    # scan starts as soon as ~1MB lands instead of waiting for a cold
    # 4MB transfer (observed 17-27us startup stalls otherwise).
    lens = [min(10, ch)] * 4
    while sum(lens) < iters:
        lens.append(ch)
    starts = [0]
    for ln in lens:
        starts.append(starts[-1] + ln)
    idx_of = []
    for ci, ln in enumerate(lens):
        idx_of += [ci] * ln
    ctiles = {}

    def c_chunk(ci):
        if ci >= len(lens) or starts[ci] >= iters:
            return None
        if ci not in ctiles:
            tl = vpool.tile([K, ch * cols], F16, tag="cs", name=f"cs{ci}")
            hi = min(starts[ci + 1], iters)
            w = (hi - starts[ci]) * cols
            # split rows 0:100 / 100:103: the HWDGE spreads a DMA's
            # rows across engines only for round row counts (100 -> 10
            # engines, 3 -> 3); a 103-row DMA lands on ONE engine and
            # serializes the whole chunk load at ~31GB/s.
            nc.sync.dma_start(tl[0:N, 0:w],
                              aps["c_t"][0:N, starts[ci] * cols:hi * cols])
            nc.sync.dma_start(tl[N:K, 0:w],
                              aps["c_t"][N:K, starts[ci] * cols:hi * cols])
            ctiles[ci] = tl
        return ctiles[ci]

    c_chunk(0)
    c_chunk(1)

    def drain(k_lo, k_hi):
        """DMA the tanh(out) rows for iterations k_lo..k_hi (inclusive,
        contiguous in the ring) to DRAM."""
        c0 = (k_lo % NTH) * cols
        c1 = c0 + (k_hi - k_lo + 1) * cols
        nc.sync.dma_start(aps["o_out"][:, k_lo * cols:k_lo * cols + c1 - c0],
                          tht[N:K, c0:c1])

    for k in range(iters):
        ci = idx_of[k]
        if k == starts[ci]:
            c_chunk(ci + 2)   # two chunks ahead (triple buffered)
        cc = (k - starts[ci]) * cols
        ctile = ctiles[ci]

        qs = ((k // 2) % hq) * gw       # read slot col (parity k%2)
        qn = (((k + 1) // 2) % hq) * gw  # write slot col (parity (k+1)%2)
        tc0 = (k % NTH) * cols
        for g in range(groups):
            rd = qts[g][k % 2]
            wr = qts[g][(k + 1) % 2]
            # ACT: th = tanh(z | o) psum -> sbuf fp16
            act_inst = nc.scalar.activation(
                tht[0:K, tc0 + g * gw:tc0 + (g + 1) * gw],
                rd[0:K, qs:qs + gw], tanh)
            # prep: z' = 0.85 z + C  (psum+sbuf -> psum, other parity);
            # on DVE by default, optionally on the idle Pool engine
            # (1.2GHz vs DVE 0.96GHz) to shorten the serial chain
            eng = nc.gpsimd if stt_pool else nc.vector
            stt_inst = eng.scalar_tensor_tensor(
                wr[0:K, qn:qn + gw], rd[0:K, qs:qs + gw],
                float(DECAY), ctile[0:K, cc + g * gw:cc + (g + 1) * gw],
                mult, add)
            # The tile framework keeps a single linear dependency chain
            # per PSUM tile, so the STT would wait for the same-step
            # tanh even though both only READ the z slot.  Cut that
            # reader-after-reader edge (inline: sem waits are assigned
            # from these edges at TileContext exit), inheriting the
            # ACT's own deps (the producing matmul).
            if cut == 1:
                # cut the edge, inherit the ACT's own deps
                ai, si = act_inst.ins, stt_inst.ins
                for tname, _info in list(si.sync_dependencies()):
                    if tname == ai.name:
                        si.remove_dependency(tname)
                        si.merge_dependencies_from(ai)
            elif cut == 2:
                # cut the edge, add a clean edge to the producing matmul
                ai, si = act_inst.ins, stt_inst.ins
                for tname, info in list(si.sync_dependencies()):
                    if tname == ai.name:
                        si.remove_dependency(tname)
                        if prev_mm[g] is not None:
                            si.add_dependency(prev_mm[g].ins.name, info)
            # chain MM accumulates onto the prepped slot
            prev_mm[g] = nc.tensor.matmul(
                wr[0:K, qn:qn + gw], wb[:, :],
                tht[0:K, tc0 + g * gw:tc0 + (g + 1) * gw],
                start=False, stop=True, skip_group_check=True)
        if filler:
            # keep the PE pipeline warm with a throwaway matmul
            nc.tensor.matmul(fts[0:K, 0:filler], wb[:, :], zt[:, 0:filler],
                             start=True, stop=True)

        if k % DRAIN == DRAIN - 1:
            drain(k - DRAIN + 1, k)
    # tail
    rem = iters % DRAIN
    if rem:
        drain(iters - rem, iters - 1)

    for p in (ppool, tpool, vpool, cpool):
        p.release()


def _dedup_ldweights(nc):
    """Remove legalizer-inserted LDWEIGHTS that reload an identical
    stationary; merge their deps into the following matmul."""
    removed = 0
    for f in nc.m.functions:
        for blk in f.blocks:
            insts = list(blk.instructions)
            last_key = None
            keep = []
            pending = []
            for inst in insts:
                nm = type(inst).__name__
                if nm == "InstLdweights":
                    key = (str(inst.ins[0]), str(inst.tile_position),
                           str(inst.perf_mode), bool(inst.is_transpose))
                    if key == last_key:
                        pending.append(inst)
                        removed += 1
                        continue
                    last_key = key
                    keep.append(inst)
                elif nm == "InstMatmult":
                    for ld in pending:
                        inst.merge_dependencies_from(ld)
                    pending = []
                    keep.append(inst)
                else:
                    keep.append(inst)
            assert not pending, "dangling removed LDWEIGHTS"
            if len(keep) != len(insts):
                blk.instructions = keep
    return removed


def _swap_stt_waits(nc):
    """Post-compile variant of the ACT/STT parallelization: the
    scheduler serialized STT(k) behind the same-step tanh ACT(k) (both
    only READ the z slot) and, by wait-dominance elision, materialized
    ONLY an Activation-sem wait on the STT.  Replace that wait with a
    copy of the paired ACT's own PE wait (the producing matmul), which
    is the STT's true dependency.  This edits only the final SyncInfo
    the hardware executes; scheduling/ticks/queues are untouched."""
    swapped = 0
    for f in nc.m.functions:
        for blk in f.blocks:
            pe_tick = 0
            pe_tmpl = None      # a SyncWait on the PE sem, as template
            mm_tick = {}        # dst slot ap -> PE tick of last writing MM
            for inst in blk.instructions:
                nm = type(inst).__name__
                si = inst.sync_info
                if si is None:
                    continue
                if pe_tmpl is None:
                    for w in si.on_wait:
                        if (w.ant_name or "").startswith("PE_"):
                            pe_tmpl = w
                            break
                for u in si.on_update:
                    if (u.ant_name or "").startswith("PE_"):
                        pe_tick += u.update_value or 1
                if nm == "InstMatmult":
                    mm_tick[str(inst.outs[0])] = pe_tick
                    continue
                if nm != "InstTensorScalarPtr" or pe_tmpl is None:
                    continue
                need = mm_tick.get(str(inst.ins[0]))
                if need is None:
                    continue        # priming phase: keep serialized wait
                new_wait = []
                hit = False
                for w in si.on_wait:
                    if (w.ant_name or "").startswith("Activation"):
                        hit = True
                        new_wait.append(mybir.SyncWait(
                            sync_type=pe_tmpl.sync_type, id=pe_tmpl.id,
                            wait_mode=pe_tmpl.wait_mode,
                            ant_name=pe_tmpl.ant_name, wait_value=need))
                    else:
                        new_wait.append(w)
                if hit:
                    inst.sync_info = mybir.SyncInfo(
                        on_wait=new_wait, on_update=list(si.on_update))
                    swapped += 1
    return swapped


def _parallelize_act_stt(nc):
    """The Tile framework keeps a single linear dependency chain per
    PSUM tile, so the per-step DVE prep (STT) waits for the same-step
    tanh (ACT) even though both only READ the z slot.  Remove each
    STT -> ACT edge where the ACT's input AP equals the STT's in0 AP
    (reader-after-reader on the same slot), merging the ACT's own deps
    (the producing matmul) into the STT."""
    removed = 0
    for f in nc.m.functions:
        for blk in f.blocks:
            by_name = {}
            for inst in blk.instructions:
                by_name[inst.name] = inst
            for inst in blk.instructions:
                if type(inst).__name__ != "InstTensorScalarPtr":
                    continue
                src = str(inst.ins[0])
                for tname, _info in list(inst.sync_dependencies()):
                    dep = by_name.get(tname)
                    if dep is None or type(dep).__name__ != "InstActivation":
                        continue
                    if str(dep.ins[0]) != src:
                        continue
                    inst.remove_dependency(tname)
                    inst.merge_dependencies_from(dep)
                    removed += 1
    return removed


def build_nc(*, iters, cols, groups=4, ch=40, dedup=True,
             filler=0, cut=0, stt_pool=0, num_devices=NCORES):
    nc = bacc.Bacc("TRN2", target_bir_lowering=False, debug=False,
                   num_devices=num_devices)
    # Row stride must not be 8KB-aligned: DMA packets are spread over
    # engines by a source-address hash, and 2^k-aligned row strides make
    # every row hash to the SAME engine (observed: 103-row chunk loads
    # serialized onto one engine at ~31GB/s).  An odd element count
    # cycles row starts through all hash buckets.
    lp = iters * cols + 63
    aps = {
        "m_mat": nc.dram_tensor("m_mat", [K, K], F16,
                                kind="ExternalInput").ap(),
        "c_t": nc.dram_tensor("c_t", [K, lp], F16,
                              kind="ExternalInput").ap(),
        "o_out": nc.dram_tensor("o_out", [NB, lp], F16,
                                kind="ExternalOutput").ap(),
    }
    with tile.TileContext(nc) as tcx:
        emit_scan(tcx, nc, aps, iters=iters, cols=cols, groups=groups, ch=ch,
                  filler=filler, cut=cut, stt_pool=stt_pool)
    if dedup:
        _dedup_ldweights(nc)
        # with a single resident stationary, moving waits onto the one
        # surviving LDWEIGHTS would be wrong — keep waits on matmuls
        nc.move_matmul_waits_to_ldweights = lambda: None
    if cut == 3:
        # run the wait swap inside compile, right before the ISA
        # encodings are generated (post-compile edits to sync_info
        # don't reach the serialized program)
        orig_codegen = nc.codegen_inst_isa_subclasses

        def _patched_codegen():
            _swap_stt_waits(nc)
            orig_codegen()

        nc.codegen_inst_isa_subclasses = _patched_codegen
    nc.compile()
    return nc


def make_m_mat(recurrent_weights, input_weights, output_weights):
    """Device stationary, lhsT [K, K]: columns 0:100 produce the z
    update from the tanh rows (u-rows zeroed: the tanh(o) rhs rows must
    contribute nothing — the u drive arrives via the host C stream);
    columns 100:103 produce the out-IIR increment DT*W_out*th."""
    m = np.zeros((K, K), np.float32)
    m[:N, :N] = DT * recurrent_weights.T
    m[:N, N:] = DT * output_weights.T
    return np.ascontiguousarray(m).astype(np.float16)


def make_mz_host(recurrent_weights, input_weights, output_weights):
    """Host-side C projection [K, K]: rows 0:100 = DT*[W_rec, W_in]
    (the u path lives here), rows 100:103 = DT*[W_out, 0] (the readout
    noise drive)."""
    mz = np.zeros((K, K), np.float32)
    mz[:N, :N] = DT * recurrent_weights
    mz[:N, N:] = DT * input_weights
    mz[N:, :N] = DT * output_weights
    return mz


def make_v(inputs, noise, *, s, warm, iters, cols):
    """v stream [K, iters, cols] f32 for one chain (time shard).

    inputs [cols, T, NB], noise [cols, T, N] (batch-block slices).
    Iteration k=0 is the bootstrap block [0 ; u_{s-warm}/DT]; iteration
    k>=1 covers global step g = s - warm + k - 1 (g<0 -> zeros)."""
    v = np.zeros((K, iters, cols), np.float32)
    g0 = s - warm
    if 0 <= g0 < T:
        v[N:, 0] = inputs[:, g0].T / DT
    for k in range(1, iters):
        g = g0 + k - 1
        if g < 0 or g >= T:
            continue
        v[:N, k] = noise[:, g].T * (NOISE_STD / DT)
        un = inputs[:, g + 1].T if g + 1 < T else 0.0
        v[N:, k] = (un - DECAY * inputs[:, g].T) / DT
    return v


def make_c(v, mz):
    """Host C = Mz_aug v, fp16, [K, iters*cols + 63] (padded rows to
    keep the DRAM row stride off the DMA engine-hash pathology)."""
    k_, it, cols = v.shape
    c = np.zeros((K, it * cols + 63), np.float16)
    c[:, :it * cols] = (mz @ v.reshape(K, it * cols)).astype(np.float16)
    return c


def make_in_maps(inputs, noise, recurrent_weights, input_weights,
                 output_weights, *, groups, iters, plan):
    """Per-core input maps.  Core i runs chains i*groups..(i+1)*groups-1;
    chain q = (bblock, shard) = divmod(q, 2*groups).  The c stream per
    core interleaves its chains' 128-col blocks within each iteration."""
    m = make_m_mat(recurrent_weights, input_weights, output_weights)
    mz = make_mz_host(recurrent_weights, input_weights, output_weights)
    in_maps = []
    for core in range(NCORES):
        vs = []
        for g in range(groups):
            q = core * groups + g
            bb, sh = divmod(q, 2 * groups)
            bsl = slice(bb * GW, (bb + 1) * GW)
            ui = np.ascontiguousarray(inputs[bsl]).astype(np.float32)
            nz = np.ascontiguousarray(noise[bsl]).astype(np.float32)
            s, warm, r = plan[sh]
            vs.append(make_v(ui, nz, s=s, warm=warm, iters=iters, cols=GW))
        v = np.concatenate([vv[:, :, None, :] for vv in vs], axis=2)
        v = v.reshape(K, iters, groups * GW)
        in_maps.append({"m_mat": m, "c_t": make_c(v, mz)})
    return in_maps


def gather_out(results, *, groups, iters, plan):
    """Decode the tanh-encoded device out IIR: out = atanh(o_out)."""
    out = np.empty((B, T, NB), np.float32)
    cols = groups * GW
    for core in range(NCORES):
        o = results[core]["o_out"][:, :iters * cols].astype(np.float32)
        o = np.arctanh(np.clip(o, -0.999999, 0.999999))
        o = o.reshape(NB, iters, groups, GW)
        for g in range(groups):
            q = core * groups + g
            bb, sh = divmod(q, 2 * groups)
            bsl = slice(bb * GW, (bb + 1) * GW)
            s, warm, r = plan[sh]
            # o[:, k] = out IIR of iteration k-1 = global step s-warm+k-2
            out[bsl, s:s + r] = o[:, warm + 2:warm + 2 + r, g].transpose(2, 1, 0)
    return out


_NC_CACHE = {}


def kernel(inputs, noise, recurrent_weights, input_weights, output_weights,
           **run_kwargs):
    cfg = dict(run_kwargs.pop("cfg", {"filler": 0}))
    groups = cfg.setdefault("groups", 4)
    plan, iters = plan_for(groups)
    cfg.setdefault("iters", iters)
    cfg.setdefault("cols", groups * GW)
    key = tuple(sorted(cfg.items()))
    if key not in _NC_CACHE:
        _NC_CACHE[key] = build_nc(**cfg)
    nc = _NC_CACHE[key]
    in_maps = make_in_maps(inputs, noise, recurrent_weights,
                           input_weights, output_weights, groups=groups,
                           iters=cfg["iters"], plan=plan)
    res = run_bass_kernel_spmd(nc, in_maps, core_ids=list(range(NCORES)),
                               **run_kwargs)
    out = gather_out(res.results, groups=groups,
                     iters=cfg["iters"], plan=plan)
    if run_kwargs.get("trace"):
        return out, res
    return out


# revision 53
# speedup vs baseline: 3.8635x; 1.0078x over previous
"""Trainium2 Bass kernel for the ContinuousRNN problem.

Reference (per batch row b):
    h_0 = 0                               # [N], N=100
    z_t = W_rec h_t + W_in u_t
    h_{t+1} = 0.85 h_t + 0.15 tanh(z_t) + NOISE_STD noise_t
    out_t = W_out h_{t+1}

z-space reformulation (state z_t, N rows):
    z_{t+1} = 0.85 z_t + Mz (th_t + v_t)
      Mz   = 0.15 [W_rec, W_in]                          (100x103)
      th_t = [tanh(z_t) ; 0]
      v_t  = [noise_t*NS/DT ; (u_{t+1}-0.85 u_t)/DT]
    bootstrap (h=0): z_0 = Mz [0 ; u_0/DT] = W_in u_0
    out_t = 0.85 out_{t-1} + 0.15 W_out (tanh(z_t) + noise_t*NS/DT)
    (the out IIR is a linear readout of the device-produced tanh
    stream; it runs on host, exactly mirroring the device recurrence)

Per-core per-step critical path is two hops:
    MM_t -> { ACT tanh (psum->sbuf fp16)  ||  DVE prep z' = 0.85 z + C } -> MM_{t+1}
where MM_{t+1} accumulates (start=False) onto the DVE-prepped psum slot.
That accumulation works because each z bank is primed once by a
start=True matmul (sets the psum has_written bits, which non-PE writes
do not clear).  C_t = Mz v_t is host-precomputed and streamed.  The PE
never reloads weights (redundant LDWEIGHTS are deduped by a post-tile
pass).  The tanh stream drains from SBUF by DMA.

The Tile framework tracks PSUM deps as a single linear chain per tile
(each accessor waits for the previous accessor), which would serialize
ACT and DVE (both read the same z slot).  A post-pass
(_parallelize_act_stt) removes those reader-after-reader edges so ACT
and DVE truly run in parallel.

Sharding: each core runs `groups` independent chains of 128 batch
columns; chains are (batch-block, time-shard) pairs.  With 8 cores and
n chains/core there are 2n time shards over 4 batch blocks.  The RNN
contracts (~0.983/step), so time shards s>0 warm up for L steps from
h=0; shard 0 "warms up" on zero-padded inputs (exactly h=0).  All cores
run the identical SPMD program; host slices each chain's valid range.
"""

import sys

for _p in ("/opt/trn_rl_repo",):
    if _p not in sys.path:
        sys.path.insert(0, _p)

import numpy as np

import concourse.bass as bass
import concourse.bacc as bacc
import concourse.mybir as mybir
from concourse import tile
from concourse.bass_utils import run_bass_kernel_spmd

F32 = mybir.dt.float32
F16 = mybir.dt.float16

N = 100
NB = 3
K = N + NB        # 103 (matmul contraction: tanh rows + zero-padded u rows)
B = 512
T = 2048
NCORES = 8
DT = np.float32(0.15)
NOISE_STD = np.float32(0.015)
DECAY = np.float32(0.85)

GW = 128                # batch columns per chain
L_WARM = 208            # warmup steps for time shards > 0

NQ = 8                  # z slots (2 parity tiles x 4 slots per chain)
NTH = 32                # th ring slots (slack must exceed the o-drain
                        # DMA delay behind in-flight chunk transfers)
DRAIN = 8               # drain period (iters)


def plan_for(groups):
    """Shard plan for `groups` chains/core: S time shards, per-shard
    (start, warm, r), uniform ITERS."""
    S = 2 * groups
    A = -(-(T + (S - 1) * L_WARM) // S)     # ceil
    rs = [A] + [A - L_WARM] * (S - 1)
    excess = sum(rs) - T
    rs[-1] -= excess
    assert rs[-1] > 0
    plan = []
    s = 0
    for j in range(S):
        plan.append((s, 0 if j == 0 else L_WARM, rs[j]))
        s += rs[j]
    assert s == T
    # ITERS = A + 2: one bootstrap iteration up front, plus one trailing
    # iteration so the last output's o-state gets tanh'd and drained
    # (the drained o at iteration k is the out IIR of iteration k-1).
    return plan, A + 2


def emit_scan(tc, nc, aps, *, iters, cols, groups, ch, filler=0, cut=0,
              stt_pool=0):
    """aps: m_mat [K,M] f16 (lhsT, M=K with the 3 readout columns),
    c_t [M, iters*cols] f16 (iteration-major, host-computed C = Mz v),
    o_out [NB, iters*cols] f16 (the tanh-encoded out IIR).

    The stationary's columns 100:103 compute the out IIR readout
    p = DT*W_out*th into psum rows 100:103 of the same z slot; the STT
    applies the same 0.85 decay (out IIR == z decay), and the per-step
    tanh ACT covers rows 0:103, so tanh(out_t) rides the tht ring for
    free (tanh is invertible; host applies atanh).  The stationary's
    u-rows are zeroed so the tanh(o) rhs rows contribute nothing.

    PSUM dependency tracking is per-tile, so z state is split into
    per-(group x parity) psum tiles.  C streams through SBUF (walrus
    rejects TensorScalarPtr with all-PSUM operands, and SBUF tiles get
    fine-grained dep tracking)."""
    gw = cols // groups
    mult = mybir.AluOpType.mult
    add = mybir.AluOpType.add
    tanh = mybir.ActivationFunctionType.Tanh
    hq = NQ // 2           # z slots per parity tile

    cpool = tc.alloc_tile_pool(name="const", bufs=1)
    vpool = tc.alloc_tile_pool(name="cstream", bufs=3)
    tpool = tc.alloc_tile_pool(name="th", bufs=1)
    ppool = tc.alloc_tile_pool(name="psum", bufs=1, space="PSUM")

    wb = cpool.tile([K, K], F16, name="wb")
    nc.sync.dma_start(wb[:, :], aps["m_mat"][:, :])

    zt = cpool.tile([K, 512], F16, name="zt")   # zero rhs for priming
    nc.vector.memset(zt[:, :], 0.0)

    # z state: per (group, parity) psum tiles, hq slots of [K, gw] each
    qts = [[ppool.tile([128, hq * gw], F32, name=f"qt{g}p{par}")
            for par in range(2)] for g in range(groups)]
    # scratch bank for PE-warming filler matmuls
    fts = ppool.tile([128, 512], F32, name="fts") if filler else None

    # th ring (fp16); rows 0:100 tanh(z), rows 100:103 tanh(out IIR)
    tht = tpool.tile([K, NTH * cols], F16, name="tht")
    nc.vector.memset(tht[96:K, :], 0.0)

    # prime z tiles: start=True matmuls set has_written over all z slots
    for g in range(groups):
        for par in range(2):
            w = hq * gw
            assert w <= 512
            nc.tensor.matmul(qts[g][par][0:K, 0:w], wb[:, :],
                             zt[:, 0:w], start=True, stop=True)

    prev_mm = [None] * groups
    # C staging, triple buffered.  The first chunks are SMALL so the
    # scan starts as soon as ~1MB lands instead of waiting for a cold
    # 4MB transfer (observed 17-27us startup stalls otherwise).
    lens = [min(10, ch)] * 4
    while sum(lens) < iters:
        lens.append(ch)
    starts = [0]
    for ln in lens:
        starts.append(starts[-1] + ln)
    idx_of = []
    for ci, ln in enumerate(lens):
        idx_of += [ci] * ln
    ctiles = {}

    def c_chunk(ci):
        if ci >= len(lens) or starts[ci] >= iters:
            return None
        if ci not in ctiles:
            tl = vpool.tile([K, ch * cols], F16, tag="cs", name=f"cs{ci}")
            hi = min(starts[ci + 1], iters)
            w = (hi - starts[ci]) * cols
            # split rows 0:100 / 100:103: the HWDGE spreads a DMA's
            # rows across engines only for round row counts (100 -> 10
            # engines, 3 -> 3); a 103-row DMA lands on ONE engine and
            # serializes the whole chunk load at ~31GB/s.
            nc.sync.dma_start(tl[0:N, 0:w],
                              aps["c_t"][0:N, starts[ci] * cols:hi * cols])
            nc.sync.dma_start(tl[N:K, 0:w],
                              aps["c_t"][N:K, starts[ci] * cols:hi * cols])
            ctiles[ci] = tl
        return ctiles[ci]

    c_chunk(0)
    c_chunk(1)

    def drain(k_lo, k_hi):
        """DMA the tanh(out) rows for iterations k_lo..k_hi (inclusive,
        contiguous in the ring) to DRAM."""
        c0 = (k_lo % NTH) * cols
        c1 = c0 + (k_hi - k_lo + 1) * cols
        nc.sync.dma_start(aps["o_out"][:, k_lo * cols:k_lo * cols + c1 - c0],
                          tht[N:K, c0:c1])

    for k in range(iters):
        ci = idx_of[k]
        if k == starts[ci]:
            c_chunk(ci + 2)   # two chunks ahead (triple buffered)
        cc = (k - starts[ci]) * cols
        ctile = ctiles[ci]

        qs = ((k // 2) % hq) * gw       # read slot col (parity k%2)
        qn = (((k + 1) // 2) % hq) * gw  # write slot col (parity (k+1)%2)
        tc0 = (k % NTH) * cols
        for g in range(groups):
            rd = qts[g][k % 2]
            wr = qts[g][(k + 1) % 2]
            # ACT: th = tanh(z | o) psum -> sbuf fp16
            act_inst = nc.scalar.activation(
                tht[0:K, tc0 + g * gw:tc0 + (g + 1) * gw],
                rd[0:K, qs:qs + gw], tanh)
            # prep: z' = 0.85 z + C  (psum+sbuf -> psum, other parity);
            # on DVE by default, optionally on the idle Pool engine
            # (1.2GHz vs DVE 0.96GHz) to shorten the serial chain
            eng = nc.gpsimd if stt_pool else nc.vector
            stt_inst = eng.scalar_tensor_tensor(
                wr[0:K, qn:qn + gw], rd[0:K, qs:qs + gw],
                float(DECAY), ctile[0:K, cc + g * gw:cc + (g + 1) * gw],
                mult, add)
            # The tile framework keeps a single linear dependency chain
            # per PSUM tile, so the STT would wait for the same-step
            # tanh even though both only READ the z slot.  Cut that
            # reader-after-reader edge (inline: sem waits are assigned
            # from these edges at TileContext exit), inheriting the
            # ACT's own deps (the producing matmul).
            if cut == 1:
                # cut the edge, inherit the ACT's own deps
                ai, si = act_inst.ins, stt_inst.ins
                for tname, _info in list(si.sync_dependencies()):
                    if tname == ai.name:
                        si.remove_dependency(tname)
                        si.merge_dependencies_from(ai)
            elif cut == 2:
                # cut the edge, add a clean edge to the producing matmul
                ai, si = act_inst.ins, stt_inst.ins
                for tname, info in list(si.sync_dependencies()):
                    if tname == ai.name:
                        si.remove_dependency(tname)
                        if prev_mm[g] is not None:
                            si.add_dependency(prev_mm[g].ins.name, info)
            # chain MM accumulates onto the prepped slot
            prev_mm[g] = nc.tensor.matmul(
                wr[0:K, qn:qn + gw], wb[:, :],
                tht[0:K, tc0 + g * gw:tc0 + (g + 1) * gw],
                start=False, stop=True, skip_group_check=True)
        if filler:
            # keep the PE pipeline warm with a throwaway matmul
            nc.tensor.matmul(fts[0:K, 0:filler], wb[:, :], zt[:, 0:filler],
                             start=True, stop=True)

        if k % DRAIN == DRAIN - 1:
            drain(k - DRAIN + 1, k)
    # tail
    rem = iters % DRAIN
    if rem:
        drain(iters - rem, iters - 1)

    for p in (ppool, tpool, vpool, cpool):
        p.release()


def _dedup_ldweights(nc):
    """Remove legalizer-inserted LDWEIGHTS that reload an identical
    stationary; merge their deps into the following matmul."""
    removed = 0
    for f in nc.m.functions:
        for blk in f.blocks:
            insts = list(blk.instructions)
            last_key = None
            keep = []
            pending = []
            for inst in insts:
                nm = type(inst).__name__
                if nm == "InstLdweights":
                    key = (str(inst.ins[0]), str(inst.tile_position),
                           str(inst.perf_mode), bool(inst.is_transpose))
                    if key == last_key:
                        pending.append(inst)
                        removed += 1
                        continue
                    last_key = key
                    keep.append(inst)
                elif nm == "InstMatmult":
                    for ld in pending:
                        inst.merge_dependencies_from(ld)
                    pending = []
                    keep.append(inst)
                else:
                    keep.append(inst)
            assert not pending, "dangling removed LDWEIGHTS"
            if len(keep) != len(insts):
                blk.instructions = keep
    return removed


def _swap_stt_waits(nc):
    """Post-compile variant of the ACT/STT parallelization: the
    scheduler serialized STT(k) behind the same-step tanh ACT(k) (both
    only READ the z slot) and, by wait-dominance elision, materialized
    ONLY an Activation-sem wait on the STT.  Replace that wait with a
    copy of the paired ACT's own PE wait (the producing matmul), which
    is the STT's true dependency.  This edits only the final SyncInfo
    the hardware executes; scheduling/ticks/queues are untouched."""
    swapped = 0
    for f in nc.m.functions:
        for blk in f.blocks:
            pe_tick = 0
            pe_tmpl = None      # a SyncWait on the PE sem, as template
            mm_tick = {}        # dst slot ap -> PE tick of last writing MM
            for inst in blk.instructions:
                nm = type(inst).__name__
                si = inst.sync_info
                if si is None:
                    continue
                if pe_tmpl is None:
                    for w in si.on_wait:
                        if (w.ant_name or "").startswith("PE_"):
                            pe_tmpl = w
                            break
                for u in si.on_update:
                    if (u.ant_name or "").startswith("PE_"):
                        pe_tick += u.update_value or 1
                if nm == "InstMatmult":
                    mm_tick[str(inst.outs[0])] = pe_tick
                    continue
                if nm != "InstTensorScalarPtr" or pe_tmpl is None:
                    continue
                need = mm_tick.get(str(inst.ins[0]))
                if need is None:
                    continue        # priming phase: keep serialized wait
                new_wait = []
                hit = False
                for w in si.on_wait:
                    if (w.ant_name or "").startswith("Activation"):
                        hit = True
                        new_wait.append(mybir.SyncWait(
                            sync_type=pe_tmpl.sync_type, id=pe_tmpl.id,
                            wait_mode=pe_tmpl.wait_mode,
                            ant_name=pe_tmpl.ant_name, wait_value=need))
                    else:
                        new_wait.append(w)
                if hit:
                    inst.sync_info = mybir.SyncInfo(
                        on_wait=new_wait, on_update=list(si.on_update))
                    swapped += 1
    return swapped


def _parallelize_act_stt(nc):
    """The Tile framework keeps a single linear dependency chain per
    PSUM tile, so the per-step DVE prep (STT) waits for the same-step
    tanh (ACT) even though both only READ the z slot.  Remove each
    STT -> ACT edge where the ACT's input AP equals the STT's in0 AP
    (reader-after-reader on the same slot), merging the ACT's own deps
    (the producing matmul) into the STT."""
    removed = 0
    for f in nc.m.functions:
        for blk in f.blocks:
            by_name = {}
            for inst in blk.instructions:
                by_name[inst.name] = inst
            for inst in blk.instructions:
                if type(inst).__name__ != "InstTensorScalarPtr":
                    continue
                src = str(inst.ins[0])
                for tname, _info in list(inst.sync_dependencies()):
                    dep = by_name.get(tname)
                    if dep is None or type(dep).__name__ != "InstActivation":
                        continue
                    if str(dep.ins[0]) != src:
                        continue
                    inst.remove_dependency(tname)
                    inst.merge_dependencies_from(dep)
                    removed += 1
    return removed


def build_nc(*, iters, cols, groups=4, ch=40, dedup=True,
             filler=0, cut=0, stt_pool=0, num_devices=NCORES):
    nc = bacc.Bacc("TRN2", target_bir_lowering=False, debug=False,
                   num_devices=num_devices)
    # Row stride must not be 8KB-aligned: DMA packets are spread over
    # engines by a source-address hash, and 2^k-aligned row strides make
    # every row hash to the SAME engine (observed: 103-row chunk loads
    # serialized onto one engine at ~31GB/s).  An odd element count
    # cycles row starts through all hash buckets.
    lp = iters * cols + 63
    aps = {
        "m_mat": nc.dram_tensor("m_mat", [K, K], F16,
                                kind="ExternalInput").ap(),
        "c_t": nc.dram_tensor("c_t", [K, lp], F16,
                              kind="ExternalInput").ap(),
        "o_out": nc.dram_tensor("o_out", [NB, lp], F16,
                                kind="ExternalOutput").ap(),
    }
    with tile.TileContext(nc) as tcx:
        emit_scan(tcx, nc, aps, iters=iters, cols=cols, groups=groups, ch=ch,
                  filler=filler, cut=cut, stt_pool=stt_pool)
    if dedup:
        _dedup_ldweights(nc)
        # with a single resident stationary, moving waits onto the one
        # surviving LDWEIGHTS would be wrong — keep waits on matmuls
        nc.move_matmul_waits_to_ldweights = lambda: None
    if cut == 3:
        # run the wait swap inside compile, right before the ISA
        # encodings are generated (post-compile edits to sync_info
        # don't reach the serialized program)
        orig_codegen = nc.codegen_inst_isa_subclasses

        def _patched_codegen():
            _swap_stt_waits(nc)
            orig_codegen()

        nc.codegen_inst_isa_subclasses = _patched_codegen
    nc.compile()
    return nc


def make_m_mat(recurrent_weights, input_weights, output_weights):
    """Device stationary, lhsT [K, K]: columns 0:100 produce the z
    update from the tanh rows (u-rows zeroed: the tanh(o) rhs rows must
    contribute nothing — the u drive arrives via the host C stream);
    columns 100:103 produce the out-IIR increment DT*W_out*th."""
    m = np.zeros((K, K), np.float32)
    m[:N, :N] = DT * recurrent_weights.T
    m[:N, N:] = DT * output_weights.T
    return np.ascontiguousarray(m).astype(np.float16)


def make_mz_host(recurrent_weights, input_weights, output_weights):
    """Host-side C projection [K, K]: rows 0:100 = DT*[W_rec, W_in]
    (the u path lives here), rows 100:103 = DT*[W_out, 0] (the readout
    noise drive)."""
    mz = np.zeros((K, K), np.float32)
    mz[:N, :N] = DT * recurrent_weights
    mz[:N, N:] = DT * input_weights
    mz[N:, :N] = DT * output_weights
    return mz


def make_v(inputs, noise, *, s, warm, iters, cols):
    """v stream [K, iters, cols] f32 for one chain (time shard).

    inputs [cols, T, NB], noise [cols, T, N] (batch-block slices).
    Iteration k=0 is the bootstrap block [0 ; u_{s-warm}/DT]; iteration
    k>=1 covers global step g = s - warm + k - 1 (g<0 -> zeros)."""
    v = np.zeros((K, iters, cols), np.float32)
    g0 = s - warm
    if 0 <= g0 < T:
        v[N:, 0] = inputs[:, g0].T / DT
    for k in range(1, iters):
        g = g0 + k - 1
        if g < 0 or g >= T:
            continue
        v[:N, k] = noise[:, g].T * (NOISE_STD / DT)
        un = inputs[:, g + 1].T if g + 1 < T else 0.0
        v[N:, k] = (un - DECAY * inputs[:, g].T) / DT
    return v


def make_c(v, mz):
    """Host C = Mz_aug v, fp16, [K, iters*cols + 63] (padded rows to
    keep the DRAM row stride off the DMA engine-hash pathology)."""
    k_, it, cols = v.shape
    c = np.zeros((K, it * cols + 63), np.float16)
    c[:, :it * cols] = (mz @ v.reshape(K, it * cols)).astype(np.float16)
    return c


def make_in_maps(inputs, noise, recurrent_weights, input_weights,
                 output_weights, *, groups, iters, plan):
    """Per-core input maps.  Core i runs chains i*groups..(i+1)*groups-1;
    chain q = (bblock, shard) = divmod(q, 2*groups).  The c stream per
    core interleaves its chains' 128-col blocks within each iteration."""
    m = make_m_mat(recurrent_weights, input_weights, output_weights)
    mz = make_mz_host(recurrent_weights, input_weights, output_weights)
    in_maps = []
    for core in range(NCORES):
        vs = []
        for g in range(groups):
            q = core * groups + g
            bb, sh = divmod(q, 2 * groups)
            bsl = slice(bb * GW, (bb + 1) * GW)
            ui = np.ascontiguousarray(inputs[bsl]).astype(np.float32)
            nz = np.ascontiguousarray(noise[bsl]).astype(np.float32)
            s, warm, r = plan[sh]
            vs.append(make_v(ui, nz, s=s, warm=warm, iters=iters, cols=GW))
        v = np.concatenate([vv[:, :, None, :] for vv in vs], axis=2)
        v = v.reshape(K, iters, groups * GW)
        in_maps.append({"m_mat": m, "c_t": make_c(v, mz)})
    return in_maps


def gather_out(results, *, groups, iters, plan):
    """Decode the tanh-encoded device out IIR: out = atanh(o_out)."""
    out = np.empty((B, T, NB), np.float32)
    cols = groups * GW
    for core in range(NCORES):
        o = results[core]["o_out"][:, :iters * cols].astype(np.float32)
        o = np.arctanh(np.clip(o, -0.999999, 0.999999))
        o = o.reshape(NB, iters, groups, GW)
        for g in range(groups):
            q = core * groups + g
            bb, sh = divmod(q, 2 * groups)
            bsl = slice(bb * GW, (bb + 1) * GW)
            s, warm, r = plan[sh]
            # o[:, k] = out IIR of iteration k-1 = global step s-warm+k-2
            out[bsl, s:s + r] = o[:, warm + 2:warm + 2 + r, g].transpose(2, 1, 0)
    return out


_NC_CACHE = {}


def kernel(inputs, noise, recurrent_weights, input_weights, output_weights,
           **run_kwargs):
    cfg = dict(run_kwargs.pop("cfg", {"filler": 0}))
    groups = cfg.setdefault("groups", 4)
    plan, iters = plan_for(groups)
    cfg.setdefault("iters", iters)
    cfg.setdefault("cols", groups * GW)
    key = tuple(sorted(cfg.items()))
    if key not in _NC_CACHE:
        _NC_CACHE[key] = build_nc(**cfg)
    nc = _NC_CACHE[key]
    in_maps = make_in_maps(inputs, noise, recurrent_weights,
                           input_weights, output_weights, groups=groups,
                           iters=cfg["iters"], plan=plan)
    res = run_bass_kernel_spmd(nc, in_maps, core_ids=list(range(NCORES)),
                               **run_kwargs)
    out = gather_out(res.results, groups=groups,
                     iters=cfg["iters"], plan=plan)
    if run_kwargs.get("trace"):
        return out, res
    return out


# revision 54
# speedup vs baseline: 4.0299x; 1.0431x over previous
"""Trainium2 Bass kernel for the ContinuousRNN problem.

Reference (per batch row b):
    h_0 = 0                               # [N], N=100
    z_t = W_rec h_t + W_in u_t
    h_{t+1} = 0.85 h_t + 0.15 tanh(z_t) + NOISE_STD noise_t
    out_t = W_out h_{t+1}

z-space reformulation (state z_t, N rows):
    z_{t+1} = 0.85 z_t + Mz (th_t + v_t)
      Mz   = 0.15 [W_rec, W_in]                          (100x103)
      th_t = [tanh(z_t) ; 0]
      v_t  = [noise_t*NS/DT ; (u_{t+1}-0.85 u_t)/DT]
    bootstrap (h=0): z_0 = Mz [0 ; u_0/DT] = W_in u_0
    out_t = 0.85 out_{t-1} + 0.15 W_out (tanh(z_t) + noise_t*NS/DT)
    (the out IIR is a linear readout of the device-produced tanh
    stream; it runs on host, exactly mirroring the device recurrence)

Per-core per-step critical path is two hops:
    MM_t -> { ACT tanh (psum->sbuf fp16)  ||  DVE prep z' = 0.85 z + C } -> MM_{t+1}
where MM_{t+1} accumulates (start=False) onto the DVE-prepped psum slot.
That accumulation works because each z bank is primed once by a
start=True matmul (sets the psum has_written bits, which non-PE writes
do not clear).  C_t = Mz v_t is host-precomputed and streamed.  The PE
never reloads weights (redundant LDWEIGHTS are deduped by a post-tile
pass).  The tanh stream drains from SBUF by DMA.

The Tile framework tracks PSUM deps as a single linear chain per tile
(each accessor waits for the previous accessor), which would serialize
ACT and DVE (both read the same z slot).  A post-pass
(_parallelize_act_stt) removes those reader-after-reader edges so ACT
and DVE truly run in parallel.

Sharding: each core runs `groups` independent chains of 128 batch
columns; chains are (batch-block, time-shard) pairs.  With 8 cores and
n chains/core there are 2n time shards over 4 batch blocks.  The RNN
contracts (~0.983/step), so time shards s>0 warm up for L steps from
h=0; shard 0 "warms up" on zero-padded inputs (exactly h=0).  All cores
run the identical SPMD program; host slices each chain's valid range.
"""

import sys

for _p in ("/opt/trn_rl_repo",):
    if _p not in sys.path:
        sys.path.insert(0, _p)

import numpy as np

import concourse.bass as bass
import concourse.bacc as bacc
import concourse.mybir as mybir
from concourse import tile
from concourse.bass_utils import run_bass_kernel_spmd

F32 = mybir.dt.float32
F16 = mybir.dt.float16

N = 100
NB = 3
K = N + NB        # 103 (matmul contraction: tanh rows + zero-padded u rows)
B = 512
T = 2048
NCORES = 8
DT = np.float32(0.15)
NOISE_STD = np.float32(0.015)
DECAY = np.float32(0.85)

GW = 128                # batch columns per chain
L_WARM = 208            # warmup steps for time shards > 0

NQ = 8                  # z slots (2 parity tiles x 4 slots per chain)
NTH = 32                # th ring slots (slack must exceed the o-drain
                        # DMA delay behind in-flight chunk transfers)
DRAIN = 8               # drain period (iters)


def plan_for(groups):
    """Shard plan for `groups` chains/core: S time shards, per-shard
    (start, warm, r), uniform ITERS."""
    S = 2 * groups
    A = -(-(T + (S - 1) * L_WARM) // S)     # ceil
    rs = [A] + [A - L_WARM] * (S - 1)
    excess = sum(rs) - T
    rs[-1] -= excess
    assert rs[-1] > 0
    plan = []
    s = 0
    for j in range(S):
        plan.append((s, 0 if j == 0 else L_WARM, rs[j]))
        s += rs[j]
    assert s == T
    # ITERS = A + 2: one bootstrap iteration up front, plus one trailing
    # iteration so the last output's o-state gets tanh'd and drained
    # (the drained o at iteration k is the out IIR of iteration k-1).
    return plan, A + 2


def emit_scan(tc, nc, aps, *, iters, cols, groups, ch, filler=0, cut=0,
              stt_pool=0):
    """aps: m_mat [K,M] f16 (lhsT, M=K with the 3 readout columns),
    c_t [M, iters*cols] f16 (iteration-major, host-computed C = Mz v),
    o_out [NB, iters*cols] f16 (the tanh-encoded out IIR).

    The stationary's columns 100:103 compute the out IIR readout
    p = DT*W_out*th into psum rows 100:103 of the same z slot; the STT
    applies the same 0.85 decay (out IIR == z decay), and the per-step
    tanh ACT covers rows 0:103, so tanh(out_t) rides the tht ring for
    free (tanh is invertible; host applies atanh).  The stationary's
    u-rows are zeroed so the tanh(o) rhs rows contribute nothing.

    PSUM dependency tracking is per-tile, so z state is split into
    per-(group x parity) psum tiles.  C streams through SBUF (walrus
    rejects TensorScalarPtr with all-PSUM operands, and SBUF tiles get
    fine-grained dep tracking)."""
    gw = cols // groups
    mult = mybir.AluOpType.mult
    add = mybir.AluOpType.add
    tanh = mybir.ActivationFunctionType.Tanh
    hq = NQ // 2           # z slots per parity tile

    cpool = tc.alloc_tile_pool(name="const", bufs=1)
    vpool = tc.alloc_tile_pool(name="cstream", bufs=3)
    tpool = tc.alloc_tile_pool(name="th", bufs=1)
    ppool = tc.alloc_tile_pool(name="psum", bufs=1, space="PSUM")

    wb = cpool.tile([K, K], F16, name="wb")
    nc.sync.dma_start(wb[:, :], aps["m_mat"][:, :])

    zt = cpool.tile([K, 512], F16, name="zt")   # zero rhs for priming
    nc.vector.memset(zt[:, :], 0.0)

    # z state: per (group, parity) psum tiles, hq slots of [K, gw] each
    qts = [[ppool.tile([128, hq * gw], F32, name=f"qt{g}p{par}")
            for par in range(2)] for g in range(groups)]
    # scratch bank for PE-warming filler matmuls
    fts = ppool.tile([128, 512], F32, name="fts") if filler else None

    # th ring (fp16); rows 0:100 tanh(z), rows 100:103 tanh(out IIR)
    tht = tpool.tile([K, NTH * cols], F16, name="tht")
    nc.vector.memset(tht[96:K, :], 0.0)

    # prime z tiles: start=True matmuls set has_written over all z slots
    for g in range(groups):
        for par in range(2):
            w = hq * gw
            assert w <= 512
            nc.tensor.matmul(qts[g][par][0:K, 0:w], wb[:, :],
                             zt[:, 0:w], start=True, stop=True)

    prev_mm = [None] * groups
    # C staging, triple buffered.  The first chunks are SMALL so the
    # scan starts as soon as ~1MB lands instead of waiting for a cold
    # 4MB transfer (observed 17-27us startup stalls otherwise).
    lens = [min(10, ch)] * 4 + [min(20, ch)] * 4
    while sum(lens) < iters:
        lens.append(ch)
    starts = [0]
    for ln in lens:
        starts.append(starts[-1] + ln)
    idx_of = []
    for ci, ln in enumerate(lens):
        idx_of += [ci] * ln
    ctiles = {}

    def c_chunk(ci):
        if ci >= len(lens) or starts[ci] >= iters:
            return None
        if ci not in ctiles:
            tl = vpool.tile([K, ch * cols], F16, tag="cs", name=f"cs{ci}")
            hi = min(starts[ci + 1], iters)
            w = (hi - starts[ci]) * cols
            # split rows 0:100 / 100:103: the HWDGE spreads a DMA's
            # rows across engines only for round row counts (100 -> 10
            # engines, 3 -> 3); a 103-row DMA lands on ONE engine and
            # serializes the whole chunk load at ~31GB/s.
            nc.sync.dma_start(tl[0:N, 0:w],
                              aps["c_t"][0:N, starts[ci] * cols:hi * cols])
            nc.sync.dma_start(tl[N:K, 0:w],
                              aps["c_t"][N:K, starts[ci] * cols:hi * cols])
            ctiles[ci] = tl
        return ctiles[ci]

    c_chunk(0)
    c_chunk(1)

    def drain(k_lo, k_hi):
        """DMA the tanh(out) rows for iterations k_lo..k_hi (inclusive,
        contiguous in the ring) to DRAM."""
        c0 = (k_lo % NTH) * cols
        c1 = c0 + (k_hi - k_lo + 1) * cols
        nc.sync.dma_start(aps["o_out"][:, k_lo * cols:k_lo * cols + c1 - c0],
                          tht[N:K, c0:c1])

    for k in range(iters):
        ci = idx_of[k]
        if k == starts[ci]:
            c_chunk(ci + 2)   # two chunks ahead (triple buffered)
        cc = (k - starts[ci]) * cols
        ctile = ctiles[ci]

        qs = ((k // 2) % hq) * gw       # read slot col (parity k%2)
        qn = (((k + 1) // 2) % hq) * gw  # write slot col (parity (k+1)%2)
        tc0 = (k % NTH) * cols
        for g in range(groups):
            rd = qts[g][k % 2]
            wr = qts[g][(k + 1) % 2]
            # ACT: th = tanh(z | o) psum -> sbuf fp16
            act_inst = nc.scalar.activation(
                tht[0:K, tc0 + g * gw:tc0 + (g + 1) * gw],
                rd[0:K, qs:qs + gw], tanh)
            # prep: z' = 0.85 z + C  (psum+sbuf -> psum, other parity);
            # on DVE by default, optionally on the idle Pool engine
            # (1.2GHz vs DVE 0.96GHz) to shorten the serial chain
            eng = nc.gpsimd if stt_pool else nc.vector
            stt_inst = eng.scalar_tensor_tensor(
                wr[0:K, qn:qn + gw], rd[0:K, qs:qs + gw],
                float(DECAY), ctile[0:K, cc + g * gw:cc + (g + 1) * gw],
                mult, add)
            # The tile framework keeps a single linear dependency chain
            # per PSUM tile, so the STT would wait for the same-step
            # tanh even though both only READ the z slot.  Cut that
            # reader-after-reader edge (inline: sem waits are assigned
            # from these edges at TileContext exit), inheriting the
            # ACT's own deps (the producing matmul).
            if cut == 1:
                # cut the edge, inherit the ACT's own deps
                ai, si = act_inst.ins, stt_inst.ins
                for tname, _info in list(si.sync_dependencies()):
                    if tname == ai.name:
                        si.remove_dependency(tname)
                        si.merge_dependencies_from(ai)
            elif cut == 2:
                # cut the edge, add a clean edge to the producing matmul
                ai, si = act_inst.ins, stt_inst.ins
                for tname, info in list(si.sync_dependencies()):
                    if tname == ai.name:
                        si.remove_dependency(tname)
                        if prev_mm[g] is not None:
                            si.add_dependency(prev_mm[g].ins.name, info)
            # chain MM accumulates onto the prepped slot
            prev_mm[g] = nc.tensor.matmul(
                wr[0:K, qn:qn + gw], wb[:, :],
                tht[0:K, tc0 + g * gw:tc0 + (g + 1) * gw],
                start=False, stop=True, skip_group_check=True)
        if filler:
            # keep the PE pipeline warm with a throwaway matmul
            nc.tensor.matmul(fts[0:K, 0:filler], wb[:, :], zt[:, 0:filler],
                             start=True, stop=True)

        if k % DRAIN == DRAIN - 1:
            drain(k - DRAIN + 1, k)
    # tail
    rem = iters % DRAIN
    if rem:
        drain(iters - rem, iters - 1)

    for p in (ppool, tpool, vpool, cpool):
        p.release()


def _dedup_ldweights(nc):
    """Remove legalizer-inserted LDWEIGHTS that reload an identical
    stationary; merge their deps into the following matmul."""
    removed = 0
    for f in nc.m.functions:
        for blk in f.blocks:
            insts = list(blk.instructions)
            last_key = None
            keep = []
            pending = []
            for inst in insts:
                nm = type(inst).__name__
                if nm == "InstLdweights":
                    key = (str(inst.ins[0]), str(inst.tile_position),
                           str(inst.perf_mode), bool(inst.is_transpose))
                    if key == last_key:
                        pending.append(inst)
                        removed += 1
                        continue
                    last_key = key
                    keep.append(inst)
                elif nm == "InstMatmult":
                    for ld in pending:
                        inst.merge_dependencies_from(ld)
                    pending = []
                    keep.append(inst)
                else:
                    keep.append(inst)
            assert not pending, "dangling removed LDWEIGHTS"
            if len(keep) != len(insts):
                blk.instructions = keep
    return removed


def _swap_stt_waits(nc):
    """Post-compile variant of the ACT/STT parallelization: the
    scheduler serialized STT(k) behind the same-step tanh ACT(k) (both
    only READ the z slot) and, by wait-dominance elision, materialized
    ONLY an Activation-sem wait on the STT.  Replace that wait with a
    copy of the paired ACT's own PE wait (the producing matmul), which
    is the STT's true dependency.  This edits only the final SyncInfo
    the hardware executes; scheduling/ticks/queues are untouched."""
    swapped = 0
    for f in nc.m.functions:
        for blk in f.blocks:
            pe_tick = 0
            pe_tmpl = None      # a SyncWait on the PE sem, as template
            mm_tick = {}        # dst slot ap -> PE tick of last writing MM
            for inst in blk.instructions:
                nm = type(inst).__name__
                si = inst.sync_info
                if si is None:
                    continue
                if pe_tmpl is None:
                    for w in si.on_wait:
                        if (w.ant_name or "").startswith("PE_"):
                            pe_tmpl = w
                            break
                for u in si.on_update:
                    if (u.ant_name or "").startswith("PE_"):
                        pe_tick += u.update_value or 1
                if nm == "InstMatmult":
                    mm_tick[str(inst.outs[0])] = pe_tick
                    continue
                if nm != "InstTensorScalarPtr" or pe_tmpl is None:
                    continue
                need = mm_tick.get(str(inst.ins[0]))
                if need is None:
                    continue        # priming phase: keep serialized wait
                new_wait = []
                hit = False
                for w in si.on_wait:
                    if (w.ant_name or "").startswith("Activation"):
                        hit = True
                        new_wait.append(mybir.SyncWait(
                            sync_type=pe_tmpl.sync_type, id=pe_tmpl.id,
                            wait_mode=pe_tmpl.wait_mode,
                            ant_name=pe_tmpl.ant_name, wait_value=need))
                    else:
                        new_wait.append(w)
                if hit:
                    inst.sync_info = mybir.SyncInfo(
                        on_wait=new_wait, on_update=list(si.on_update))
                    swapped += 1
    return swapped


def _parallelize_act_stt(nc):
    """The Tile framework keeps a single linear dependency chain per
    PSUM tile, so the per-step DVE prep (STT) waits for the same-step
    tanh (ACT) even though both only READ the z slot.  Remove each
    STT -> ACT edge where the ACT's input AP equals the STT's in0 AP
    (reader-after-reader on the same slot), merging the ACT's own deps
    (the producing matmul) into the STT."""
    removed = 0
    for f in nc.m.functions:
        for blk in f.blocks:
            by_name = {}
            for inst in blk.instructions:
                by_name[inst.name] = inst
            for inst in blk.instructions:
                if type(inst).__name__ != "InstTensorScalarPtr":
                    continue
                src = str(inst.ins[0])
                for tname, _info in list(inst.sync_dependencies()):
                    dep = by_name.get(tname)
                    if dep is None or type(dep).__name__ != "InstActivation":
                        continue
                    if str(dep.ins[0]) != src:
                        continue
                    inst.remove_dependency(tname)
                    inst.merge_dependencies_from(dep)
                    removed += 1
    return removed


def build_nc(*, iters, cols, groups=4, ch=40, dedup=True,
             filler=0, cut=0, stt_pool=0, num_devices=NCORES):
    nc = bacc.Bacc("TRN2", target_bir_lowering=False, debug=False,
                   num_devices=num_devices)
    # Row stride must not be 8KB-aligned: DMA packets are spread over
    # engines by a source-address hash, and 2^k-aligned row strides make
    # every row hash to the SAME engine (observed: 103-row chunk loads
    # serialized onto one engine at ~31GB/s).  An odd element count
    # cycles row starts through all hash buckets.
    lp = iters * cols + 63
    aps = {
        "m_mat": nc.dram_tensor("m_mat", [K, K], F16,
                                kind="ExternalInput").ap(),
        "c_t": nc.dram_tensor("c_t", [K, lp], F16,
                              kind="ExternalInput").ap(),
        "o_out": nc.dram_tensor("o_out", [NB, lp], F16,
                                kind="ExternalOutput").ap(),
    }
    with tile.TileContext(nc) as tcx:
        emit_scan(tcx, nc, aps, iters=iters, cols=cols, groups=groups, ch=ch,
                  filler=filler, cut=cut, stt_pool=stt_pool)
    if dedup:
        _dedup_ldweights(nc)
        # with a single resident stationary, moving waits onto the one
        # surviving LDWEIGHTS would be wrong — keep waits on matmuls
        nc.move_matmul_waits_to_ldweights = lambda: None
    if cut == 3:
        # run the wait swap inside compile, right before the ISA
        # encodings are generated (post-compile edits to sync_info
        # don't reach the serialized program)
        orig_codegen = nc.codegen_inst_isa_subclasses

        def _patched_codegen():
            _swap_stt_waits(nc)
            orig_codegen()

        nc.codegen_inst_isa_subclasses = _patched_codegen
    nc.compile()
    return nc


def make_m_mat(recurrent_weights, input_weights, output_weights):
    """Device stationary, lhsT [K, K]: columns 0:100 produce the z
    update from the tanh rows (u-rows zeroed: the tanh(o) rhs rows must
    contribute nothing — the u drive arrives via the host C stream);
    columns 100:103 produce the out-IIR increment DT*W_out*th."""
    m = np.zeros((K, K), np.float32)
    m[:N, :N] = DT * recurrent_weights.T
    m[:N, N:] = DT * output_weights.T
    return np.ascontiguousarray(m).astype(np.float16)


def make_mz_host(recurrent_weights, input_weights, output_weights):
    """Host-side C projection [K, K]: rows 0:100 = DT*[W_rec, W_in]
    (the u path lives here), rows 100:103 = DT*[W_out, 0] (the readout
    noise drive)."""
    mz = np.zeros((K, K), np.float32)
    mz[:N, :N] = DT * recurrent_weights
    mz[:N, N:] = DT * input_weights
    mz[N:, :N] = DT * output_weights
    return mz


def make_v(inputs, noise, *, s, warm, iters, cols):
    """v stream [K, iters, cols] f32 for one chain (time shard).

    inputs [cols, T, NB], noise [cols, T, N] (batch-block slices).
    Iteration k=0 is the bootstrap block [0 ; u_{s-warm}/DT]; iteration
    k>=1 covers global step g = s - warm + k - 1 (g<0 -> zeros)."""
    v = np.zeros((K, iters, cols), np.float32)
    g0 = s - warm
    if 0 <= g0 < T:
        v[N:, 0] = inputs[:, g0].T / DT
    for k in range(1, iters):
        g = g0 + k - 1
        if g < 0 or g >= T:
            continue
        v[:N, k] = noise[:, g].T * (NOISE_STD / DT)
        un = inputs[:, g + 1].T if g + 1 < T else 0.0
        v[N:, k] = (un - DECAY * inputs[:, g].T) / DT
    return v


def make_c(v, mz):
    """Host C = Mz_aug v, fp16, [K, iters*cols + 63] (padded rows to
    keep the DRAM row stride off the DMA engine-hash pathology)."""
    k_, it, cols = v.shape
    c = np.zeros((K, it * cols + 63), np.float16)
    c[:, :it * cols] = (mz @ v.reshape(K, it * cols)).astype(np.float16)
    return c


def make_in_maps(inputs, noise, recurrent_weights, input_weights,
                 output_weights, *, groups, iters, plan):
    """Per-core input maps.  Core i runs chains i*groups..(i+1)*groups-1;
    chain q = (bblock, shard) = divmod(q, 2*groups).  The c stream per
    core interleaves its chains' 128-col blocks within each iteration."""
    m = make_m_mat(recurrent_weights, input_weights, output_weights)
    mz = make_mz_host(recurrent_weights, input_weights, output_weights)
    in_maps = []
    for core in range(NCORES):
        vs = []
        for g in range(groups):
            q = core * groups + g
            bb, sh = divmod(q, 2 * groups)
            bsl = slice(bb * GW, (bb + 1) * GW)
            ui = np.ascontiguousarray(inputs[bsl]).astype(np.float32)
            nz = np.ascontiguousarray(noise[bsl]).astype(np.float32)
            s, warm, r = plan[sh]
            vs.append(make_v(ui, nz, s=s, warm=warm, iters=iters, cols=GW))
        v = np.concatenate([vv[:, :, None, :] for vv in vs], axis=2)
        v = v.reshape(K, iters, groups * GW)
        in_maps.append({"m_mat": m, "c_t": make_c(v, mz)})
    return in_maps


def gather_out(results, *, groups, iters, plan):
    """Decode the tanh-encoded device out IIR: out = atanh(o_out)."""
    out = np.empty((B, T, NB), np.float32)
    cols = groups * GW
    for core in range(NCORES):
        o = results[core]["o_out"][:, :iters * cols].astype(np.float32)
        o = np.arctanh(np.clip(o, -0.999999, 0.999999))
        o = o.reshape(NB, iters, groups, GW)
        for g in range(groups):
            q = core * groups + g
            bb, sh = divmod(q, 2 * groups)
            bsl = slice(bb * GW, (bb + 1) * GW)
            s, warm, r = plan[sh]
            # o[:, k] = out IIR of iteration k-1 = global step s-warm+k-2
            out[bsl, s:s + r] = o[:, warm + 2:warm + 2 + r, g].transpose(2, 1, 0)
    return out


_NC_CACHE = {}


def kernel(inputs, noise, recurrent_weights, input_weights, output_weights,
           **run_kwargs):
    cfg = dict(run_kwargs.pop("cfg", {"filler": 0}))
    groups = cfg.setdefault("groups", 4)
    plan, iters = plan_for(groups)
    cfg.setdefault("iters", iters)
    cfg.setdefault("cols", groups * GW)
    key = tuple(sorted(cfg.items()))
    if key not in _NC_CACHE:
        _NC_CACHE[key] = build_nc(**cfg)
    nc = _NC_CACHE[key]
    in_maps = make_in_maps(inputs, noise, recurrent_weights,
                           input_weights, output_weights, groups=groups,
                           iters=cfg["iters"], plan=plan)
    res = run_bass_kernel_spmd(nc, in_maps, core_ids=list(range(NCORES)),
                               **run_kwargs)
    out = gather_out(res.results, groups=groups,
                     iters=cfg["iters"], plan=plan)
    if run_kwargs.get("trace"):
        return out, res
    return out
